# revision 5
# baseline (speedup 1.0000x reference)
"""Causal self-attention Trainium2 Bass kernel, v4.

B=4, T=2048, C=2048, H=16, D=128, fp32 I/O. DP=4 x TP=2 (Megatron
head-group split); host sums TP pairs.

Per-core structure:
  [QK proj h0] [V proj] [fused: attn(h) + QK proj(h+1)] x8 [out proj]

- f16 activations/weights, fp32 PSUM accumulation everywhere.
- Fully SBUF-resident intermediates; q/k tiles rotate (lifetime ~2 head
  slots), wqk weight tiles stream with bufs=4, wp streams per 512-col
  block during the cb-outer output projection.
- Attention: score pairs packed into [128,1024] PSUM tiles -> one exp
  per pair; exp tiles merged into per-si accumulators S_a (DVE, even
  pairs) / S_b (gpsimd, odd pairs); per-si two ones-matmuls reduce them
  into sum[1,512]; reciprocal -> partition_broadcast -> normalize fused
  into the av PSUM->SBUF evacuation.
- QK projection of head h+1 interleaves between attention pairs of
  head h (4-matmul units, cc-contiguous into one PSUM bank) so the PE
  never waits on ACT exp.
- PSUM fused phase: sc pair 2 + av 3 + proj 2 + sum 1 = 8 banks; the
  output projection reuses the av/proj pools.
"""

import math
import os
import sys

import numpy as np

for _p in ("/opt/trn_rl_repo",):
    if _p not in sys.path:
        sys.path.insert(0, _p)

import ml_dtypes
import concourse.bass as bass
import concourse.mybir as mybir
from concourse import bacc
from concourse.tile import TileContext

B, T, C, H, D = 4, 2048, 2048, 16, 128
P = 128
NCORES = 8
HL = 8           # heads per core
FL = HL * D      # local feature dim = 1024
NCC = C // P     # 16 contraction chunks
NTC = T // P     # 16 t chunks
NSB = T // 512   # 4 t superblocks
NCB = C // 512   # 4 output col blocks
EXP_SCALE = 1.0 / math.sqrt(D)

f32 = mybir.dt.float32
f16 = mybir.dt.float16
np_f16 = np.float16


def _pair_layout(si, pr):
    """Packing of score pair pr (j-chunks 2pr, 2pr+1) of superblock si into a
    [128, 1024] PSUM tile. Returns [(jj, d_off, col_off, width), ...]."""
    js = (2 * pr, 2 * pr + 1)
    d0 = max(0, js[0] * P - si * 512)
    w0 = 512 - d0
    d1 = max(0, js[1] * P - si * 512)
    w1 = 512 - d1
    o1 = w0 if (w0 + w1) <= 512 else 512
    return [(js[0], d0, 0, w0), (js[1], d1, o1, w1)]


def build_nc():
    nc = bacc.Bacc()
    xt_d = nc.declare_dram_parameter("xt", [C, T], f16, isOutput=False)
    wqk_d = nc.declare_dram_parameter("wqk", [16, P, C], f16, isOutput=False)
    wv_d = nc.declare_dram_parameter("wv", [NCC, P, FL], f16, isOutput=False)
    wp_d = nc.declare_dram_parameter("wp", [HL, P, C], f16, isOutput=False)
    mask_d = nc.declare_dram_parameter("mask", [P, P], f16, isOutput=False)
    out_d = nc.declare_dram_parameter("out", [T, C], f32, isOutput=True)

    ACT = mybir.ActivationFunctionType

    with TileContext(nc) as tc:
        with tc.tile_pool(name="const", bufs=1) as cpool, \
             tc.tile_pool(name="avtp", bufs=1) as avt_pool:
            mask_sb = cpool.tile([P, P], f16)
            ones_sb = cpool.tile([P, 1], f16)
            nc.sync.dma_start(mask_sb[:], mask_d[:])
            # mask column 127 is all-ones; reuse it as the ones vector.
            nc.vector.tensor_copy(out=ones_sb[:], in_=mask_sb[:, P - 1:P])
            avts = [avt_pool.tile([P, T], f16, tag=f"avt{hh}",
                                  name=f"avt{hh}") for hh in range(HL)]

            state = {"cpi": 0}

            def evac(dst, src):
                if state["cpi"] % 2 == 0:
                    nc.vector.tensor_copy(out=dst, in_=src)
                else:
                    nc.scalar.copy(out=dst, in_=src)
                state["cpi"] += 1

            with tc.tile_pool(name="qkp", bufs=2) as qk_pool, \
                 tc.tile_pool(name="vp", bufs=1) as v_pool, \
                 tc.tile_pool(name="wqkp", bufs=4) as wqk_pool, \
                 tc.tile_pool(name="xtp", bufs=1) as xt_pool:
                vsb = [v_pool.tile([P, FL], f16, tag=f"v{tb}", name=f"v{tb}")
                       for tb in range(NTC)]
                xts = [xt_pool.tile([P, T], f16, tag=f"xt{cc}",
                                    name=f"xt{cc}") for cc in range(NCC)]

                qk_tiles = {}   # j -> rotating [P, T] tile
                wqk_tiles = {}  # j -> rotating [P, C] weight tile

                def fetch_wqk(j):
                    wt = wqk_pool.tile([P, C], f16, tag="wqk",
                                       name=f"wqk{j}")
                    nc.sync.dma_start(wt[:], wqk_d[j])
                    wqk_tiles[j] = wt

                def new_qk_tile(j):
                    t_ = qk_pool.tile([P, T], f16,
                                      tag="q" if j < HL else "k",
                                      name=f"qk{j}")
                    qk_tiles[j] = t_
                    return t_

                # DMA order: head-0 weights first (gates the first matmul),
                # then xt split across sync+scalar queues, wv halves behind
                # the xt halves, remaining wqk last on sync.
                fetch_wqk(0)
                fetch_wqk(8)
                for cc in range(8):
                    nc.sync.dma_start(xts[cc][:], xt_d[cc * P:(cc + 1) * P, :])
                for cc in range(8, NCC):
                    nc.scalar.dma_start(xts[cc][:],
                                        xt_d[cc * P:(cc + 1) * P, :])
                with tc.tile_pool(name="wvp", bufs=1) as wv_pool:
                    wvs = [wv_pool.tile([P, FL], f16, tag=f"wv{cc}",
                                        name=f"wv{cc}") for cc in range(NCC)]
                    for cc in range(8):
                        nc.sync.dma_start(wvs[cc][:], wv_d[cc])
                    for cc in range(8, NCC):
                        nc.scalar.dma_start(wvs[cc][:], wv_d[cc])
                    fetch_wqk(1)
                    fetch_wqk(9)

                    with tc.tile_pool(name="pp", bufs=8, space="PSUM") as pp:
                        # ---------- QK projection, head 0 ----------
                        for j in (0, 8):
                            dst = new_qk_tile(j)
                            for sb in range(NSB):
                                ps = pp.tile([P, 512], f32, tag="pp",
                                             name=f"pj{j}_{sb}")
                                for cc in range(NCC):
                                    nc.tensor.matmul(
                                        ps[:],
                                        wqk_tiles[j][:, cc * P:(cc + 1) * P],
                                        xts[cc][:, sb * 512:(sb + 1) * 512],
                                        start=(cc == 0), stop=(cc == NCC - 1))
                                evac(dst[:, sb * 512:(sb + 1) * 512], ps[:])

                        # ---------- V projection ----------
                        for tb in range(NTC):
                            for vb in range(2):
                                ps = pp.tile([P, 512], f32, tag="pp",
                                             name=f"pv{tb}_{vb}")
                                for cc in range(NCC):
                                    nc.tensor.matmul(
                                        ps[:], xts[cc][:, tb * P:(tb + 1) * P],
                                        wvs[cc][:, vb * 512:(vb + 1) * 512],
                                        start=(cc == 0), stop=(cc == NCC - 1))
                                evac(vsb[tb][:, vb * 512:(vb + 1) * 512],
                                     ps[:])
                # wvs + pp freed

                # ---------- fused attention + next-head QK proj ----------
                with tc.tile_pool(name="scp", bufs=1, space="PSUM") as scp, \
                     tc.tile_pool(name="avp", bufs=3, space="PSUM") as avp, \
                     tc.tile_pool(name="pjp", bufs=2, space="PSUM") as pjp, \
                     tc.tile_pool(name="sup", bufs=1, space="PSUM") as sup, \
                     tc.tile_pool(name="etp", bufs=3) as etp, \
                     tc.tile_pool(name="Sp", bufs=2) as Sp, \
                     tc.tile_pool(name="stp", bufs=2) as stp, \
                     tc.tile_pool(name="wpp", bufs=2) as wp_pool, \
                     tc.tile_pool(name="obp", bufs=4) as obp:
                    # deferred si tails: each tail (sum-MMs, recip, broadcast,
                    # normalize) is emitted two pairs after its si completes
                    # so the in-order PE stream never blocks on the S chains.
                    gp = 0            # global pair counter
                    tails = []        # (ready_at_gp, emit_fn)

                    def flush_tails(now):
                        while tails and tails[0][0] <= now:
                            tails.pop(0)[1]()

                    for h in range(HL):
                        qt, kt = qk_tiles[h], qk_tiles[HL + h]
                        # prefetch weights for head h+2's projection
                        if h + 2 < HL:
                            fetch_wqk(h + 2)
                            fetch_wqk(HL + h + 2)
                        # proj work units for head h+1
                        units = []
                        if h + 1 < HL:
                            for j in (h + 1, HL + h + 1):
                                new_qk_tile(j)
                                for sb in range(NSB):
                                    for cc0 in range(0, NCC, 4):
                                        units.append((j, sb, cc0))
                        if h == HL - 1:
                            # prefetch wp column-block 0 for the output
                            # projection (scalar queue is idle here)
                            wp_t = {}
                            for f in range(HL):
                                t_ = wp_pool.tile([P, 512], f16,
                                                  tag=f"wp{f}",
                                                  name=f"wp{f}_0")
                                nc.scalar.dma_start(t_[:], wp_d[f][:, 0:512])
                                wp_t[f] = t_
                        pjt = {}
                        ui = 0
                        pair_no = 0

                        def emit_units(target):
                            nonlocal ui
                            while ui < min(target, len(units)):
                                (j, sb, cc0) = units[ui]
                                if cc0 == 0:
                                    pjt[(j, sb)] = pjp.tile(
                                        [P, 512], f32, tag="pj",
                                        name=f"pj{j}_{sb}")
                                ps = pjt[(j, sb)]
                                for cc in range(cc0, cc0 + 4):
                                    nc.tensor.matmul(
                                        ps[:],
                                        wqk_tiles[j][:, cc * P:(cc + 1) * P],
                                        xts[cc][:, sb * 512:(sb + 1) * 512],
                                        start=(cc == 0), stop=(cc == NCC - 1))
                                if cc0 == NCC - 4:
                                    # DVE only: an ACT-queued copy would
                                    # head-of-line block the exp stream.
                                    nc.vector.tensor_copy(
                                        out=qk_tiles[j][:, sb * 512:
                                                        (sb + 1) * 512],
                                        in_=ps[:])
                                    del pjt[(j, sb)]
                                ui += 1

                        for si in range(NSB):
                            njc = 4 * si + 4
                            npr = njc // 2
                            av_ps = avp.tile([P, 512], f32, tag="av")
                            S_a = Sp.tile([P, 512], f16, tag="Sa")
                            S_b = None
                            if si >= 1:  # gpsimd handles pairs pr%4==3
                                S_b = Sp.tile([P, 512], f16, tag="Sb")
                                nc.gpsimd.memset(S_b[:], 0.0)
                            first_a = True
                            pend = None
                            for pr in range(npr):
                                infos = _pair_layout(si, pr)
                                sc = scp.tile([P, 1024], f32, tag="sc")
                                for (jj, dd, oo, ww) in infos:
                                    nc.tensor.matmul(
                                        sc[:, oo:oo + ww],
                                        kt[:, jj * P:(jj + 1) * P],
                                        qt[:, si * 512 + dd:(si + 1) * 512],
                                        start=True, stop=True)
                                et = etp.tile([P, 1024], f16, tag="et")
                                end = infos[-1][2] + infos[-1][3]
                                nc.scalar.activation(
                                    et[:, :end], sc[:, :end], ACT.Exp,
                                    scale=EXP_SCALE)
                                for (jj, dd, oo, ww) in infos:
                                    if jj >= 4 * si:  # diagonal 128-block
                                        nc.vector.tensor_mul(
                                            out=et[:, oo:oo + P],
                                            in0=et[:, oo:oo + P],
                                            in1=mask_sb[:])
                                # S accumulation: pr%4==3 pairs on gpsimd
                                # (into zeroed S_b), the rest on DVE (S_a).
                                if pr % 4 == 3:
                                    for (jj, dd, oo, ww) in infos:
                                        nc.gpsimd.tensor_add(
                                            out=S_b[:, dd:], in0=S_b[:, dd:],
                                            in1=et[:, oo:oo + ww])
                                else:
                                    for (jj, dd, oo, ww) in infos:
                                        if first_a:
                                            nc.vector.tensor_copy(
                                                out=S_a[:, dd:],
                                                in_=et[:, oo:oo + ww])
                                            first_a = False
                                        else:
                                            nc.vector.tensor_add(
                                                out=S_a[:, dd:],
                                                in0=S_a[:, dd:],
                                                in1=et[:, oo:oo + ww])
                                pair_no += 1
                                emit_units(2 * pair_no)
                                if pend is not None:
                                    pet, pinfos = pend
                                    for (jj, dd, oo, ww) in pinfos:
                                        nc.tensor.matmul(
                                            av_ps[:, dd:],
                                            vsb[jj][:, h * P:(h + 1) * P],
                                            pet[:, oo:oo + ww],
                                            start=(jj == 0), stop=False)
                                pend = (et, infos)
                                gp += 1
                                flush_tails(gp - 2)
                            pet, pinfos = pend
                            for (jj, dd, oo, ww) in pinfos:
                                nc.tensor.matmul(
                                    av_ps[:, dd:],
                                    vsb[jj][:, h * P:(h + 1) * P],
                                    pet[:, oo:oo + ww],
                                    start=(jj == 0), stop=(jj == njc - 1))

                            def make_tail(h=h, si=si, av_ps=av_ps, S_a=S_a,
                                          S_b=S_b):
                                def emit_tail():
                                    sum_ps = sup.tile([1, 512], f32,
                                                      tag="sum",
                                                      name=f"sum{h}_{si}")
                                    nc.tensor.matmul(
                                        sum_ps[:], ones_sb[:], S_a[:],
                                        start=True, stop=(S_b is None))
                                    if S_b is not None:
                                        nc.tensor.matmul(
                                            sum_ps[:], ones_sb[:], S_b[:],
                                            start=False, stop=True)
                                    rec = stp.tile([1, 512], f32, tag="rec",
                                                   name=f"rec{h}_{si}")
                                    nc.vector.reciprocal_approx_fast(
                                        out=rec[:], in_=sum_ps[:])
                                    recb = stp.tile([P, 512], f32, tag="recb",
                                                    name=f"recb{h}_{si}")
                                    nc.gpsimd.partition_broadcast(
                                        recb[:], rec[:])
                                    nc.vector.tensor_mul(
                                        out=avts[h][:, si * 512:
                                                    (si + 1) * 512],
                                        in0=av_ps[:], in1=recb[:])
                                return emit_tail

                            tails.append((gp + 2, make_tail()))
                        emit_units(len(units))
                    flush_tails(10 ** 9)

                    # ---------- output projection (cb-outer, wp streamed) ---
                    gi = 0
                    for cb in range(NCB):
                        if cb + 1 < NCB:
                            wp_next = {}
                            for f in range(HL):
                                t_ = wp_pool.tile([P, 512], f16,
                                                  tag=f"wp{f}",
                                                  name=f"wp{f}_{cb + 1}")
                                nc.sync.dma_start(
                                    t_[:], wp_d[f][:, (cb + 1) * 512:
                                                   (cb + 2) * 512])
                                wp_next[f] = t_
                        for tch in range(NTC):
                            pool = avp if gi % 2 == 0 else pjp
                            tag = "av" if gi % 2 == 0 else "pj"
                            ps = pool.tile([P, 512], f32, tag=tag,
                                           name=f"po{cb}_{tch}")
                            gi += 1
                            for f in range(HL):
                                nc.tensor.matmul(
                                    ps[:], avts[f][:, tch * P:(tch + 1) * P],
                                    wp_t[f][:, :],
                                    start=(f == 0), stop=(f == HL - 1))
                            ob = obp.tile([P, 512], f32, tag="ob")
                            evac(ob[:], ps[:])
                            nc.sync.dma_start(
                                out_d[tch * P:(tch + 1) * P,
                                      cb * 512:(cb + 1) * 512], ob[:])
                        if cb + 1 < NCB:
                            wp_t = wp_next
    nc.compile()
    return nc


def _make_mask():
    pp_ = np.arange(P)[:, None]
    ff = np.arange(P)[None, :]
    return np.where(ff >= pp_, 1.0, 0.0).astype(np_f16)


def _prep_inputs(x, w_qkv, w_proj):
    mask = _make_mask()
    per_g = {}
    for g in range(2):
        q = w_qkv[:, g * FL:(g + 1) * FL]
        k = w_qkv[:, C + g * FL:C + (g + 1) * FL]
        v = w_qkv[:, 2 * C + g * FL:2 * C + (g + 1) * FL]
        wqk_cat = np.concatenate([q, k], axis=1)  # [C, 2048]
        wqk_p = np.ascontiguousarray(
            wqk_cat.reshape(NCC, P, 16, P).transpose(2, 1, 0, 3)
            .reshape(16, P, C)).astype(np_f16)
        wv_p = np.ascontiguousarray(v.reshape(NCC, P, FL)).astype(np_f16)
        wp_p = np.ascontiguousarray(
            w_proj[g * FL:(g + 1) * FL, :].reshape(HL, P, C)).astype(np_f16)
        per_g[g] = (wqk_p, wv_p, wp_p)
    in_maps = []
    for core in range(NCORES):
        b, g = core // 2, core % 2
        wqk_p, wv_p, wp_p = per_g[g]
        in_maps.append({
            "xt": np.ascontiguousarray(x[b].T).astype(np_f16),
            "wqk": wqk_p,
            "wv": wv_p,
            "wp": wp_p,
            "mask": mask,
        })
    return in_maps


_nc_cache = None
last_results = None  # BassKernelResults of the most recent run (for test.py)


def kernel(x, w_qkv, w_proj):
    global _nc_cache, last_results
    from concourse.bass_utils import run_bass_kernel_spmd

    x = np.asarray(x, dtype=np.float32)
    w_qkv = np.asarray(w_qkv, dtype=np.float32)
    w_proj = np.asarray(w_proj, dtype=np.float32)

    if _nc_cache is None:
        _nc_cache = build_nc()
    nc = _nc_cache

    in_maps = _prep_inputs(x, w_qkv, w_proj)
    trace = bool(int(os.environ.get("KERNEL_TRACE", "0")))
    res = run_bass_kernel_spmd(nc, in_maps, list(range(NCORES)), trace=trace)
    last_results = res

    out = np.empty((B, T, C), dtype=np.float32)
    for b in range(B):
        out[b] = res.results[2 * b]["out"] + res.results[2 * b + 1]["out"]
    return out


# revision 6
# speedup vs baseline: 1.1683x; 1.1683x over previous
"""Causal self-attention Trainium2 Bass kernel, v4.

B=4, T=2048, C=2048, H=16, D=128, fp32 I/O. DP=4 x TP=2 (Megatron
head-group split); host sums TP pairs.

Per-core structure:
  [QK proj h0] [V proj] [fused: attn(h) + QK proj(h+1)] x8 [out proj]

- f16 activations/weights, fp32 PSUM accumulation everywhere.
- Fully SBUF-resident intermediates; q/k tiles rotate (lifetime ~2 head
  slots), wqk weight tiles stream with bufs=4, wp streams per 512-col
  block during the cb-outer output projection.
- Attention: score pairs packed into [128,1024] PSUM tiles -> one exp
  per pair; exp tiles merged into per-si accumulators S_a (DVE, even
  pairs) / S_b (gpsimd, odd pairs); per-si two ones-matmuls reduce them
  into sum[1,512]; reciprocal -> partition_broadcast -> normalize fused
  into the av PSUM->SBUF evacuation.
- QK projection of head h+1 interleaves between attention pairs of
  head h (4-matmul units, cc-contiguous into one PSUM bank) so the PE
  never waits on ACT exp.
- PSUM fused phase: sc pair 2 + av 3 + proj 2 + sum 1 = 8 banks; the
  output projection reuses the av/proj pools.
"""

import math
import os
import sys

import numpy as np

for _p in ("/opt/trn_rl_repo",):
    if _p not in sys.path:
        sys.path.insert(0, _p)

import ml_dtypes
import concourse.bass as bass
import concourse.mybir as mybir
from concourse import bacc
from concourse.tile import TileContext

B, T, C, H, D = 4, 2048, 2048, 16, 128
P = 128
NCORES = 8
HL = 8           # heads per core
FL = HL * D      # local feature dim = 1024
NCC = C // P     # 16 contraction chunks
NTC = T // P     # 16 t chunks
NSB = T // 512   # 4 t superblocks
NCB = C // 512   # 4 output col blocks
EXP_SCALE = 1.0 / math.sqrt(D)

f32 = mybir.dt.float32
f16 = mybir.dt.float16
np_f16 = np.float16


def _pair_layout(si, pr):
    """Packing of score pair pr (j-chunks 2pr, 2pr+1) of superblock si into a
    [128, 1024] PSUM tile. Returns [(jj, d_off, col_off, width), ...]."""
    js = (2 * pr, 2 * pr + 1)
    d0 = max(0, js[0] * P - si * 512)
    w0 = 512 - d0
    d1 = max(0, js[1] * P - si * 512)
    w1 = 512 - d1
    o1 = w0 if (w0 + w1) <= 512 else 512
    return [(js[0], d0, 0, w0), (js[1], d1, o1, w1)]


def build_nc():
    nc = bacc.Bacc()
    xt_d = nc.declare_dram_parameter("xt", [C, T], f16, isOutput=False)
    wqk_d = nc.declare_dram_parameter("wqk", [16, P, C], f16, isOutput=False)
    wv_d = nc.declare_dram_parameter("wv", [NCC, P, FL], f16, isOutput=False)
    wp_d = nc.declare_dram_parameter("wp", [HL, P, C], f16, isOutput=False)
    mask_d = nc.declare_dram_parameter("mask", [P, P], f16, isOutput=False)
    out_d = nc.declare_dram_parameter("out", [T, C], f32, isOutput=True)

    ACT = mybir.ActivationFunctionType

    with TileContext(nc) as tc:
        with tc.tile_pool(name="const", bufs=1) as cpool, \
             tc.tile_pool(name="avtp", bufs=1) as avt_pool:
            mask_sb = cpool.tile([P, P], f16)
            ones_sb = cpool.tile([P, 1], f16)
            nc.sync.dma_start(mask_sb[:], mask_d[:])
            # mask column 127 is all-ones; reuse it as the ones vector.
            nc.vector.tensor_copy(out=ones_sb[:], in_=mask_sb[:, P - 1:P])
            avts = [avt_pool.tile([P, T], f16, tag=f"avt{hh}",
                                  name=f"avt{hh}") for hh in range(HL)]

            state = {"cpi": 0}

            def evac(dst, src):
                if state["cpi"] % 2 == 0:
                    nc.vector.tensor_copy(out=dst, in_=src)
                else:
                    nc.scalar.copy(out=dst, in_=src)
                state["cpi"] += 1

            with tc.tile_pool(name="qkp", bufs=2) as qk_pool, \
                 tc.tile_pool(name="vp", bufs=1) as v_pool, \
                 tc.tile_pool(name="wqkp", bufs=4) as wqk_pool, \
                 tc.tile_pool(name="xtp", bufs=1) as xt_pool:
                vsb = [v_pool.tile([P, FL], f16, tag=f"v{tb}", name=f"v{tb}")
                       for tb in range(NTC)]
                xts = [xt_pool.tile([P, T], f16, tag=f"xt{cc}",
                                    name=f"xt{cc}") for cc in range(NCC)]

                qk_tiles = {}   # j -> rotating [P, T] tile
                wqk_tiles = {}  # j -> rotating [P, C] weight tile

                def fetch_wqk(j):
                    wt = wqk_pool.tile([P, C], f16, tag="wqk",
                                       name=f"wqk{j}")
                    nc.sync.dma_start(wt[:], wqk_d[j])
                    wqk_tiles[j] = wt

                def new_qk_tile(j):
                    t_ = qk_pool.tile([P, T], f16,
                                      tag="q" if j < HL else "k",
                                      name=f"qk{j}")
                    qk_tiles[j] = t_
                    return t_

                # DMA order: head-0 weights first (gates the first matmul),
                # then xt split across sync+scalar queues, wv halves behind
                # the xt halves, remaining wqk last on sync.
                fetch_wqk(0)
                fetch_wqk(8)
                for cc in range(8):
                    nc.sync.dma_start(xts[cc][:], xt_d[cc * P:(cc + 1) * P, :])
                for cc in range(8, NCC):
                    nc.scalar.dma_start(xts[cc][:],
                                        xt_d[cc * P:(cc + 1) * P, :])
                with tc.tile_pool(name="wvp", bufs=1) as wv_pool:
                    wvs = [wv_pool.tile([P, FL], f16, tag=f"wv{cc}",
                                        name=f"wv{cc}") for cc in range(NCC)]
                    for cc in range(8):
                        nc.sync.dma_start(wvs[cc][:], wv_d[cc])
                    for cc in range(8, NCC):
                        nc.scalar.dma_start(wvs[cc][:], wv_d[cc])
                    fetch_wqk(1)
                    fetch_wqk(9)

                    with tc.tile_pool(name="pp", bufs=8, space="PSUM") as pp:
                        # ---------- QK projection, head 0 ----------
                        for j in (0, 8):
                            dst = new_qk_tile(j)
                            for sb in range(NSB):
                                ps = pp.tile([P, 512], f32, tag="pp",
                                             name=f"pj{j}_{sb}")
                                for cc in range(NCC):
                                    nc.tensor.matmul(
                                        ps[:],
                                        wqk_tiles[j][:, cc * P:(cc + 1) * P],
                                        xts[cc][:, sb * 512:(sb + 1) * 512],
                                        start=(cc == 0), stop=(cc == NCC - 1))
                                evac(dst[:, sb * 512:(sb + 1) * 512], ps[:])

                        # ---------- V projection ----------
                        for tb in range(NTC):
                            for vb in range(2):
                                ps = pp.tile([P, 512], f32, tag="pp",
                                             name=f"pv{tb}_{vb}")
                                for cc in range(NCC):
                                    nc.tensor.matmul(
                                        ps[:], xts[cc][:, tb * P:(tb + 1) * P],
                                        wvs[cc][:, vb * 512:(vb + 1) * 512],
                                        start=(cc == 0), stop=(cc == NCC - 1))
                                evac(vsb[tb][:, vb * 512:(vb + 1) * 512],
                                     ps[:])
                # wvs + pp freed

                # ---------- fused attention + next-head QK proj ----------
                with tc.tile_pool(name="scp", bufs=1, space="PSUM") as scp, \
                     tc.tile_pool(name="avp", bufs=3, space="PSUM") as avp, \
                     tc.tile_pool(name="pjp", bufs=2, space="PSUM") as pjp, \
                     tc.tile_pool(name="sup", bufs=1, space="PSUM") as sup, \
                     tc.tile_pool(name="etp", bufs=3) as etp, \
                     tc.tile_pool(name="Sp", bufs=2) as Sp, \
                     tc.tile_pool(name="stp", bufs=2) as stp, \
                     tc.tile_pool(name="wpp", bufs=2) as wp_pool, \
                     tc.tile_pool(name="obp", bufs=4) as obp:
                    # deferred si tails: each tail (sum-MMs, recip, broadcast,
                    # normalize) is emitted two pairs after its si completes
                    # so the in-order PE stream never blocks on the S chains.
                    gp = 0            # global pair counter
                    tails = []        # (ready_at_gp, emit_fn)

                    def flush_tails(now):
                        while tails and tails[0][0] <= now:
                            tails.pop(0)[1]()

                    for h in range(HL):
                        qt, kt = qk_tiles[h], qk_tiles[HL + h]
                        # prefetch weights for head h+2's projection
                        if h + 2 < HL:
                            fetch_wqk(h + 2)
                            fetch_wqk(HL + h + 2)
                        # proj work units for head h+1
                        units = []
                        if h + 1 < HL:
                            for j in (h + 1, HL + h + 1):
                                new_qk_tile(j)
                                for sb in range(NSB):
                                    for cc0 in range(0, NCC, 4):
                                        units.append((j, sb, cc0))
                        if h == HL - 1:
                            # prefetch wp column-block 0 for the output
                            # projection (scalar queue is idle here)
                            wp_t = {}
                            for f in range(HL):
                                t_ = wp_pool.tile([P, 512], f16,
                                                  tag=f"wp{f}",
                                                  name=f"wp{f}_0")
                                nc.scalar.dma_start(t_[:], wp_d[f][:, 0:512])
                                wp_t[f] = t_
                        pjt = {}
                        ui = 0
                        pair_no = 0

                        evac_q = []

                        def drain_evacs():
                            for (j, sb, ps) in evac_q:
                                nc.vector.tensor_copy(
                                    out=qk_tiles[j][:, sb * 512:
                                                    (sb + 1) * 512],
                                    in_=ps[:])
                            del evac_q[:]

                        def emit_units(target):
                            # lazy evacs from the previous call: by now the
                            # group's matmuls have executed, so the DVE copy
                            # won't sit blocked at the head of the queue.
                            nonlocal ui
                            drain_evacs()
                            while ui < min(target, len(units)):
                                (j, sb, cc0) = units[ui]
                                if cc0 == 0:
                                    pjt[(j, sb)] = pjp.tile(
                                        [P, 512], f32, tag="pj",
                                        name=f"pj{j}_{sb}")
                                ps = pjt[(j, sb)]
                                for cc in range(cc0, cc0 + 4):
                                    nc.tensor.matmul(
                                        ps[:],
                                        wqk_tiles[j][:, cc * P:(cc + 1) * P],
                                        xts[cc][:, sb * 512:(sb + 1) * 512],
                                        start=(cc == 0), stop=(cc == NCC - 1))
                                if cc0 == NCC - 4:
                                    evac_q.append((j, sb, pjt.pop((j, sb))))
                                ui += 1

                        for si in range(NSB):
                            njc = 4 * si + 4
                            npr = njc // 2
                            av_ps = avp.tile([P, 512], f32, tag="av")
                            S_a = Sp.tile([P, 512], f16, tag="Sa")
                            S_b = None
                            if si >= 1:  # gpsimd handles pairs pr%4==3
                                S_b = Sp.tile([P, 512], f16, tag="Sb")
                                nc.gpsimd.memset(S_b[:], 0.0)
                            first_a = True
                            pend = None
                            for pr in range(npr):
                                infos = _pair_layout(si, pr)
                                sc = scp.tile([P, 1024], f32, tag="sc")
                                for (jj, dd, oo, ww) in infos:
                                    nc.tensor.matmul(
                                        sc[:, oo:oo + ww],
                                        kt[:, jj * P:(jj + 1) * P],
                                        qt[:, si * 512 + dd:(si + 1) * 512],
                                        start=True, stop=True)
                                et = etp.tile([P, 1024], f16, tag="et")
                                end = infos[-1][2] + infos[-1][3]
                                nc.scalar.activation(
                                    et[:, :end], sc[:, :end], ACT.Exp,
                                    scale=EXP_SCALE)
                                for (jj, dd, oo, ww) in infos:
                                    if jj >= 4 * si:  # diagonal 128-block
                                        nc.vector.tensor_mul(
                                            out=et[:, oo:oo + P],
                                            in0=et[:, oo:oo + P],
                                            in1=mask_sb[:])
                                # S accumulation: pr%4==3 pairs on gpsimd
                                # (into zeroed S_b), the rest on DVE (S_a).
                                if pr % 4 == 3:
                                    for (jj, dd, oo, ww) in infos:
                                        nc.gpsimd.tensor_add(
                                            out=S_b[:, dd:], in0=S_b[:, dd:],
                                            in1=et[:, oo:oo + ww])
                                else:
                                    for (jj, dd, oo, ww) in infos:
                                        if first_a:
                                            nc.vector.tensor_copy(
                                                out=S_a[:, dd:],
                                                in_=et[:, oo:oo + ww])
                                            first_a = False
                                        else:
                                            nc.vector.tensor_add(
                                                out=S_a[:, dd:],
                                                in0=S_a[:, dd:],
                                                in1=et[:, oo:oo + ww])
                                pair_no += 1
                                emit_units(2 * pair_no)
                                if pend is not None:
                                    pet, pinfos = pend
                                    for (jj, dd, oo, ww) in pinfos:
                                        nc.tensor.matmul(
                                            av_ps[:, dd:],
                                            vsb[jj][:, h * P:(h + 1) * P],
                                            pet[:, oo:oo + ww],
                                            start=(jj == 0), stop=False)
                                pend = (et, infos)
                                gp += 1
                                flush_tails(gp)
                            pet, pinfos = pend
                            for (jj, dd, oo, ww) in pinfos:
                                nc.tensor.matmul(
                                    av_ps[:, dd:],
                                    vsb[jj][:, h * P:(h + 1) * P],
                                    pet[:, oo:oo + ww],
                                    start=(jj == 0), stop=(jj == njc - 1))

                            def make_tail(h=h, si=si, av_ps=av_ps, S_a=S_a,
                                          S_b=S_b):
                                def emit_tail():
                                    sum_ps = sup.tile([1, 512], f32,
                                                      tag="sum",
                                                      name=f"sum{h}_{si}")
                                    nc.tensor.matmul(
                                        sum_ps[:], ones_sb[:], S_a[:],
                                        start=True, stop=(S_b is None))
                                    if S_b is not None:
                                        nc.tensor.matmul(
                                            sum_ps[:], ones_sb[:], S_b[:],
                                            start=False, stop=True)
                                    rec = stp.tile([1, 512], f32, tag="rec",
                                                   name=f"rec{h}_{si}")
                                    nc.vector.reciprocal_approx_fast(
                                        out=rec[:], in_=sum_ps[:])
                                    recb = stp.tile([P, 512], f32, tag="recb",
                                                    name=f"recb{h}_{si}")
                                    nc.gpsimd.partition_broadcast(
                                        recb[:], rec[:])
                                    nc.vector.tensor_mul(
                                        out=avts[h][:, si * 512:
                                                    (si + 1) * 512],
                                        in0=av_ps[:], in1=recb[:])
                                return emit_tail

                            tails.append((gp + 2, make_tail()))
                        emit_units(len(units))
                        drain_evacs()
                    flush_tails(10 ** 9)

                    # ---------- output projection (cb-outer, wp streamed) ---
                    gi = 0
                    for cb in range(NCB):
                        if cb + 1 < NCB:
                            wp_next = {}
                            for f in range(HL):
                                t_ = wp_pool.tile([P, 512], f16,
                                                  tag=f"wp{f}",
                                                  name=f"wp{f}_{cb + 1}")
                                nc.sync.dma_start(
                                    t_[:], wp_d[f][:, (cb + 1) * 512:
                                                   (cb + 2) * 512])
                                wp_next[f] = t_
                        for tch in range(NTC):
                            pool = avp if gi % 2 == 0 else pjp
                            tag = "av" if gi % 2 == 0 else "pj"
                            ps = pool.tile([P, 512], f32, tag=tag,
                                           name=f"po{cb}_{tch}")
                            gi += 1
                            for f in range(HL):
                                nc.tensor.matmul(
                                    ps[:], avts[f][:, tch * P:(tch + 1) * P],
                                    wp_t[f][:, :],
                                    start=(f == 0), stop=(f == HL - 1))
                            ob = obp.tile([P, 512], f32, tag="ob")
                            evac(ob[:], ps[:])
                            nc.sync.dma_start(
                                out_d[tch * P:(tch + 1) * P,
                                      cb * 512:(cb + 1) * 512], ob[:])
                        if cb + 1 < NCB:
                            wp_t = wp_next
    nc.compile()
    return nc


def _make_mask():
    pp_ = np.arange(P)[:, None]
    ff = np.arange(P)[None, :]
    return np.where(ff >= pp_, 1.0, 0.0).astype(np_f16)


def _prep_inputs(x, w_qkv, w_proj):
    mask = _make_mask()
    per_g = {}
    for g in range(2):
        q = w_qkv[:, g * FL:(g + 1) * FL]
        k = w_qkv[:, C + g * FL:C + (g + 1) * FL]
        v = w_qkv[:, 2 * C + g * FL:2 * C + (g + 1) * FL]
        wqk_cat = np.concatenate([q, k], axis=1)  # [C, 2048]
        wqk_p = np.ascontiguousarray(
            wqk_cat.reshape(NCC, P, 16, P).transpose(2, 1, 0, 3)
            .reshape(16, P, C)).astype(np_f16)
        wv_p = np.ascontiguousarray(v.reshape(NCC, P, FL)).astype(np_f16)
        wp_p = np.ascontiguousarray(
            w_proj[g * FL:(g + 1) * FL, :].reshape(HL, P, C)).astype(np_f16)
        per_g[g] = (wqk_p, wv_p, wp_p)
    in_maps = []
    for core in range(NCORES):
        b, g = core // 2, core % 2
        wqk_p, wv_p, wp_p = per_g[g]
        in_maps.append({
            "xt": np.ascontiguousarray(x[b].T).astype(np_f16),
            "wqk": wqk_p,
            "wv": wv_p,
            "wp": wp_p,
            "mask": mask,
        })
    return in_maps


_nc_cache = None
last_results = None  # BassKernelResults of the most recent run (for test.py)


def kernel(x, w_qkv, w_proj):
    global _nc_cache, last_results
    from concourse.bass_utils import run_bass_kernel_spmd

    x = np.asarray(x, dtype=np.float32)
    w_qkv = np.asarray(w_qkv, dtype=np.float32)
    w_proj = np.asarray(w_proj, dtype=np.float32)

    if _nc_cache is None:
        _nc_cache = build_nc()
    nc = _nc_cache

    in_maps = _prep_inputs(x, w_qkv, w_proj)
    trace = bool(int(os.environ.get("KERNEL_TRACE", "0")))
    res = run_bass_kernel_spmd(nc, in_maps, list(range(NCORES)), trace=trace)
    last_results = res

    out = np.empty((B, T, C), dtype=np.float32)
    for b in range(B):
        out[b] = res.results[2 * b]["out"] + res.results[2 * b + 1]["out"]
    return out


# revision 7
# speedup vs baseline: 1.1890x; 1.0177x over previous
"""Causal self-attention Trainium2 Bass kernel, v4.

B=4, T=2048, C=2048, H=16, D=128, fp32 I/O. DP=4 x TP=2 (Megatron
head-group split); host sums TP pairs.

Per-core structure:
  [QK proj h0] [V proj] [fused: attn(h) + QK proj(h+1)] x8 [out proj]

- f16 activations/weights, fp32 PSUM accumulation everywhere.
- Fully SBUF-resident intermediates; q/k tiles rotate (lifetime ~2 head
  slots), wqk weight tiles stream with bufs=4, wp streams per 512-col
  block during the cb-outer output projection.
- Attention: score pairs packed into [128,1024] PSUM tiles -> one exp
  per pair; exp tiles merged into per-si accumulators S_a (DVE, even
  pairs) / S_b (gpsimd, odd pairs); per-si two ones-matmuls reduce them
  into sum[1,512]; reciprocal -> partition_broadcast -> normalize fused
  into the av PSUM->SBUF evacuation.
- QK projection of head h+1 interleaves between attention pairs of
  head h (4-matmul units, cc-contiguous into one PSUM bank) so the PE
  never waits on ACT exp.
- PSUM fused phase: sc pair 2 + av 3 + proj 2 + sum 1 = 8 banks; the
  output projection reuses the av/proj pools.
"""

import math
import os
import sys

import numpy as np

for _p in ("/opt/trn_rl_repo",):
    if _p not in sys.path:
        sys.path.insert(0, _p)

import ml_dtypes
import concourse.bass as bass
import concourse.mybir as mybir
from concourse import bacc
from concourse.tile import TileContext

B, T, C, H, D = 4, 2048, 2048, 16, 128
P = 128
NCORES = 8
HL = 8           # heads per core
FL = HL * D      # local feature dim = 1024
NCC = C // P     # 16 contraction chunks
NTC = T // P     # 16 t chunks
NSB = T // 512   # 4 t superblocks
NCB = C // 512   # 4 output col blocks
EXP_SCALE = 1.0 / math.sqrt(D)

f32 = mybir.dt.float32
f16 = mybir.dt.float16
np_f16 = np.float16


def _pair_layout(si, pr):
    """Packing of score pair pr (j-chunks 2pr, 2pr+1) of superblock si into a
    [128, 1024] PSUM tile. Returns [(jj, d_off, col_off, width), ...]."""
    js = (2 * pr, 2 * pr + 1)
    d0 = max(0, js[0] * P - si * 512)
    w0 = 512 - d0
    d1 = max(0, js[1] * P - si * 512)
    w1 = 512 - d1
    o1 = w0 if (w0 + w1) <= 512 else 512
    return [(js[0], d0, 0, w0), (js[1], d1, o1, w1)]


def build_nc():
    nc = bacc.Bacc()
    xt_d = nc.declare_dram_parameter("xt", [C, T], f16, isOutput=False)
    wqk_d = nc.declare_dram_parameter("wqk", [16, P, C], f16, isOutput=False)
    wv_d = nc.declare_dram_parameter("wv", [NCC, P, FL], f16, isOutput=False)
    wp_d = nc.declare_dram_parameter("wp", [HL, P, C], f16, isOutput=False)
    mask_d = nc.declare_dram_parameter("mask", [P, P], f16, isOutput=False)
    out_d = nc.declare_dram_parameter("out", [T, C], f32, isOutput=True)

    ACT = mybir.ActivationFunctionType

    with TileContext(nc) as tc:
        with tc.tile_pool(name="const", bufs=1) as cpool, \
             tc.tile_pool(name="avtp", bufs=1) as avt_pool:
            mask_sb = cpool.tile([P, P], f16)
            ones_sb = cpool.tile([P, 1], f16)
            nc.sync.dma_start(mask_sb[:], mask_d[:])
            # mask column 127 is all-ones; reuse it as the ones vector.
            nc.vector.tensor_copy(out=ones_sb[:], in_=mask_sb[:, P - 1:P])
            avts = [avt_pool.tile([P, T], f16, tag=f"avt{hh}",
                                  name=f"avt{hh}") for hh in range(HL)]

            state = {"cpi": 0}

            def evac(dst, src):
                if state["cpi"] % 2 == 0:
                    nc.vector.tensor_copy(out=dst, in_=src)
                else:
                    nc.scalar.copy(out=dst, in_=src)
                state["cpi"] += 1

            with tc.tile_pool(name="qkp", bufs=2) as qk_pool, \
                 tc.tile_pool(name="vp", bufs=1) as v_pool, \
                 tc.tile_pool(name="wqkp", bufs=4) as wqk_pool, \
                 tc.tile_pool(name="xtp", bufs=1) as xt_pool:
                vsb = [v_pool.tile([P, FL], f16, tag=f"v{tb}", name=f"v{tb}")
                       for tb in range(NTC)]
                xts = [xt_pool.tile([P, T], f16, tag=f"xt{cc}",
                                    name=f"xt{cc}") for cc in range(NCC)]

                qk_tiles = {}   # j -> rotating [P, T] tile
                wqk_tiles = {}  # j -> rotating [P, C] weight tile

                def fetch_wqk(j):
                    wt = wqk_pool.tile([P, C], f16, tag="wqk",
                                       name=f"wqk{j}")
                    nc.sync.dma_start(wt[:], wqk_d[j])
                    wqk_tiles[j] = wt

                def new_qk_tile(j):
                    t_ = qk_pool.tile([P, T], f16,
                                      tag="q" if j < HL else "k",
                                      name=f"qk{j}")
                    qk_tiles[j] = t_
                    return t_

                # DMA order: head-0 weights first (gates the first matmul),
                # then xt split across sync+scalar queues, wv halves behind
                # the xt halves, remaining wqk last on sync.
                fetch_wqk(0)
                fetch_wqk(8)
                for cc in range(8):
                    nc.sync.dma_start(xts[cc][:], xt_d[cc * P:(cc + 1) * P, :])
                for cc in range(8, NCC):
                    nc.scalar.dma_start(xts[cc][:],
                                        xt_d[cc * P:(cc + 1) * P, :])
                with tc.tile_pool(name="wvp", bufs=1) as wv_pool:
                    wvs = [wv_pool.tile([P, FL], f16, tag=f"wv{cc}",
                                        name=f"wv{cc}") for cc in range(NCC)]
                    for cc in range(8):
                        nc.sync.dma_start(wvs[cc][:], wv_d[cc])
                    for cc in range(8, NCC):
                        nc.scalar.dma_start(wvs[cc][:], wv_d[cc])
                    fetch_wqk(1)
                    fetch_wqk(9)

                    with tc.tile_pool(name="pp", bufs=8, space="PSUM") as pp:
                        # ---------- QK projection, head 0 ----------
                        for j in (0, 8):
                            dst = new_qk_tile(j)
                            for sb in range(NSB):
                                ps = pp.tile([P, 512], f32, tag="pp",
                                             name=f"pj{j}_{sb}")
                                for cc in range(NCC):
                                    nc.tensor.matmul(
                                        ps[:],
                                        wqk_tiles[j][:, cc * P:(cc + 1) * P],
                                        xts[cc][:, sb * 512:(sb + 1) * 512],
                                        start=(cc == 0), stop=(cc == NCC - 1))
                                evac(dst[:, sb * 512:(sb + 1) * 512], ps[:])

                        # ---------- V projection ----------
                        for tb in range(NTC):
                            for vb in range(2):
                                ps = pp.tile([P, 512], f32, tag="pp",
                                             name=f"pv{tb}_{vb}")
                                for cc in range(NCC):
                                    nc.tensor.matmul(
                                        ps[:], xts[cc][:, tb * P:(tb + 1) * P],
                                        wvs[cc][:, vb * 512:(vb + 1) * 512],
                                        start=(cc == 0), stop=(cc == NCC - 1))
                                evac(vsb[tb][:, vb * 512:(vb + 1) * 512],
                                     ps[:])
                # wvs + pp freed

                # ---------- fused attention + next-head QK proj ----------
                with tc.tile_pool(name="scp", bufs=1, space="PSUM") as scp, \
                     tc.tile_pool(name="avp", bufs=3, space="PSUM") as avp, \
                     tc.tile_pool(name="pjp", bufs=2, space="PSUM") as pjp, \
                     tc.tile_pool(name="sup", bufs=1, space="PSUM") as sup, \
                     tc.tile_pool(name="etp", bufs=3) as etp, \
                     tc.tile_pool(name="Sp", bufs=2) as Sp, \
                     tc.tile_pool(name="stp", bufs=2) as stp, \
                     tc.tile_pool(name="wpp", bufs=2) as wp_pool, \
                     tc.tile_pool(name="obp", bufs=4) as obp:
                    # deferred si tails: each tail (sum-MMs, recip, broadcast,
                    # normalize) is emitted two pairs after its si completes
                    # so the in-order PE stream never blocks on the S chains.
                    gp = 0            # global pair counter
                    tails = []        # (ready_at_gp, emit_fn)

                    def flush_tails(now):
                        while tails and tails[0][0] <= now:
                            tails.pop(0)[1]()

                    for h in range(HL):
                        qt, kt = qk_tiles[h], qk_tiles[HL + h]
                        # prefetch weights for head h+2's projection
                        if h + 2 < HL:
                            fetch_wqk(h + 2)
                            fetch_wqk(HL + h + 2)
                        # proj work units for head h+1
                        units = []
                        if h + 1 < HL:
                            for j in (h + 1, HL + h + 1):
                                new_qk_tile(j)
                                for sb in range(NSB):
                                    for cc0 in range(0, NCC, 4):
                                        units.append((j, sb, cc0))
                        if h == HL - 1:
                            # prefetch wp column-block 0 for the output
                            # projection (scalar queue is idle here)
                            wp_t = {}
                            for f in range(HL):
                                t_ = wp_pool.tile([P, 512], f16,
                                                  tag=f"wp{f}",
                                                  name=f"wp{f}_0")
                                nc.scalar.dma_start(t_[:], wp_d[f][:, 0:512])
                                wp_t[f] = t_
                        pjt = {}
                        ui = 0
                        pair_no = 0

                        evac_q = []

                        def drain_evacs():
                            for (j, sb, ps) in evac_q:
                                nc.vector.tensor_copy(
                                    out=qk_tiles[j][:, sb * 512:
                                                    (sb + 1) * 512],
                                    in_=ps[:])
                            del evac_q[:]

                        def emit_units(target):
                            # lazy evacs from the previous call: by now the
                            # group's matmuls have executed, so the DVE copy
                            # won't sit blocked at the head of the queue.
                            nonlocal ui
                            drain_evacs()
                            while ui < min(target, len(units)):
                                (j, sb, cc0) = units[ui]
                                if cc0 == 0:
                                    pjt[(j, sb)] = pjp.tile(
                                        [P, 512], f32, tag="pj",
                                        name=f"pj{j}_{sb}")
                                ps = pjt[(j, sb)]
                                for cc in range(cc0, cc0 + 4):
                                    nc.tensor.matmul(
                                        ps[:],
                                        wqk_tiles[j][:, cc * P:(cc + 1) * P],
                                        xts[cc][:, sb * 512:(sb + 1) * 512],
                                        start=(cc == 0), stop=(cc == NCC - 1))
                                if cc0 == NCC - 4:
                                    evac_q.append((j, sb, pjt.pop((j, sb))))
                                ui += 1

                        for si in range(NSB):
                            njc = 4 * si + 4
                            npr = njc // 2
                            av_ps = avp.tile([P, 512], f32, tag="av")
                            S_a = Sp.tile([P, 512], f16, tag="Sa")
                            S_b = None
                            if si >= 1:  # gpsimd handles pairs pr%4==3
                                S_b = Sp.tile([P, 512], f16, tag="Sb")
                                nc.gpsimd.memset(S_b[:], 0.0)
                            first_a = True
                            pend = None
                            for pr in range(npr):
                                infos = _pair_layout(si, pr)
                                sc = scp.tile([P, 1024], f32, tag="sc")
                                for (jj, dd, oo, ww) in infos:
                                    nc.tensor.matmul(
                                        sc[:, oo:oo + ww],
                                        kt[:, jj * P:(jj + 1) * P],
                                        qt[:, si * 512 + dd:(si + 1) * 512],
                                        start=True, stop=True)
                                et = etp.tile([P, 1024], f16, tag="et")
                                end = infos[-1][2] + infos[-1][3]
                                nc.scalar.activation(
                                    et[:, :end], sc[:, :end], ACT.Exp,
                                    scale=EXP_SCALE)
                                for (jj, dd, oo, ww) in infos:
                                    if jj >= 4 * si:  # diagonal 128-block
                                        nc.vector.tensor_mul(
                                            out=et[:, oo:oo + P],
                                            in0=et[:, oo:oo + P],
                                            in1=mask_sb[:])
                                # S accumulation: pr%4==3 pairs on gpsimd
                                # (into zeroed S_b), the rest on DVE (S_a).
                                if pr % 4 == 3:
                                    for (jj, dd, oo, ww) in infos:
                                        nc.gpsimd.tensor_add(
                                            out=S_b[:, dd:], in0=S_b[:, dd:],
                                            in1=et[:, oo:oo + ww])
                                else:
                                    for (jj, dd, oo, ww) in infos:
                                        if first_a:
                                            nc.vector.tensor_copy(
                                                out=S_a[:, dd:],
                                                in_=et[:, oo:oo + ww])
                                            first_a = False
                                        else:
                                            nc.vector.tensor_add(
                                                out=S_a[:, dd:],
                                                in0=S_a[:, dd:],
                                                in1=et[:, oo:oo + ww])
                                pair_no += 1
                                emit_units(2 * pair_no)
                                if pend is not None:
                                    pet, pinfos = pend
                                    for (jj, dd, oo, ww) in pinfos:
                                        nc.tensor.matmul(
                                            av_ps[:, dd:],
                                            vsb[jj][:, h * P:(h + 1) * P],
                                            pet[:, oo:oo + ww],
                                            start=(jj == 0), stop=False)
                                pend = (et, infos)
                                gp += 1
                                flush_tails(gp)
                            pet, pinfos = pend
                            for (jj, dd, oo, ww) in pinfos:
                                nc.tensor.matmul(
                                    av_ps[:, dd:],
                                    vsb[jj][:, h * P:(h + 1) * P],
                                    pet[:, oo:oo + ww],
                                    start=(jj == 0), stop=(jj == njc - 1))

                            # tail split into three flush points so each
                            # cross-engine hop is emitted only after its
                            # dependency has had time to complete (no
                            # head-of-line blocking in the in-order queues).
                            def make_t1(h=h, si=si, S_a=S_a, S_b=S_b,
                                        box=None):
                                def emit():
                                    sum_ps = sup.tile([1, 512], f32,
                                                      tag="sum",
                                                      name=f"sum{h}_{si}")
                                    nc.tensor.matmul(
                                        sum_ps[:], ones_sb[:], S_a[:],
                                        start=True, stop=(S_b is None))
                                    if S_b is not None:
                                        nc.tensor.matmul(
                                            sum_ps[:], ones_sb[:], S_b[:],
                                            start=False, stop=True)
                                    rec = stp.tile([1, 512], f32, tag="rec",
                                                   name=f"rec{h}_{si}")
                                    nc.vector.reciprocal_approx_fast(
                                        out=rec[:], in_=sum_ps[:])
                                    box["rec"] = rec
                                return emit

                            def make_t2(h=h, si=si, box=None):
                                def emit():
                                    recb = stp.tile([P, 512], f32, tag="recb",
                                                    name=f"recb{h}_{si}")
                                    nc.gpsimd.partition_broadcast(
                                        recb[:], box["rec"][:])
                                    box["recb"] = recb
                                return emit

                            def make_t3(h=h, si=si, av_ps=av_ps, box=None):
                                def emit():
                                    nc.vector.tensor_mul(
                                        out=avts[h][:, si * 512:
                                                    (si + 1) * 512],
                                        in0=av_ps[:], in1=box["recb"][:])
                                return emit

                            box = {}
                            tails.append((gp + 1, make_t1(box=box)))
                            tails.append((gp + 2, make_t2(box=box)))
                            tails.append((gp + 3, make_t3(box=box)))
                        emit_units(len(units))
                        drain_evacs()
                    flush_tails(10 ** 9)

                    # ---------- output projection (cb-outer, wp streamed) ---
                    gi = 0
                    for cb in range(NCB):
                        if cb + 1 < NCB:
                            wp_next = {}
                            for f in range(HL):
                                t_ = wp_pool.tile([P, 512], f16,
                                                  tag=f"wp{f}",
                                                  name=f"wp{f}_{cb + 1}")
                                nc.sync.dma_start(
                                    t_[:], wp_d[f][:, (cb + 1) * 512:
                                                   (cb + 2) * 512])
                                wp_next[f] = t_
                        for tch in range(NTC):
                            pool = avp if gi % 2 == 0 else pjp
                            tag = "av" if gi % 2 == 0 else "pj"
                            ps = pool.tile([P, 512], f32, tag=tag,
                                           name=f"po{cb}_{tch}")
                            gi += 1
                            for f in range(HL):
                                nc.tensor.matmul(
                                    ps[:], avts[f][:, tch * P:(tch + 1) * P],
                                    wp_t[f][:, :],
                                    start=(f == 0), stop=(f == HL - 1))
                            ob = obp.tile([P, 512], f32, tag="ob")
                            evac(ob[:], ps[:])
                            nc.sync.dma_start(
                                out_d[tch * P:(tch + 1) * P,
                                      cb * 512:(cb + 1) * 512], ob[:])
                        if cb + 1 < NCB:
                            wp_t = wp_next
    nc.compile()
    return nc


def _make_mask():
    pp_ = np.arange(P)[:, None]
    ff = np.arange(P)[None, :]
    return np.where(ff >= pp_, 1.0, 0.0).astype(np_f16)


def _prep_inputs(x, w_qkv, w_proj):
    mask = _make_mask()
    per_g = {}
    for g in range(2):
        q = w_qkv[:, g * FL:(g + 1) * FL]
        k = w_qkv[:, C + g * FL:C + (g + 1) * FL]
        v = w_qkv[:, 2 * C + g * FL:2 * C + (g + 1) * FL]
        wqk_cat = np.concatenate([q, k], axis=1)  # [C, 2048]
        wqk_p = np.ascontiguousarray(
            wqk_cat.reshape(NCC, P, 16, P).transpose(2, 1, 0, 3)
            .reshape(16, P, C)).astype(np_f16)
        wv_p = np.ascontiguousarray(v.reshape(NCC, P, FL)).astype(np_f16)
        wp_p = np.ascontiguousarray(
            w_proj[g * FL:(g + 1) * FL, :].reshape(HL, P, C)).astype(np_f16)
        per_g[g] = (wqk_p, wv_p, wp_p)
    in_maps = []
    for core in range(NCORES):
        b, g = core // 2, core % 2
        wqk_p, wv_p, wp_p = per_g[g]
        in_maps.append({
            "xt": np.ascontiguousarray(x[b].T).astype(np_f16),
            "wqk": wqk_p,
            "wv": wv_p,
            "wp": wp_p,
            "mask": mask,
        })
    return in_maps


_nc_cache = None
last_results = None  # BassKernelResults of the most recent run (for test.py)


def kernel(x, w_qkv, w_proj):
    global _nc_cache, last_results
    from concourse.bass_utils import run_bass_kernel_spmd

    x = np.asarray(x, dtype=np.float32)
    w_qkv = np.asarray(w_qkv, dtype=np.float32)
    w_proj = np.asarray(w_proj, dtype=np.float32)

    if _nc_cache is None:
        _nc_cache = build_nc()
    nc = _nc_cache

    in_maps = _prep_inputs(x, w_qkv, w_proj)
    trace = bool(int(os.environ.get("KERNEL_TRACE", "0")))
    res = run_bass_kernel_spmd(nc, in_maps, list(range(NCORES)), trace=trace)
    last_results = res

    out = np.empty((B, T, C), dtype=np.float32)
    for b in range(B):
        out[b] = res.results[2 * b]["out"] + res.results[2 * b + 1]["out"]
    return out


# revision 8
# speedup vs baseline: 1.1932x; 1.0035x over previous
"""Causal self-attention Trainium2 Bass kernel, v4.

B=4, T=2048, C=2048, H=16, D=128, fp32 I/O. DP=4 x TP=2 (Megatron
head-group split); host sums TP pairs.

Per-core structure:
  [QK proj h0] [V proj] [fused: attn(h) + QK proj(h+1)] x8 [out proj]

- f16 activations/weights, fp32 PSUM accumulation everywhere.
- Fully SBUF-resident intermediates; q/k tiles rotate (lifetime ~2 head
  slots), wqk weight tiles stream with bufs=4, wp streams per 512-col
  block during the cb-outer output projection.
- Attention: score pairs packed into [128,1024] PSUM tiles -> one exp
  per pair; exp tiles merged into per-si accumulators S_a (DVE, even
  pairs) / S_b (gpsimd, odd pairs); per-si two ones-matmuls reduce them
  into sum[1,512]; reciprocal -> partition_broadcast -> normalize fused
  into the av PSUM->SBUF evacuation.
- QK projection of head h+1 interleaves between attention pairs of
  head h (4-matmul units, cc-contiguous into one PSUM bank) so the PE
  never waits on ACT exp.
- PSUM fused phase: sc pair 2 + av 3 + proj 2 + sum 1 = 8 banks; the
  output projection reuses the av/proj pools.
"""

import math
import os
import sys

import numpy as np

for _p in ("/opt/trn_rl_repo",):
    if _p not in sys.path:
        sys.path.insert(0, _p)

import ml_dtypes
import concourse.bass as bass
import concourse.mybir as mybir
from concourse import bacc
from concourse.tile import TileContext

B, T, C, H, D = 4, 2048, 2048, 16, 128
P = 128
NCORES = 8
HL = 8           # heads per core
FL = HL * D      # local feature dim = 1024
NCC = C // P     # 16 contraction chunks
NTC = T // P     # 16 t chunks
NSB = T // 512   # 4 t superblocks
NCB = C // 512   # 4 output col blocks
EXP_SCALE = 1.0 / math.sqrt(D)

f32 = mybir.dt.float32
f16 = mybir.dt.float16
np_f16 = np.float16


def _pair_layout(si, pr):
    """Packing of score pair pr (j-chunks 2pr, 2pr+1) of superblock si into a
    [128, 1024] PSUM tile. Returns [(jj, d_off, col_off, width), ...]."""
    js = (2 * pr, 2 * pr + 1)
    d0 = max(0, js[0] * P - si * 512)
    w0 = 512 - d0
    d1 = max(0, js[1] * P - si * 512)
    w1 = 512 - d1
    o1 = w0 if (w0 + w1) <= 512 else 512
    return [(js[0], d0, 0, w0), (js[1], d1, o1, w1)]


def build_nc():
    nc = bacc.Bacc()
    xt_d = nc.declare_dram_parameter("xt", [C, T], f16, isOutput=False)
    wqk_d = nc.declare_dram_parameter("wqk", [16, P, C], f16, isOutput=False)
    wv_d = nc.declare_dram_parameter("wv", [NCC, P, FL], f16, isOutput=False)
    wp_d = nc.declare_dram_parameter("wp", [HL, P, C], f16, isOutput=False)
    mask_d = nc.declare_dram_parameter("mask", [P, P], f16, isOutput=False)
    out_d = nc.declare_dram_parameter("out", [T, C], f32, isOutput=True)

    ACT = mybir.ActivationFunctionType

    with TileContext(nc) as tc:
        with tc.tile_pool(name="const", bufs=1) as cpool, \
             tc.tile_pool(name="avtp", bufs=1) as avt_pool:
            mask_sb = cpool.tile([P, P], f16)
            ones_sb = cpool.tile([P, 1], f16)
            nc.sync.dma_start(mask_sb[:], mask_d[:])
            # mask column 127 is all-ones; reuse it as the ones vector.
            nc.vector.tensor_copy(out=ones_sb[:], in_=mask_sb[:, P - 1:P])
            avts = [avt_pool.tile([P, T], f16, tag=f"avt{hh}",
                                  name=f"avt{hh}") for hh in range(HL)]

            state = {"cpi": 0}

            def evac(dst, src):
                if state["cpi"] % 2 == 0:
                    nc.vector.tensor_copy(out=dst, in_=src)
                else:
                    nc.scalar.copy(out=dst, in_=src)
                state["cpi"] += 1

            with tc.tile_pool(name="qkp", bufs=2) as qk_pool, \
                 tc.tile_pool(name="vp", bufs=1) as v_pool, \
                 tc.tile_pool(name="wqkp", bufs=4) as wqk_pool, \
                 tc.tile_pool(name="xtp", bufs=1) as xt_pool:
                vsb = [v_pool.tile([P, FL], f16, tag=f"v{tb}", name=f"v{tb}")
                       for tb in range(NTC)]
                xts = [xt_pool.tile([P, T], f16, tag=f"xt{cc}",
                                    name=f"xt{cc}") for cc in range(NCC)]

                qk_tiles = {}   # j -> rotating [P, T] tile
                wqk_tiles = {}  # j -> rotating [P, C] weight tile

                def fetch_wqk(j):
                    wt = wqk_pool.tile([P, C], f16, tag="wqk",
                                       name=f"wqk{j}")
                    nc.sync.dma_start(wt[:], wqk_d[j])
                    wqk_tiles[j] = wt

                def new_qk_tile(j):
                    t_ = qk_pool.tile([P, T], f16,
                                      tag="q" if j < HL else "k",
                                      name=f"qk{j}")
                    qk_tiles[j] = t_
                    return t_

                # DMA order: head-0 weights first (gates the first matmul),
                # then xt split across sync+scalar queues, wv halves behind
                # the xt halves, remaining wqk last on sync.
                fetch_wqk(0)
                fetch_wqk(8)
                for cc in range(8):
                    nc.sync.dma_start(xts[cc][:], xt_d[cc * P:(cc + 1) * P, :])
                for cc in range(8, NCC):
                    nc.scalar.dma_start(xts[cc][:],
                                        xt_d[cc * P:(cc + 1) * P, :])
                with tc.tile_pool(name="wvp", bufs=1) as wv_pool:
                    wvs = [wv_pool.tile([P, FL], f16, tag=f"wv{cc}",
                                        name=f"wv{cc}") for cc in range(NCC)]
                    for cc in range(8):
                        nc.sync.dma_start(wvs[cc][:], wv_d[cc])
                    for cc in range(8, NCC):
                        nc.scalar.dma_start(wvs[cc][:], wv_d[cc])
                    fetch_wqk(1)
                    fetch_wqk(9)

                    with tc.tile_pool(name="pp", bufs=8, space="PSUM") as pp:
                        # ---------- QK projection, head 0 ----------
                        for j in (0, 8):
                            dst = new_qk_tile(j)
                            for sb in range(NSB):
                                ps = pp.tile([P, 512], f32, tag="pp",
                                             name=f"pj{j}_{sb}")
                                for cc in range(NCC):
                                    nc.tensor.matmul(
                                        ps[:],
                                        wqk_tiles[j][:, cc * P:(cc + 1) * P],
                                        xts[cc][:, sb * 512:(sb + 1) * 512],
                                        start=(cc == 0), stop=(cc == NCC - 1))
                                evac(dst[:, sb * 512:(sb + 1) * 512], ps[:])

                        # ---------- V projection ----------
                        for tb in range(NTC):
                            for vb in range(2):
                                ps = pp.tile([P, 512], f32, tag="pp",
                                             name=f"pv{tb}_{vb}")
                                for cc in range(NCC):
                                    nc.tensor.matmul(
                                        ps[:], xts[cc][:, tb * P:(tb + 1) * P],
                                        wvs[cc][:, vb * 512:(vb + 1) * 512],
                                        start=(cc == 0), stop=(cc == NCC - 1))
                                evac(vsb[tb][:, vb * 512:(vb + 1) * 512],
                                     ps[:])
                # wvs + pp freed

                # ---------- fused attention + next-head QK proj ----------
                with tc.tile_pool(name="scp", bufs=1, space="PSUM") as scp, \
                     tc.tile_pool(name="avp", bufs=3, space="PSUM") as avp, \
                     tc.tile_pool(name="pjp", bufs=2, space="PSUM") as pjp, \
                     tc.tile_pool(name="sup", bufs=1, space="PSUM") as sup, \
                     tc.tile_pool(name="etp", bufs=3) as etp, \
                     tc.tile_pool(name="Sp", bufs=2) as Sp, \
                     tc.tile_pool(name="stp", bufs=2) as stp, \
                     tc.tile_pool(name="wpp", bufs=2) as wp_pool, \
                     tc.tile_pool(name="obp", bufs=4) as obp:
                    # deferred si tails: each tail (sum-MMs, recip, broadcast,
                    # normalize) is emitted two pairs after its si completes
                    # so the in-order PE stream never blocks on the S chains.
                    gp = 0            # global pair counter
                    tails = []        # (ready_at_gp, emit_fn)

                    def flush_tails(now):
                        while tails and tails[0][0] <= now:
                            tails.pop(0)[1]()

                    for h in range(HL):
                        qt, kt = qk_tiles[h], qk_tiles[HL + h]
                        # prefetch weights for head h+2's projection
                        if h + 2 < HL:
                            fetch_wqk(h + 2)
                            fetch_wqk(HL + h + 2)
                        # proj work units for head h+1
                        units = []
                        if h + 1 < HL:
                            for j in (h + 1, HL + h + 1):
                                new_qk_tile(j)
                                for sb in range(NSB):
                                    for cc0 in range(0, NCC, 4):
                                        units.append((j, sb, cc0))
                        if h == HL - 1:
                            # prefetch wp column-block 0 for the output
                            # projection (scalar queue is idle here)
                            wp_t = {}
                            for f in range(HL):
                                t_ = wp_pool.tile([P, 512], f16,
                                                  tag=f"wp{f}",
                                                  name=f"wp{f}_0")
                                nc.scalar.dma_start(t_[:], wp_d[f][:, 0:512])
                                wp_t[f] = t_
                        pjt = {}
                        ui = 0
                        pair_no = 0

                        evac_q = []

                        def drain_evacs():
                            for (j, sb, ps) in evac_q:
                                nc.vector.tensor_copy(
                                    out=qk_tiles[j][:, sb * 512:
                                                    (sb + 1) * 512],
                                    in_=ps[:])
                            del evac_q[:]

                        def emit_units(target):
                            # lazy evacs from the previous call: by now the
                            # group's matmuls have executed, so the DVE copy
                            # won't sit blocked at the head of the queue.
                            nonlocal ui
                            drain_evacs()
                            while ui < min(target, len(units)):
                                (j, sb, cc0) = units[ui]
                                if cc0 == 0:
                                    pjt[(j, sb)] = pjp.tile(
                                        [P, 512], f32, tag="pj",
                                        name=f"pj{j}_{sb}")
                                ps = pjt[(j, sb)]
                                for cc in range(cc0, cc0 + 4):
                                    nc.tensor.matmul(
                                        ps[:],
                                        wqk_tiles[j][:, cc * P:(cc + 1) * P],
                                        xts[cc][:, sb * 512:(sb + 1) * 512],
                                        start=(cc == 0), stop=(cc == NCC - 1))
                                if cc0 == NCC - 4:
                                    evac_q.append((j, sb, pjt.pop((j, sb))))
                                ui += 1

                        for si in range(NSB):
                            njc = 4 * si + 4
                            npr = njc // 2
                            av_ps = avp.tile([P, 512], f32, tag="av")
                            S_a = Sp.tile([P, 512], f16, tag="Sa")
                            S_b = None
                            if si >= 1:  # gpsimd handles pairs pr%4==3
                                S_b = Sp.tile([P, 512], f16, tag="Sb")
                                nc.gpsimd.memset(S_b[:], 0.0)
                            first_a = True
                            pend = None
                            for pr in range(npr):
                                infos = _pair_layout(si, pr)
                                sc = scp.tile([P, 1024], f32, tag="sc")
                                for (jj, dd, oo, ww) in infos:
                                    nc.tensor.matmul(
                                        sc[:, oo:oo + ww],
                                        kt[:, jj * P:(jj + 1) * P],
                                        qt[:, si * 512 + dd:(si + 1) * 512],
                                        start=True, stop=True)
                                et = etp.tile([P, 1024], f16, tag="et")
                                end = infos[-1][2] + infos[-1][3]
                                nc.scalar.activation(
                                    et[:, :end], sc[:, :end], ACT.Exp,
                                    scale=EXP_SCALE)
                                for (jj, dd, oo, ww) in infos:
                                    if jj >= 4 * si:  # diagonal 128-block
                                        nc.vector.tensor_mul(
                                            out=et[:, oo:oo + P],
                                            in0=et[:, oo:oo + P],
                                            in1=mask_sb[:])
                                # S accumulation: pr%4==3 pairs on gpsimd
                                # (into zeroed S_b), the rest on DVE (S_a).
                                if pr % 4 == 3:
                                    for (jj, dd, oo, ww) in infos:
                                        nc.gpsimd.tensor_add(
                                            out=S_b[:, dd:], in0=S_b[:, dd:],
                                            in1=et[:, oo:oo + ww])
                                else:
                                    for (jj, dd, oo, ww) in infos:
                                        if first_a:
                                            nc.vector.tensor_copy(
                                                out=S_a[:, dd:],
                                                in_=et[:, oo:oo + ww])
                                            first_a = False
                                        else:
                                            nc.vector.tensor_add(
                                                out=S_a[:, dd:],
                                                in0=S_a[:, dd:],
                                                in1=et[:, oo:oo + ww])
                                pair_no += 1
                                emit_units(3 * pair_no)
                                if pend is not None:
                                    pet, pinfos = pend
                                    for (jj, dd, oo, ww) in pinfos:
                                        nc.tensor.matmul(
                                            av_ps[:, dd:],
                                            vsb[jj][:, h * P:(h + 1) * P],
                                            pet[:, oo:oo + ww],
                                            start=(jj == 0), stop=False)
                                pend = (et, infos)
                                gp += 1
                                flush_tails(gp)
                            pet, pinfos = pend
                            for (jj, dd, oo, ww) in pinfos:
                                nc.tensor.matmul(
                                    av_ps[:, dd:],
                                    vsb[jj][:, h * P:(h + 1) * P],
                                    pet[:, oo:oo + ww],
                                    start=(jj == 0), stop=(jj == njc - 1))

                            # tail split into three flush points so each
                            # cross-engine hop is emitted only after its
                            # dependency has had time to complete (no
                            # head-of-line blocking in the in-order queues).
                            def make_t1(h=h, si=si, S_a=S_a, S_b=S_b,
                                        box=None):
                                def emit():
                                    sum_ps = sup.tile([1, 512], f32,
                                                      tag="sum",
                                                      name=f"sum{h}_{si}")
                                    nc.tensor.matmul(
                                        sum_ps[:], ones_sb[:], S_a[:],
                                        start=True, stop=(S_b is None))
                                    if S_b is not None:
                                        nc.tensor.matmul(
                                            sum_ps[:], ones_sb[:], S_b[:],
                                            start=False, stop=True)
                                    rec = stp.tile([1, 512], f32, tag="rec",
                                                   name=f"rec{h}_{si}")
                                    nc.vector.reciprocal_approx_fast(
                                        out=rec[:], in_=sum_ps[:])
                                    box["rec"] = rec
                                return emit

                            def make_t2(h=h, si=si, box=None):
                                def emit():
                                    recb = stp.tile([P, 512], f32, tag="recb",
                                                    name=f"recb{h}_{si}")
                                    nc.gpsimd.partition_broadcast(
                                        recb[:], box["rec"][:])
                                    box["recb"] = recb
                                return emit

                            def make_t3(h=h, si=si, av_ps=av_ps, box=None):
                                def emit():
                                    nc.vector.tensor_mul(
                                        out=avts[h][:, si * 512:
                                                    (si + 1) * 512],
                                        in0=av_ps[:], in1=box["recb"][:])
                                return emit

                            box = {}
                            tails.append((gp + 2, make_t1(box=box)))
                            tails.append((gp + 3, make_t2(box=box)))
                            tails.append((gp + 4, make_t3(box=box)))
                        emit_units(len(units))
                        drain_evacs()
                    flush_tails(10 ** 9)

                    # ---------- output projection (cb-outer, wp streamed) ---
                    gi = 0
                    for cb in range(NCB):
                        if cb + 1 < NCB:
                            wp_next = {}
                            for f in range(HL):
                                t_ = wp_pool.tile([P, 512], f16,
                                                  tag=f"wp{f}",
                                                  name=f"wp{f}_{cb + 1}")
                                nc.sync.dma_start(
                                    t_[:], wp_d[f][:, (cb + 1) * 512:
                                                   (cb + 2) * 512])
                                wp_next[f] = t_
                        for tch in range(NTC):
                            pool = avp if gi % 2 == 0 else pjp
                            tag = "av" if gi % 2 == 0 else "pj"
                            ps = pool.tile([P, 512], f32, tag=tag,
                                           name=f"po{cb}_{tch}")
                            gi += 1
                            for f in range(HL):
                                nc.tensor.matmul(
                                    ps[:], avts[f][:, tch * P:(tch + 1) * P],
                                    wp_t[f][:, :],
                                    start=(f == 0), stop=(f == HL - 1))
                            ob = obp.tile([P, 512], f32, tag="ob")
                            evac(ob[:], ps[:])
                            nc.sync.dma_start(
                                out_d[tch * P:(tch + 1) * P,
                                      cb * 512:(cb + 1) * 512], ob[:])
                        if cb + 1 < NCB:
                            wp_t = wp_next
    nc.compile()
    return nc


def _make_mask():
    pp_ = np.arange(P)[:, None]
    ff = np.arange(P)[None, :]
    return np.where(ff >= pp_, 1.0, 0.0).astype(np_f16)


def _prep_inputs(x, w_qkv, w_proj):
    mask = _make_mask()
    per_g = {}
    for g in range(2):
        q = w_qkv[:, g * FL:(g + 1) * FL]
        k = w_qkv[:, C + g * FL:C + (g + 1) * FL]
        v = w_qkv[:, 2 * C + g * FL:2 * C + (g + 1) * FL]
        wqk_cat = np.concatenate([q, k], axis=1)  # [C, 2048]
        wqk_p = np.ascontiguousarray(
            wqk_cat.reshape(NCC, P, 16, P).transpose(2, 1, 0, 3)
            .reshape(16, P, C)).astype(np_f16)
        wv_p = np.ascontiguousarray(v.reshape(NCC, P, FL)).astype(np_f16)
        wp_p = np.ascontiguousarray(
            w_proj[g * FL:(g + 1) * FL, :].reshape(HL, P, C)).astype(np_f16)
        per_g[g] = (wqk_p, wv_p, wp_p)
    in_maps = []
    for core in range(NCORES):
        b, g = core // 2, core % 2
        wqk_p, wv_p, wp_p = per_g[g]
        in_maps.append({
            "xt": np.ascontiguousarray(x[b].T).astype(np_f16),
            "wqk": wqk_p,
            "wv": wv_p,
            "wp": wp_p,
            "mask": mask,
        })
    return in_maps


_nc_cache = None
last_results = None  # BassKernelResults of the most recent run (for test.py)


def kernel(x, w_qkv, w_proj):
    global _nc_cache, last_results
    from concourse.bass_utils import run_bass_kernel_spmd

    x = np.asarray(x, dtype=np.float32)
    w_qkv = np.asarray(w_qkv, dtype=np.float32)
    w_proj = np.asarray(w_proj, dtype=np.float32)

    if _nc_cache is None:
        _nc_cache = build_nc()
    nc = _nc_cache

    in_maps = _prep_inputs(x, w_qkv, w_proj)
    trace = bool(int(os.environ.get("KERNEL_TRACE", "0")))
    res = run_bass_kernel_spmd(nc, in_maps, list(range(NCORES)), trace=trace)
    last_results = res

    out = np.empty((B, T, C), dtype=np.float32)
    for b in range(B):
        out[b] = res.results[2 * b]["out"] + res.results[2 * b + 1]["out"]
    return out


# revision 9
# speedup vs baseline: 1.3582x; 1.1383x over previous
"""Causal self-attention Trainium2 Bass kernel, v4.

B=4, T=2048, C=2048, H=16, D=128, fp32 I/O. DP=4 x TP=2 (Megatron
head-group split); host sums TP pairs.

Per-core structure:
  [QK proj h0] [V proj] [fused: attn(h) + QK proj(h+1)] x8 [out proj]

- f16 activations/weights, fp32 PSUM accumulation everywhere.
- Fully SBUF-resident intermediates; q/k tiles rotate (lifetime ~2 head
  slots), wqk weight tiles stream with bufs=4, wp streams per 512-col
  block during the cb-outer output projection.
- Attention: score pairs packed into [128,1024] PSUM tiles -> one exp
  per pair; exp tiles merged into per-si accumulators S_a (DVE, even
  pairs) / S_b (gpsimd, odd pairs); per-si two ones-matmuls reduce them
  into sum[1,512]; reciprocal -> partition_broadcast -> normalize fused
  into the av PSUM->SBUF evacuation.
- QK projection of head h+1 interleaves between attention pairs of
  head h (4-matmul units, cc-contiguous into one PSUM bank) so the PE
  never waits on ACT exp.
- PSUM fused phase: sc pair 2 + av 3 + proj 2 + sum 1 = 8 banks; the
  output projection reuses the av/proj pools.
"""

import math
import os
import sys

import numpy as np

for _p in ("/opt/trn_rl_repo",):
    if _p not in sys.path:
        sys.path.insert(0, _p)

import ml_dtypes
import concourse.bass as bass
import concourse.mybir as mybir
from concourse import bacc
from concourse.tile import TileContext

B, T, C, H, D = 4, 2048, 2048, 16, 128
P = 128
NCORES = 8
HL = 8           # heads per core
FL = HL * D      # local feature dim = 1024
NCC = C // P     # 16 contraction chunks
NTC = T // P     # 16 t chunks
NSB = T // 512   # 4 t superblocks
NCB = C // 512   # 4 output col blocks
EXP_SCALE = 1.0 / math.sqrt(D)

f32 = mybir.dt.float32
f16 = mybir.dt.float16
np_f16 = np.float16


def _pair_layout(si, pr):
    """Packing of score pair pr (j-chunks 2pr, 2pr+1) of superblock si into a
    [128, 1024] PSUM tile. Returns [(jj, d_off, col_off, width), ...]."""
    js = (2 * pr, 2 * pr + 1)
    d0 = max(0, js[0] * P - si * 512)
    w0 = 512 - d0
    d1 = max(0, js[1] * P - si * 512)
    w1 = 512 - d1
    o1 = w0 if (w0 + w1) <= 512 else 512
    return [(js[0], d0, 0, w0), (js[1], d1, o1, w1)]


def build_nc():
    nc = bacc.Bacc()
    xt_d = nc.declare_dram_parameter("xt", [C, T], f16, isOutput=False)
    wqk_d = nc.declare_dram_parameter("wqk", [16, P, C], f16, isOutput=False)
    wv_d = nc.declare_dram_parameter("wv", [NCC, P, FL], f16, isOutput=False)
    wp_d = nc.declare_dram_parameter("wp", [HL, P, C], f16, isOutput=False)
    mask_d = nc.declare_dram_parameter("mask", [P, P], f16, isOutput=False)
    out_d = nc.declare_dram_parameter("out", [T, C], f32, isOutput=True)

    ACT = mybir.ActivationFunctionType

    with TileContext(nc) as tc:
        with tc.tile_pool(name="const", bufs=1) as cpool, \
             tc.tile_pool(name="avtp", bufs=1) as avt_pool:
            mask_sb = cpool.tile([P, P], f16)
            ones_sb = cpool.tile([P, 1], f16)
            nc.sync.dma_start(mask_sb[:], mask_d[:])
            # mask column 127 is all-ones; reuse it as the ones vector.
            nc.vector.tensor_copy(out=ones_sb[:], in_=mask_sb[:, P - 1:P])
            avts = [avt_pool.tile([P, T], f16, tag=f"avt{hh}",
                                  name=f"avt{hh}") for hh in range(HL)]

            state = {"cpi": 0}

            def evac(dst, src):
                if state["cpi"] % 2 == 0:
                    nc.vector.tensor_copy(out=dst, in_=src)
                else:
                    nc.scalar.copy(out=dst, in_=src)
                state["cpi"] += 1

            with tc.tile_pool(name="qkp", bufs=2) as qk_pool, \
                 tc.tile_pool(name="vp", bufs=1) as v_pool, \
                 tc.tile_pool(name="wqkp", bufs=4) as wqk_pool, \
                 tc.tile_pool(name="xtp", bufs=1) as xt_pool:
                vsb = [v_pool.tile([P, FL], f16, tag=f"v{tb}", name=f"v{tb}")
                       for tb in range(NTC)]
                xts = [xt_pool.tile([P, T], f16, tag=f"xt{cc}",
                                    name=f"xt{cc}") for cc in range(NCC)]

                qk_tiles = {}   # j -> rotating [P, T] tile
                wqk_tiles = {}  # j -> rotating [P, C] weight tile

                def fetch_wqk(j):
                    wt = wqk_pool.tile([P, C], f16, tag="wqk",
                                       name=f"wqk{j}")
                    nc.sync.dma_start(wt[:], wqk_d[j])
                    wqk_tiles[j] = wt

                def new_qk_tile(j):
                    t_ = qk_pool.tile([P, T], f16,
                                      tag="q" if j < HL else "k",
                                      name=f"qk{j}")
                    qk_tiles[j] = t_
                    return t_

                # DMA order: head-0 weights first (gates the first matmul),
                # then xt split across sync+scalar queues, wv halves behind
                # the xt halves, remaining wqk last on sync.
                fetch_wqk(0)
                fetch_wqk(8)
                for cc in range(8):
                    nc.sync.dma_start(xts[cc][:], xt_d[cc * P:(cc + 1) * P, :])
                for cc in range(8, NCC):
                    nc.scalar.dma_start(xts[cc][:],
                                        xt_d[cc * P:(cc + 1) * P, :])
                with tc.tile_pool(name="wvp", bufs=1) as wv_pool:
                    wvs = [wv_pool.tile([P, FL], f16, tag=f"wv{cc}",
                                        name=f"wv{cc}") for cc in range(NCC)]
                    for cc in range(8):
                        nc.sync.dma_start(wvs[cc][:], wv_d[cc])
                    for cc in range(8, NCC):
                        nc.scalar.dma_start(wvs[cc][:], wv_d[cc])
                    fetch_wqk(1)
                    fetch_wqk(9)

                    with tc.tile_pool(name="pp", bufs=8, space="PSUM") as pp:
                        # ---------- QK projection, head 0 ----------
                        for j in (0, 8):
                            dst = new_qk_tile(j)
                            for sb in range(NSB):
                                ps = pp.tile([P, 512], f32, tag="pp",
                                             name=f"pj{j}_{sb}")
                                for cc in range(NCC):
                                    nc.tensor.matmul(
                                        ps[:],
                                        wqk_tiles[j][:, cc * P:(cc + 1) * P],
                                        xts[cc][:, sb * 512:(sb + 1) * 512],
                                        start=(cc == 0), stop=(cc == NCC - 1))
                                evac(dst[:, sb * 512:(sb + 1) * 512], ps[:])

                        # ---------- V projection ----------
                        for tb in range(NTC):
                            for vb in range(2):
                                ps = pp.tile([P, 512], f32, tag="pp",
                                             name=f"pv{tb}_{vb}")
                                for cc in range(NCC):
                                    nc.tensor.matmul(
                                        ps[:], xts[cc][:, tb * P:(tb + 1) * P],
                                        wvs[cc][:, vb * 512:(vb + 1) * 512],
                                        start=(cc == 0), stop=(cc == NCC - 1))
                                evac(vsb[tb][:, vb * 512:(vb + 1) * 512],
                                     ps[:])
                # wvs + pp freed

                # ---------- fused attention + next-head QK proj ----------
                with tc.tile_pool(name="scp", bufs=1, space="PSUM") as scp, \
                     tc.tile_pool(name="avp", bufs=2, space="PSUM") as avp, \
                     tc.tile_pool(name="pjp", bufs=2, space="PSUM") as pjp, \
                     tc.tile_pool(name="sup", bufs=2, space="PSUM") as sup, \
                     tc.tile_pool(name="etp", bufs=3) as etp, \
                     tc.tile_pool(name="Sp", bufs=2) as Sp, \
                     tc.tile_pool(name="stp", bufs=2) as stp, \
                     tc.tile_pool(name="wpp", bufs=2) as wp_pool, \
                     tc.tile_pool(name="obp", bufs=4) as obp:
                    # deferred si tails: each tail (sum-MMs, recip, broadcast,
                    # normalize) is emitted two pairs after its si completes
                    # so the in-order PE stream never blocks on the S chains.
                    gp = 0            # global pair counter
                    tails = []        # (ready_at_gp, emit_fn)

                    def flush_tails(now):
                        while tails and tails[0][0] <= now:
                            tails.pop(0)[1]()

                    for h in range(HL):
                        qt, kt = qk_tiles[h], qk_tiles[HL + h]
                        # prefetch weights for head h+2's projection
                        if h + 2 < HL:
                            fetch_wqk(h + 2)
                            fetch_wqk(HL + h + 2)
                        # proj work units for head h+1
                        units = []
                        if h + 1 < HL:
                            for j in (h + 1, HL + h + 1):
                                new_qk_tile(j)
                                for sb in range(NSB):
                                    for cc0 in range(0, NCC, 4):
                                        units.append((j, sb, cc0))
                        if h == HL - 1:
                            # prefetch wp column-block 0 for the output
                            # projection (scalar queue is idle here)
                            wp_t = {}
                            for f in range(HL):
                                t_ = wp_pool.tile([P, 512], f16,
                                                  tag=f"wp{f}",
                                                  name=f"wp{f}_0")
                                nc.scalar.dma_start(t_[:], wp_d[f][:, 0:512])
                                wp_t[f] = t_
                        pjt = {}
                        ui = 0
                        pair_no = 0

                        evac_q = []

                        def drain_evacs():
                            for (j, sb, ps) in evac_q:
                                nc.vector.tensor_copy(
                                    out=qk_tiles[j][:, sb * 512:
                                                    (sb + 1) * 512],
                                    in_=ps[:])
                            del evac_q[:]

                        def emit_units(target):
                            # lazy evacs from the previous call: by now the
                            # group's matmuls have executed, so the DVE copy
                            # won't sit blocked at the head of the queue.
                            nonlocal ui
                            drain_evacs()
                            while ui < min(target, len(units)):
                                (j, sb, cc0) = units[ui]
                                if cc0 == 0:
                                    pjt[(j, sb)] = pjp.tile(
                                        [P, 512], f32, tag="pj",
                                        name=f"pj{j}_{sb}")
                                ps = pjt[(j, sb)]
                                for cc in range(cc0, cc0 + 4):
                                    nc.tensor.matmul(
                                        ps[:],
                                        wqk_tiles[j][:, cc * P:(cc + 1) * P],
                                        xts[cc][:, sb * 512:(sb + 1) * 512],
                                        start=(cc == 0), stop=(cc == NCC - 1))
                                if cc0 == NCC - 4:
                                    evac_q.append((j, sb, pjt.pop((j, sb))))
                                ui += 1

                        for si in range(NSB):
                            njc = 4 * si + 4
                            npr = njc // 2
                            av_ps = avp.tile([P, 512], f32, tag="av")
                            sum_ps = sup.tile([1, 512], f32, tag="sum")
                            pend = None
                            for pr in range(npr):
                                infos = _pair_layout(si, pr)
                                sc = scp.tile([P, 1024], f32, tag="sc")
                                for (jj, dd, oo, ww) in infos:
                                    nc.tensor.matmul(
                                        sc[:, oo:oo + ww],
                                        kt[:, jj * P:(jj + 1) * P],
                                        qt[:, si * 512 + dd:(si + 1) * 512],
                                        start=True, stop=True)
                                et = etp.tile([P, 1024], f16, tag="et")
                                end = infos[-1][2] + infos[-1][3]
                                nc.scalar.activation(
                                    et[:, :end], sc[:, :end], ACT.Exp,
                                    scale=EXP_SCALE)
                                for (jj, dd, oo, ww) in infos:
                                    if jj >= 4 * si:  # diagonal 128-block
                                        nc.vector.tensor_mul(
                                            out=et[:, oo:oo + P],
                                            in0=et[:, oo:oo + P],
                                            in1=mask_sb[:])
                                pair_no += 1
                                emit_units(3 * pair_no)
                                if pend is not None:
                                    pet, pinfos = pend
                                    for (jj, dd, oo, ww) in pinfos:
                                        nc.tensor.matmul(
                                            av_ps[:, dd:],
                                            vsb[jj][:, h * P:(h + 1) * P],
                                            pet[:, oo:oo + ww],
                                            start=(jj == 0), stop=False)
                                    for (jj, dd, oo, ww) in pinfos:
                                        nc.tensor.matmul(
                                            sum_ps[:, dd:], ones_sb[:],
                                            pet[:, oo:oo + ww],
                                            start=(jj == 0), stop=False)
                                pend = (et, infos)
                                gp += 1
                                flush_tails(gp)
                            pet, pinfos = pend
                            for (jj, dd, oo, ww) in pinfos:
                                nc.tensor.matmul(
                                    av_ps[:, dd:],
                                    vsb[jj][:, h * P:(h + 1) * P],
                                    pet[:, oo:oo + ww],
                                    start=(jj == 0), stop=(jj == njc - 1))
                            for (jj, dd, oo, ww) in pinfos:
                                nc.tensor.matmul(
                                    sum_ps[:, dd:], ones_sb[:],
                                    pet[:, oo:oo + ww],
                                    start=(jj == 0), stop=(jj == njc - 1))

                            # tail: recip -> broadcast -> normalize, each
                            # emitted with growing slack so no in-order queue
                            # ever blocks on a cross-engine dependency.
                            def make_t1(h=h, si=si, sum_ps=sum_ps, box=None):
                                def emit():
                                    rec = stp.tile([1, 512], f32, tag="rec",
                                                   name=f"rec{h}_{si}")
                                    nc.vector.reciprocal_approx_fast(
                                        out=rec[:], in_=sum_ps[:])
                                    box["rec"] = rec
                                return emit

                            def make_t2(h=h, si=si, box=None):
                                def emit():
                                    recb = stp.tile([P, 512], f32, tag="recb",
                                                    name=f"recb{h}_{si}")
                                    nc.gpsimd.partition_broadcast(
                                        recb[:], box["rec"][:])
                                    box["recb"] = recb
                                return emit

                            def make_t3(h=h, si=si, av_ps=av_ps, box=None):
                                def emit():
                                    nc.vector.tensor_mul(
                                        out=avts[h][:, si * 512:
                                                    (si + 1) * 512],
                                        in0=av_ps[:], in1=box["recb"][:])
                                return emit

                            box = {}
                            tails.append((gp + 2, make_t1(box=box)))
                            tails.append((gp + 3, make_t2(box=box)))
                            tails.append((gp + 4, make_t3(box=box)))
                        emit_units(len(units))
                        drain_evacs()
                    flush_tails(10 ** 9)

                    # ---------- output projection (cb-outer, wp streamed) ---
                    gi = 0
                    for cb in range(NCB):
                        if cb + 1 < NCB:
                            wp_next = {}
                            for f in range(HL):
                                t_ = wp_pool.tile([P, 512], f16,
                                                  tag=f"wp{f}",
                                                  name=f"wp{f}_{cb + 1}")
                                nc.sync.dma_start(
                                    t_[:], wp_d[f][:, (cb + 1) * 512:
                                                   (cb + 2) * 512])
                                wp_next[f] = t_
                        for tch in range(NTC):
                            pool = avp if gi % 2 == 0 else pjp
                            tag = "av" if gi % 2 == 0 else "pj"
                            ps = pool.tile([P, 512], f32, tag=tag,
                                           name=f"po{cb}_{tch}")
                            gi += 1
                            for f in range(HL):
                                nc.tensor.matmul(
                                    ps[:], avts[f][:, tch * P:(tch + 1) * P],
                                    wp_t[f][:, :],
                                    start=(f == 0), stop=(f == HL - 1))
                            ob = obp.tile([P, 512], f32, tag="ob")
                            evac(ob[:], ps[:])
                            nc.sync.dma_start(
                                out_d[tch * P:(tch + 1) * P,
                                      cb * 512:(cb + 1) * 512], ob[:])
                        if cb + 1 < NCB:
                            wp_t = wp_next
    nc.compile()
    return nc


def _make_mask():
    pp_ = np.arange(P)[:, None]
    ff = np.arange(P)[None, :]
    return np.where(ff >= pp_, 1.0, 0.0).astype(np_f16)


def _prep_inputs(x, w_qkv, w_proj):
    mask = _make_mask()
    per_g = {}
    for g in range(2):
        q = w_qkv[:, g * FL:(g + 1) * FL]
        k = w_qkv[:, C + g * FL:C + (g + 1) * FL]
        v = w_qkv[:, 2 * C + g * FL:2 * C + (g + 1) * FL]
        wqk_cat = np.concatenate([q, k], axis=1)  # [C, 2048]
        wqk_p = np.ascontiguousarray(
            wqk_cat.reshape(NCC, P, 16, P).transpose(2, 1, 0, 3)
            .reshape(16, P, C)).astype(np_f16)
        wv_p = np.ascontiguousarray(v.reshape(NCC, P, FL)).astype(np_f16)
        wp_p = np.ascontiguousarray(
            w_proj[g * FL:(g + 1) * FL, :].reshape(HL, P, C)).astype(np_f16)
        per_g[g] = (wqk_p, wv_p, wp_p)
    in_maps = []
    for core in range(NCORES):
        b, g = core // 2, core % 2
        wqk_p, wv_p, wp_p = per_g[g]
        in_maps.append({
            "xt": np.ascontiguousarray(x[b].T).astype(np_f16),
            "wqk": wqk_p,
            "wv": wv_p,
            "wp": wp_p,
            "mask": mask,
        })
    return in_maps


_nc_cache = None
last_results = None  # BassKernelResults of the most recent run (for test.py)


def kernel(x, w_qkv, w_proj):
    global _nc_cache, last_results
    from concourse.bass_utils import run_bass_kernel_spmd

    x = np.asarray(x, dtype=np.float32)
    w_qkv = np.asarray(w_qkv, dtype=np.float32)
    w_proj = np.asarray(w_proj, dtype=np.float32)

    if _nc_cache is None:
        _nc_cache = build_nc()
    nc = _nc_cache

    in_maps = _prep_inputs(x, w_qkv, w_proj)
    trace = bool(int(os.environ.get("KERNEL_TRACE", "0")))
    res = run_bass_kernel_spmd(nc, in_maps, list(range(NCORES)), trace=trace)
    last_results = res

    out = np.empty((B, T, C), dtype=np.float32)
    for b in range(B):
        out[b] = res.results[2 * b]["out"] + res.results[2 * b + 1]["out"]
    return out


# revision 10
# speedup vs baseline: 1.3584x; 1.0001x over previous
"""Causal self-attention Trainium2 Bass kernel, v4.

B=4, T=2048, C=2048, H=16, D=128, fp32 I/O. DP=4 x TP=2 (Megatron
head-group split); host sums TP pairs.

Per-core structure:
  [QK proj h0] [V proj] [fused: attn(h) + QK proj(h+1)] x8 [out proj]

- f16 activations/weights, fp32 PSUM accumulation everywhere.
- Fully SBUF-resident intermediates; q/k tiles rotate (lifetime ~2 head
  slots), wqk weight tiles stream with bufs=4, wp streams per 512-col
  block during the cb-outer output projection.
- Attention: score pairs packed into [128,1024] PSUM tiles -> one exp
  per pair; exp tiles merged into per-si accumulators S_a (DVE, even
  pairs) / S_b (gpsimd, odd pairs); per-si two ones-matmuls reduce them
  into sum[1,512]; reciprocal -> partition_broadcast -> normalize fused
  into the av PSUM->SBUF evacuation.
- QK projection of head h+1 interleaves between attention pairs of
  head h (4-matmul units, cc-contiguous into one PSUM bank) so the PE
  never waits on ACT exp.
- PSUM fused phase: sc pair 2 + av 3 + proj 2 + sum 1 = 8 banks; the
  output projection reuses the av/proj pools.
"""

import math
import os
import sys

import numpy as np

for _p in ("/opt/trn_rl_repo",):
    if _p not in sys.path:
        sys.path.insert(0, _p)

import ml_dtypes
import concourse.bass as bass
import concourse.mybir as mybir
from concourse import bacc
from concourse.tile import TileContext

B, T, C, H, D = 4, 2048, 2048, 16, 128
P = 128
NCORES = 8
HL = 8           # heads per core
FL = HL * D      # local feature dim = 1024
NCC = C // P     # 16 contraction chunks
NTC = T // P     # 16 t chunks
NSB = T // 512   # 4 t superblocks
NCB = C // 512   # 4 output col blocks
EXP_SCALE = 1.0 / math.sqrt(D)

f32 = mybir.dt.float32
f16 = mybir.dt.float16
np_f16 = np.float16


def _pair_layout(si, pr):
    """Packing of score pair pr (j-chunks 2pr, 2pr+1) of superblock si into a
    [128, 1024] PSUM tile. Returns [(jj, d_off, col_off, width), ...]."""
    js = (2 * pr, 2 * pr + 1)
    d0 = max(0, js[0] * P - si * 512)
    w0 = 512 - d0
    d1 = max(0, js[1] * P - si * 512)
    w1 = 512 - d1
    o1 = w0 if (w0 + w1) <= 512 else 512
    return [(js[0], d0, 0, w0), (js[1], d1, o1, w1)]


def build_nc():
    nc = bacc.Bacc()
    xt_d = nc.declare_dram_parameter("xt", [C, T], f16, isOutput=False)
    wqk_d = nc.declare_dram_parameter("wqk", [16, P, C], f16, isOutput=False)
    wv_d = nc.declare_dram_parameter("wv", [NCC, P, FL], f16, isOutput=False)
    wp_d = nc.declare_dram_parameter("wp", [HL, P, C], f16, isOutput=False)
    mask_d = nc.declare_dram_parameter("mask", [P, P], f16, isOutput=False)
    out_d = nc.declare_dram_parameter("out", [T, C], f32, isOutput=True)

    ACT = mybir.ActivationFunctionType

    with TileContext(nc) as tc:
        with tc.tile_pool(name="const", bufs=1) as cpool, \
             tc.tile_pool(name="avtp", bufs=1) as avt_pool:
            mask_sb = cpool.tile([P, P], f16)
            ones_sb = cpool.tile([P, 1], f16)
            nc.sync.dma_start(mask_sb[:], mask_d[:])
            # mask column 127 is all-ones; reuse it as the ones vector.
            nc.vector.tensor_copy(out=ones_sb[:], in_=mask_sb[:, P - 1:P])
            # pre-load the Exp activation table so the first attention exp
            # doesn't pay the ACT_TABLE_LOAD mid-pipeline.
            warm = cpool.tile([P, 1], f16)
            nc.scalar.activation(warm[:], ones_sb[:], ACT.Exp, scale=1.0)
            avts = [avt_pool.tile([P, T], f16, tag=f"avt{hh}",
                                  name=f"avt{hh}") for hh in range(HL)]

            state = {"cpi": 0}

            def evac(dst, src):
                if state["cpi"] % 2 == 0:
                    nc.vector.tensor_copy(out=dst, in_=src)
                else:
                    nc.scalar.copy(out=dst, in_=src)
                state["cpi"] += 1

            with tc.tile_pool(name="qkp", bufs=2) as qk_pool, \
                 tc.tile_pool(name="vp", bufs=1) as v_pool, \
                 tc.tile_pool(name="wqkp", bufs=4) as wqk_pool, \
                 tc.tile_pool(name="xtp", bufs=1) as xt_pool:
                vsb = [v_pool.tile([P, FL], f16, tag=f"v{tb}", name=f"v{tb}")
                       for tb in range(NTC)]
                xts = [xt_pool.tile([P, T], f16, tag=f"xt{cc}",
                                    name=f"xt{cc}") for cc in range(NCC)]

                qk_tiles = {}   # j -> rotating [P, T] tile
                wqk_tiles = {}  # j -> rotating [P, C] weight tile

                def fetch_wqk(j):
                    wt = wqk_pool.tile([P, C], f16, tag="wqk",
                                       name=f"wqk{j}")
                    nc.sync.dma_start(wt[:], wqk_d[j])
                    wqk_tiles[j] = wt

                def new_qk_tile(j):
                    t_ = qk_pool.tile([P, T], f16,
                                      tag="q" if j < HL else "k",
                                      name=f"qk{j}")
                    qk_tiles[j] = t_
                    return t_

                # DMA order: head-0 weights first (gates the first matmul),
                # then xt split across sync+scalar queues, wv halves behind
                # the xt halves, remaining wqk last on sync.
                fetch_wqk(0)
                fetch_wqk(8)
                for cc in range(8):
                    nc.sync.dma_start(xts[cc][:], xt_d[cc * P:(cc + 1) * P, :])
                for cc in range(8, NCC):
                    nc.scalar.dma_start(xts[cc][:],
                                        xt_d[cc * P:(cc + 1) * P, :])
                with tc.tile_pool(name="wvp", bufs=1) as wv_pool:
                    wvs = [wv_pool.tile([P, FL], f16, tag=f"wv{cc}",
                                        name=f"wv{cc}") for cc in range(NCC)]
                    for cc in range(8):
                        nc.sync.dma_start(wvs[cc][:], wv_d[cc])
                    for cc in range(8, NCC):
                        nc.scalar.dma_start(wvs[cc][:], wv_d[cc])
                    fetch_wqk(1)
                    fetch_wqk(9)

                    with tc.tile_pool(name="pp", bufs=8, space="PSUM") as pp:
                        # ---------- QK projection, head 0 ----------
                        # cc order interleaves the two DMA queues' arrival
                        # order so the first groups march behind the input
                        # transfers instead of blocking on the last chunk.
                        cc_arr = [x for p_ in range(8) for x in (p_, 8 + p_)]
                        for j in (0, 8):
                            dst = new_qk_tile(j)
                            for sb in range(NSB):
                                ps = pp.tile([P, 512], f32, tag="pp",
                                             name=f"pj{j}_{sb}")
                                for ci, cc in enumerate(cc_arr):
                                    nc.tensor.matmul(
                                        ps[:],
                                        wqk_tiles[j][:, cc * P:(cc + 1) * P],
                                        xts[cc][:, sb * 512:(sb + 1) * 512],
                                        start=(ci == 0), stop=(ci == NCC - 1))
                                evac(dst[:, sb * 512:(sb + 1) * 512], ps[:])

                        # ---------- V projection ----------
                        for tb in range(NTC):
                            for vb in range(2):
                                ps = pp.tile([P, 512], f32, tag="pp",
                                             name=f"pv{tb}_{vb}")
                                for ci, cc in enumerate(cc_arr):
                                    nc.tensor.matmul(
                                        ps[:], xts[cc][:, tb * P:(tb + 1) * P],
                                        wvs[cc][:, vb * 512:(vb + 1) * 512],
                                        start=(ci == 0), stop=(ci == NCC - 1))
                                evac(vsb[tb][:, vb * 512:(vb + 1) * 512],
                                     ps[:])
                # wvs + pp freed

                # ---------- fused attention + next-head QK proj ----------
                with tc.tile_pool(name="scp", bufs=1, space="PSUM") as scp, \
                     tc.tile_pool(name="avp", bufs=2, space="PSUM") as avp, \
                     tc.tile_pool(name="pjp", bufs=2, space="PSUM") as pjp, \
                     tc.tile_pool(name="sup", bufs=2, space="PSUM") as sup, \
                     tc.tile_pool(name="etp", bufs=4) as etp, \
                     tc.tile_pool(name="Sp", bufs=2) as Sp, \
                     tc.tile_pool(name="stp", bufs=2) as stp, \
                     tc.tile_pool(name="wpp", bufs=2) as wp_pool, \
                     tc.tile_pool(name="obp", bufs=4) as obp:
                    # deferred si tails: each tail (sum-MMs, recip, broadcast,
                    # normalize) is emitted two pairs after its si completes
                    # so the in-order PE stream never blocks on the S chains.
                    gp = 0            # global pair counter
                    tails = []        # (ready_at_gp, emit_fn)

                    def flush_tails(now):
                        while tails and tails[0][0] <= now:
                            tails.pop(0)[1]()

                    for h in range(HL):
                        qt, kt = qk_tiles[h], qk_tiles[HL + h]
                        # prefetch weights for head h+2's projection
                        if h + 2 < HL:
                            fetch_wqk(h + 2)
                            fetch_wqk(HL + h + 2)
                        # proj work units for head h+1
                        units = []
                        if h + 1 < HL:
                            for j in (h + 1, HL + h + 1):
                                new_qk_tile(j)
                                for sb in range(NSB):
                                    for cc0 in range(0, NCC, 4):
                                        units.append((j, sb, cc0))
                        if h == HL - 1:
                            # prefetch wp column-block 0 for the output
                            # projection (scalar queue is idle here)
                            wp_t = {}
                            for f in range(HL):
                                t_ = wp_pool.tile([P, 512], f16,
                                                  tag=f"wp{f}",
                                                  name=f"wp{f}_0")
                                nc.scalar.dma_start(t_[:], wp_d[f][:, 0:512])
                                wp_t[f] = t_
                        pjt = {}
                        ui = 0
                        pair_no = 0

                        evac_q = []

                        def drain_evacs():
                            for (j, sb, ps) in evac_q:
                                nc.vector.tensor_copy(
                                    out=qk_tiles[j][:, sb * 512:
                                                    (sb + 1) * 512],
                                    in_=ps[:])
                            del evac_q[:]

                        def emit_units(target):
                            # lazy evacs from the previous call: by now the
                            # group's matmuls have executed, so the DVE copy
                            # won't sit blocked at the head of the queue.
                            nonlocal ui
                            drain_evacs()
                            while ui < min(target, len(units)):
                                (j, sb, cc0) = units[ui]
                                if cc0 == 0:
                                    pjt[(j, sb)] = pjp.tile(
                                        [P, 512], f32, tag="pj",
                                        name=f"pj{j}_{sb}")
                                ps = pjt[(j, sb)]
                                for cc in range(cc0, cc0 + 4):
                                    nc.tensor.matmul(
                                        ps[:],
                                        wqk_tiles[j][:, cc * P:(cc + 1) * P],
                                        xts[cc][:, sb * 512:(sb + 1) * 512],
                                        start=(cc == 0), stop=(cc == NCC - 1))
                                if cc0 == NCC - 4:
                                    evac_q.append((j, sb, pjt.pop((j, sb))))
                                ui += 1

                        for si in range(NSB):
                            njc = 4 * si + 4
                            npr = njc // 2
                            av_ps = avp.tile([P, 512], f32, tag="av")
                            sum_ps = sup.tile([1, 512], f32, tag="sum")
                            pend = None
                            for pr in range(npr):
                                infos = _pair_layout(si, pr)
                                sc = scp.tile([P, 1024], f32, tag="sc")
                                for (jj, dd, oo, ww) in infos:
                                    nc.tensor.matmul(
                                        sc[:, oo:oo + ww],
                                        kt[:, jj * P:(jj + 1) * P],
                                        qt[:, si * 512 + dd:(si + 1) * 512],
                                        start=True, stop=True)
                                et = etp.tile([P, 1024], f16, tag="et")
                                end = infos[-1][2] + infos[-1][3]
                                nc.scalar.activation(
                                    et[:, :end], sc[:, :end], ACT.Exp,
                                    scale=EXP_SCALE)
                                for (jj, dd, oo, ww) in infos:
                                    if jj >= 4 * si:  # diagonal 128-block
                                        nc.vector.tensor_mul(
                                            out=et[:, oo:oo + P],
                                            in0=et[:, oo:oo + P],
                                            in1=mask_sb[:])
                                pair_no += 1
                                emit_units(3 * pair_no)
                                if pend is not None:
                                    pet, pinfos = pend
                                    for (jj, dd, oo, ww) in pinfos:
                                        nc.tensor.matmul(
                                            av_ps[:, dd:],
                                            vsb[jj][:, h * P:(h + 1) * P],
                                            pet[:, oo:oo + ww],
                                            start=(jj == 0), stop=False)
                                    for (jj, dd, oo, ww) in pinfos:
                                        nc.tensor.matmul(
                                            sum_ps[:, dd:], ones_sb[:],
                                            pet[:, oo:oo + ww],
                                            start=(jj == 0), stop=False)
                                pend = (et, infos)
                                gp += 1
                                flush_tails(gp)
                            pet, pinfos = pend
                            for (jj, dd, oo, ww) in pinfos:
                                nc.tensor.matmul(
                                    av_ps[:, dd:],
                                    vsb[jj][:, h * P:(h + 1) * P],
                                    pet[:, oo:oo + ww],
                                    start=(jj == 0), stop=(jj == njc - 1))
                            for (jj, dd, oo, ww) in pinfos:
                                nc.tensor.matmul(
                                    sum_ps[:, dd:], ones_sb[:],
                                    pet[:, oo:oo + ww],
                                    start=(jj == 0), stop=(jj == njc - 1))

                            # tail: recip -> broadcast -> normalize, each
                            # emitted with growing slack so no in-order queue
                            # ever blocks on a cross-engine dependency.
                            def make_t1(h=h, si=si, sum_ps=sum_ps, box=None):
                                def emit():
                                    rec = stp.tile([1, 512], f32, tag="rec",
                                                   name=f"rec{h}_{si}")
                                    nc.vector.reciprocal_approx_fast(
                                        out=rec[:], in_=sum_ps[:])
                                    box["rec"] = rec
                                return emit

                            def make_t2(h=h, si=si, box=None):
                                def emit():
                                    recb = stp.tile([P, 512], f32, tag="recb",
                                                    name=f"recb{h}_{si}")
                                    nc.gpsimd.partition_broadcast(
                                        recb[:], box["rec"][:])
                                    box["recb"] = recb
                                return emit

                            def make_t3(h=h, si=si, av_ps=av_ps, box=None):
                                def emit():
                                    nc.vector.tensor_mul(
                                        out=avts[h][:, si * 512:
                                                    (si + 1) * 512],
                                        in0=av_ps[:], in1=box["recb"][:])
                                return emit

                            box = {}
                            tails.append((gp + 2, make_t1(box=box)))
                            tails.append((gp + 3, make_t2(box=box)))
                            tails.append((gp + 4, make_t3(box=box)))
                        emit_units(len(units))
                        drain_evacs()
                    flush_tails(10 ** 9)

                    # ---------- output projection (cb-outer, wp streamed) ---
                    gi = 0
                    for cb in range(NCB):
                        if cb + 1 < NCB:
                            wp_next = {}
                            for f in range(HL):
                                t_ = wp_pool.tile([P, 512], f16,
                                                  tag=f"wp{f}",
                                                  name=f"wp{f}_{cb + 1}")
                                nc.sync.dma_start(
                                    t_[:], wp_d[f][:, (cb + 1) * 512:
                                                   (cb + 2) * 512])
                                wp_next[f] = t_
                        for tch in range(NTC):
                            pool = avp if gi % 2 == 0 else pjp
                            tag = "av" if gi % 2 == 0 else "pj"
                            ps = pool.tile([P, 512], f32, tag=tag,
                                           name=f"po{cb}_{tch}")
                            gi += 1
                            for f in range(HL):
                                nc.tensor.matmul(
                                    ps[:], avts[f][:, tch * P:(tch + 1) * P],
                                    wp_t[f][:, :],
                                    start=(f == 0), stop=(f == HL - 1))
                            ob = obp.tile([P, 512], f32, tag="ob")
                            evac(ob[:], ps[:])
                            nc.sync.dma_start(
                                out_d[tch * P:(tch + 1) * P,
                                      cb * 512:(cb + 1) * 512], ob[:])
                        if cb + 1 < NCB:
                            wp_t = wp_next
    nc.compile()
    return nc


def _make_mask():
    pp_ = np.arange(P)[:, None]
    ff = np.arange(P)[None, :]
    return np.where(ff >= pp_, 1.0, 0.0).astype(np_f16)


def _prep_inputs(x, w_qkv, w_proj):
    mask = _make_mask()
    per_g = {}
    for g in range(2):
        q = w_qkv[:, g * FL:(g + 1) * FL]
        k = w_qkv[:, C + g * FL:C + (g + 1) * FL]
        v = w_qkv[:, 2 * C + g * FL:2 * C + (g + 1) * FL]
        wqk_cat = np.concatenate([q, k], axis=1)  # [C, 2048]
        wqk_p = np.ascontiguousarray(
            wqk_cat.reshape(NCC, P, 16, P).transpose(2, 1, 0, 3)
            .reshape(16, P, C)).astype(np_f16)
        wv_p = np.ascontiguousarray(v.reshape(NCC, P, FL)).astype(np_f16)
        wp_p = np.ascontiguousarray(
            w_proj[g * FL:(g + 1) * FL, :].reshape(HL, P, C)).astype(np_f16)
        per_g[g] = (wqk_p, wv_p, wp_p)
    in_maps = []
    for core in range(NCORES):
        b, g = core // 2, core % 2
        wqk_p, wv_p, wp_p = per_g[g]
        in_maps.append({
            "xt": np.ascontiguousarray(x[b].T).astype(np_f16),
            "wqk": wqk_p,
            "wv": wv_p,
            "wp": wp_p,
            "mask": mask,
        })
    return in_maps


_nc_cache = None
last_results = None  # BassKernelResults of the most recent run (for test.py)


def kernel(x, w_qkv, w_proj):
    global _nc_cache, last_results
    from concourse.bass_utils import run_bass_kernel_spmd

    x = np.asarray(x, dtype=np.float32)
    w_qkv = np.asarray(w_qkv, dtype=np.float32)
    w_proj = np.asarray(w_proj, dtype=np.float32)

    if _nc_cache is None:
        _nc_cache = build_nc()
    nc = _nc_cache

    in_maps = _prep_inputs(x, w_qkv, w_proj)
    trace = bool(int(os.environ.get("KERNEL_TRACE", "0")))
    res = run_bass_kernel_spmd(nc, in_maps, list(range(NCORES)), trace=trace)
    last_results = res

    out = np.empty((B, T, C), dtype=np.float32)
    for b in range(B):
        out[b] = res.results[2 * b]["out"] + res.results[2 * b + 1]["out"]
    return out


# revision 11
# speedup vs baseline: 1.3698x; 1.0084x over previous
"""Causal self-attention Trainium2 Bass kernel, v4.

B=4, T=2048, C=2048, H=16, D=128, fp32 I/O. DP=4 x TP=2 (Megatron
head-group split); host sums TP pairs.

Per-core structure:
  [QK proj h0] [V proj] [fused: attn(h) + QK proj(h+1)] x8 [out proj]

- f16 activations/weights, fp32 PSUM accumulation everywhere.
- Fully SBUF-resident intermediates; q/k tiles rotate (lifetime ~2 head
  slots), wqk weight tiles stream with bufs=4, wp streams per 512-col
  block during the cb-outer output projection.
- Attention: score pairs packed into [128,1024] PSUM tiles -> one exp
  per pair; exp tiles merged into per-si accumulators S_a (DVE, even
  pairs) / S_b (gpsimd, odd pairs); per-si two ones-matmuls reduce them
  into sum[1,512]; reciprocal -> partition_broadcast -> normalize fused
  into the av PSUM->SBUF evacuation.
- QK projection of head h+1 interleaves between attention pairs of
  head h (4-matmul units, cc-contiguous into one PSUM bank) so the PE
  never waits on ACT exp.
- PSUM fused phase: sc pair 2 + av 3 + proj 2 + sum 1 = 8 banks; the
  output projection reuses the av/proj pools.
"""

import math
import os
import sys

import numpy as np

for _p in ("/opt/trn_rl_repo",):
    if _p not in sys.path:
        sys.path.insert(0, _p)

import ml_dtypes
import concourse.bass as bass
import concourse.mybir as mybir
from concourse import bacc
from concourse.tile import TileContext

B, T, C, H, D = 4, 2048, 2048, 16, 128
P = 128
NCORES = 8
HL = 8           # heads per core
FL = HL * D      # local feature dim = 1024
NCC = C // P     # 16 contraction chunks
NTC = T // P     # 16 t chunks
NSB = T // 512   # 4 t superblocks
NCB = C // 512   # 4 output col blocks
EXP_SCALE = 1.0 / math.sqrt(D)

f32 = mybir.dt.float32
f16 = mybir.dt.float16
np_f16 = np.float16


def _pair_layout(si, pr):
    """Packing of score pair pr (j-chunks 2pr, 2pr+1) of superblock si into a
    [128, 1024] PSUM tile. Returns [(jj, d_off, col_off, width), ...]."""
    js = (2 * pr, 2 * pr + 1)
    d0 = max(0, js[0] * P - si * 512)
    w0 = 512 - d0
    d1 = max(0, js[1] * P - si * 512)
    w1 = 512 - d1
    o1 = w0 if (w0 + w1) <= 512 else 512
    return [(js[0], d0, 0, w0), (js[1], d1, o1, w1)]


def build_nc():
    nc = bacc.Bacc()
    xt_d = nc.declare_dram_parameter("xt", [C, T], f16, isOutput=False)
    wqk_d = nc.declare_dram_parameter("wqk", [16, P, C], f16, isOutput=False)
    wv_d = nc.declare_dram_parameter("wv", [NCC, P, FL], f16, isOutput=False)
    wp_d = nc.declare_dram_parameter("wp", [HL, P, C], f16, isOutput=False)
    mask_d = nc.declare_dram_parameter("mask", [P, P], f16, isOutput=False)
    out_d = nc.declare_dram_parameter("out", [T, C], f32, isOutput=True)

    ACT = mybir.ActivationFunctionType

    with TileContext(nc) as tc:
        with tc.tile_pool(name="const", bufs=1) as cpool, \
             tc.tile_pool(name="avtp", bufs=1) as avt_pool:
            mask_sb = cpool.tile([P, P], f16)
            ones_sb = cpool.tile([P, 1], f16)
            nc.sync.dma_start(mask_sb[:], mask_d[:])
            # mask column 127 is all-ones; reuse it as the ones vector.
            nc.vector.tensor_copy(out=ones_sb[:], in_=mask_sb[:, P - 1:P])
            # pre-load the Exp activation table so the first attention exp
            # doesn't pay the ACT_TABLE_LOAD mid-pipeline.
            warm = cpool.tile([P, 1], f16)
            nc.scalar.activation(warm[:], ones_sb[:], ACT.Exp, scale=1.0)
            avts = [avt_pool.tile([P, T], f16, tag=f"avt{hh}",
                                  name=f"avt{hh}") for hh in range(HL)]

            state = {"cpi": 0}

            def evac(dst, src):
                if state["cpi"] % 2 == 0:
                    nc.vector.tensor_copy(out=dst, in_=src)
                else:
                    nc.scalar.copy(out=dst, in_=src)
                state["cpi"] += 1

            with tc.tile_pool(name="qkp", bufs=2) as qk_pool, \
                 tc.tile_pool(name="vp", bufs=1) as v_pool, \
                 tc.tile_pool(name="wqkp", bufs=4) as wqk_pool, \
                 tc.tile_pool(name="xtp", bufs=1) as xt_pool:
                vsb = [v_pool.tile([P, FL], f16, tag=f"v{tb}", name=f"v{tb}")
                       for tb in range(NTC)]
                xts = [xt_pool.tile([P, T], f16, tag=f"xt{cc}",
                                    name=f"xt{cc}") for cc in range(NCC)]

                qk_tiles = {}   # j -> rotating [P, T] tile
                wqk_tiles = {}  # j -> rotating [P, C] weight tile

                def fetch_wqk(j):
                    wt = wqk_pool.tile([P, C], f16, tag="wqk",
                                       name=f"wqk{j}")
                    nc.sync.dma_start(wt[:], wqk_d[j])
                    wqk_tiles[j] = wt

                def new_qk_tile(j):
                    t_ = qk_pool.tile([P, T], f16,
                                      tag="q" if j < HL else "k",
                                      name=f"qk{j}")
                    qk_tiles[j] = t_
                    return t_

                # DMA order: head-0 weights first (gates the first matmul),
                # then xt split across sync+scalar queues, wv halves behind
                # the xt halves, remaining wqk last on sync.
                fetch_wqk(0)
                fetch_wqk(8)
                for cc in range(8):
                    nc.sync.dma_start(xts[cc][:], xt_d[cc * P:(cc + 1) * P, :])
                for cc in range(8, NCC):
                    nc.scalar.dma_start(xts[cc][:],
                                        xt_d[cc * P:(cc + 1) * P, :])
                with tc.tile_pool(name="wvp", bufs=1) as wv_pool:
                    wvs = [wv_pool.tile([P, FL], f16, tag=f"wv{cc}",
                                        name=f"wv{cc}") for cc in range(NCC)]
                    for cc in range(8):
                        nc.sync.dma_start(wvs[cc][:], wv_d[cc])
                    for cc in range(8, NCC):
                        nc.scalar.dma_start(wvs[cc][:], wv_d[cc])
                    fetch_wqk(1)
                    fetch_wqk(9)

                    with tc.tile_pool(name="pp", bufs=8, space="PSUM") as pp:
                        # PE warm-up: dummy matmuls on the (tiny, first-to-
                        # arrive) mask tile keep the PE busy through the
                        # input-DMA window so HAM is at full clock when the
                        # real projection starts. Nothing reads the result.
                        jp = pp.tile([P, 512], f32, tag="pp", name="warm")
                        for _w in range(80):
                            nc.tensor.matmul(jp[:, :P], mask_sb[:],
                                             mask_sb[:], start=True,
                                             stop=True)

                        # ---------- QK projection, head 0 ----------
                        # cc order interleaves the two DMA queues' arrival
                        # order so the first groups march behind the input
                        # transfers instead of blocking on the last chunk.
                        cc_arr = [x for p_ in range(8) for x in (p_, 8 + p_)]
                        for j in (0, 8):
                            dst = new_qk_tile(j)
                            for sb in range(NSB):
                                ps = pp.tile([P, 512], f32, tag="pp",
                                             name=f"pj{j}_{sb}")
                                for ci, cc in enumerate(cc_arr):
                                    nc.tensor.matmul(
                                        ps[:],
                                        wqk_tiles[j][:, cc * P:(cc + 1) * P],
                                        xts[cc][:, sb * 512:(sb + 1) * 512],
                                        start=(ci == 0), stop=(ci == NCC - 1))
                                evac(dst[:, sb * 512:(sb + 1) * 512], ps[:])

                        # ---------- V projection ----------
                        for tb in range(NTC):
                            for vb in range(2):
                                ps = pp.tile([P, 512], f32, tag="pp",
                                             name=f"pv{tb}_{vb}")
                                for ci, cc in enumerate(cc_arr):
                                    nc.tensor.matmul(
                                        ps[:], xts[cc][:, tb * P:(tb + 1) * P],
                                        wvs[cc][:, vb * 512:(vb + 1) * 512],
                                        start=(ci == 0), stop=(ci == NCC - 1))
                                evac(vsb[tb][:, vb * 512:(vb + 1) * 512],
                                     ps[:])
                # wvs + pp freed

                # ---------- fused attention + next-head QK proj ----------
                with tc.tile_pool(name="scp", bufs=1, space="PSUM") as scp, \
                     tc.tile_pool(name="avp", bufs=2, space="PSUM") as avp, \
                     tc.tile_pool(name="pjp", bufs=2, space="PSUM") as pjp, \
                     tc.tile_pool(name="sup", bufs=2, space="PSUM") as sup, \
                     tc.tile_pool(name="etp", bufs=4) as etp, \
                     tc.tile_pool(name="Sp", bufs=2) as Sp, \
                     tc.tile_pool(name="stp", bufs=2) as stp, \
                     tc.tile_pool(name="wpp", bufs=2) as wp_pool, \
                     tc.tile_pool(name="obp", bufs=4) as obp:
                    # deferred si tails: each tail (sum-MMs, recip, broadcast,
                    # normalize) is emitted two pairs after its si completes
                    # so the in-order PE stream never blocks on the S chains.
                    gp = 0            # global pair counter
                    tails = []        # (ready_at_gp, emit_fn)

                    def flush_tails(now):
                        while tails and tails[0][0] <= now:
                            tails.pop(0)[1]()

                    for h in range(HL):
                        qt, kt = qk_tiles[h], qk_tiles[HL + h]
                        # prefetch weights for head h+2's projection
                        if h + 2 < HL:
                            fetch_wqk(h + 2)
                            fetch_wqk(HL + h + 2)
                        # proj work units for head h+1
                        units = []
                        if h + 1 < HL:
                            for j in (h + 1, HL + h + 1):
                                new_qk_tile(j)
                                for sb in range(NSB):
                                    for cc0 in range(0, NCC, 4):
                                        units.append((j, sb, cc0))
                        if h == HL - 1:
                            # prefetch wp column-block 0 for the output
                            # projection (scalar queue is idle here)
                            wp_t = {}
                            for f in range(HL):
                                t_ = wp_pool.tile([P, 512], f16,
                                                  tag=f"wp{f}",
                                                  name=f"wp{f}_0")
                                nc.scalar.dma_start(t_[:], wp_d[f][:, 0:512])
                                wp_t[f] = t_
                        pjt = {}
                        ui = 0
                        pair_no = 0

                        evac_q = []

                        def drain_evacs():
                            for (j, sb, ps) in evac_q:
                                nc.vector.tensor_copy(
                                    out=qk_tiles[j][:, sb * 512:
                                                    (sb + 1) * 512],
                                    in_=ps[:])
                            del evac_q[:]

                        def emit_units(target):
                            # lazy evacs from the previous call: by now the
                            # group's matmuls have executed, so the DVE copy
                            # won't sit blocked at the head of the queue.
                            nonlocal ui
                            drain_evacs()
                            while ui < min(target, len(units)):
                                (j, sb, cc0) = units[ui]
                                if cc0 == 0:
                                    pjt[(j, sb)] = pjp.tile(
                                        [P, 512], f32, tag="pj",
                                        name=f"pj{j}_{sb}")
                                ps = pjt[(j, sb)]
                                for cc in range(cc0, cc0 + 4):
                                    nc.tensor.matmul(
                                        ps[:],
                                        wqk_tiles[j][:, cc * P:(cc + 1) * P],
                                        xts[cc][:, sb * 512:(sb + 1) * 512],
                                        start=(cc == 0), stop=(cc == NCC - 1))
                                if cc0 == NCC - 4:
                                    evac_q.append((j, sb, pjt.pop((j, sb))))
                                ui += 1

                        for si in range(NSB):
                            njc = 4 * si + 4
                            npr = njc // 2
                            av_ps = avp.tile([P, 512], f32, tag="av")
                            sum_ps = sup.tile([1, 512], f32, tag="sum")
                            pend = None
                            for pr in range(npr):
                                infos = _pair_layout(si, pr)
                                sc = scp.tile([P, 1024], f32, tag="sc")
                                for (jj, dd, oo, ww) in infos:
                                    nc.tensor.matmul(
                                        sc[:, oo:oo + ww],
                                        kt[:, jj * P:(jj + 1) * P],
                                        qt[:, si * 512 + dd:(si + 1) * 512],
                                        start=True, stop=True)
                                et = etp.tile([P, 1024], f16, tag="et")
                                end = infos[-1][2] + infos[-1][3]
                                nc.scalar.activation(
                                    et[:, :end], sc[:, :end], ACT.Exp,
                                    scale=EXP_SCALE)
                                for (jj, dd, oo, ww) in infos:
                                    if jj >= 4 * si:  # diagonal 128-block
                                        nc.vector.tensor_mul(
                                            out=et[:, oo:oo + P],
                                            in0=et[:, oo:oo + P],
                                            in1=mask_sb[:])
                                pair_no += 1
                                emit_units(3 * pair_no)
                                if pend is not None:
                                    pet, pinfos = pend
                                    for (jj, dd, oo, ww) in pinfos:
                                        nc.tensor.matmul(
                                            av_ps[:, dd:],
                                            vsb[jj][:, h * P:(h + 1) * P],
                                            pet[:, oo:oo + ww],
                                            start=(jj == 0), stop=False)
                                    for (jj, dd, oo, ww) in pinfos:
                                        nc.tensor.matmul(
                                            sum_ps[:, dd:], ones_sb[:],
                                            pet[:, oo:oo + ww],
                                            start=(jj == 0), stop=False)
                                pend = (et, infos)
                                gp += 1
                                flush_tails(gp)
                            pet, pinfos = pend
                            for (jj, dd, oo, ww) in pinfos:
                                nc.tensor.matmul(
                                    av_ps[:, dd:],
                                    vsb[jj][:, h * P:(h + 1) * P],
                                    pet[:, oo:oo + ww],
                                    start=(jj == 0), stop=(jj == njc - 1))
                            for (jj, dd, oo, ww) in pinfos:
                                nc.tensor.matmul(
                                    sum_ps[:, dd:], ones_sb[:],
                                    pet[:, oo:oo + ww],
                                    start=(jj == 0), stop=(jj == njc - 1))

                            # tail: recip -> broadcast -> normalize, each
                            # emitted with growing slack so no in-order queue
                            # ever blocks on a cross-engine dependency.
                            def make_t1(h=h, si=si, sum_ps=sum_ps, box=None):
                                def emit():
                                    rec = stp.tile([1, 512], f32, tag="rec",
                                                   name=f"rec{h}_{si}")
                                    nc.vector.reciprocal_approx_fast(
                                        out=rec[:], in_=sum_ps[:])
                                    box["rec"] = rec
                                return emit

                            def make_t2(h=h, si=si, box=None):
                                def emit():
                                    recb = stp.tile([P, 512], f32, tag="recb",
                                                    name=f"recb{h}_{si}")
                                    nc.gpsimd.partition_broadcast(
                                        recb[:], box["rec"][:])
                                    box["recb"] = recb
                                return emit

                            def make_t3(h=h, si=si, av_ps=av_ps, box=None):
                                def emit():
                                    nc.vector.tensor_mul(
                                        out=avts[h][:, si * 512:
                                                    (si + 1) * 512],
                                        in0=av_ps[:], in1=box["recb"][:])
                                return emit

                            box = {}
                            tails.append((gp + 2, make_t1(box=box)))
                            tails.append((gp + 3, make_t2(box=box)))
                            tails.append((gp + 4, make_t3(box=box)))
                        emit_units(len(units))
                        drain_evacs()
                    flush_tails(10 ** 9)

                    # ---------- output projection (cb-outer, wp streamed) ---
                    gi = 0
                    for cb in range(NCB):
                        if cb + 1 < NCB:
                            wp_next = {}
                            for f in range(HL):
                                t_ = wp_pool.tile([P, 512], f16,
                                                  tag=f"wp{f}",
                                                  name=f"wp{f}_{cb + 1}")
                                nc.sync.dma_start(
                                    t_[:], wp_d[f][:, (cb + 1) * 512:
                                                   (cb + 2) * 512])
                                wp_next[f] = t_
                        for tch in range(NTC):
                            pool = avp if gi % 2 == 0 else pjp
                            tag = "av" if gi % 2 == 0 else "pj"
                            ps = pool.tile([P, 512], f32, tag=tag,
                                           name=f"po{cb}_{tch}")
                            gi += 1
                            for f in range(HL):
                                nc.tensor.matmul(
                                    ps[:], avts[f][:, tch * P:(tch + 1) * P],
                                    wp_t[f][:, :],
                                    start=(f == 0), stop=(f == HL - 1))
                            ob = obp.tile([P, 512], f32, tag="ob")
                            evac(ob[:], ps[:])
                            nc.sync.dma_start(
                                out_d[tch * P:(tch + 1) * P,
                                      cb * 512:(cb + 1) * 512], ob[:])
                        if cb + 1 < NCB:
                            wp_t = wp_next
    nc.compile()
    return nc


def _make_mask():
    pp_ = np.arange(P)[:, None]
    ff = np.arange(P)[None, :]
    return np.where(ff >= pp_, 1.0, 0.0).astype(np_f16)


def _prep_inputs(x, w_qkv, w_proj):
    mask = _make_mask()
    per_g = {}
    for g in range(2):
        q = w_qkv[:, g * FL:(g + 1) * FL]
        k = w_qkv[:, C + g * FL:C + (g + 1) * FL]
        v = w_qkv[:, 2 * C + g * FL:2 * C + (g + 1) * FL]
        wqk_cat = np.concatenate([q, k], axis=1)  # [C, 2048]
        wqk_p = np.ascontiguousarray(
            wqk_cat.reshape(NCC, P, 16, P).transpose(2, 1, 0, 3)
            .reshape(16, P, C)).astype(np_f16)
        wv_p = np.ascontiguousarray(v.reshape(NCC, P, FL)).astype(np_f16)
        wp_p = np.ascontiguousarray(
            w_proj[g * FL:(g + 1) * FL, :].reshape(HL, P, C)).astype(np_f16)
        per_g[g] = (wqk_p, wv_p, wp_p)
    in_maps = []
    for core in range(NCORES):
        b, g = core // 2, core % 2
        wqk_p, wv_p, wp_p = per_g[g]
        in_maps.append({
            "xt": np.ascontiguousarray(x[b].T).astype(np_f16),
            "wqk": wqk_p,
            "wv": wv_p,
            "wp": wp_p,
            "mask": mask,
        })
    return in_maps


_nc_cache = None
last_results = None  # BassKernelResults of the most recent run (for test.py)


def kernel(x, w_qkv, w_proj):
    global _nc_cache, last_results
    from concourse.bass_utils import run_bass_kernel_spmd

    x = np.asarray(x, dtype=np.float32)
    w_qkv = np.asarray(w_qkv, dtype=np.float32)
    w_proj = np.asarray(w_proj, dtype=np.float32)

    if _nc_cache is None:
        _nc_cache = build_nc()
    nc = _nc_cache

    in_maps = _prep_inputs(x, w_qkv, w_proj)
    trace = bool(int(os.environ.get("KERNEL_TRACE", "0")))
    res = run_bass_kernel_spmd(nc, in_maps, list(range(NCORES)), trace=trace)
    last_results = res

    out = np.empty((B, T, C), dtype=np.float32)
    for b in range(B):
        out[b] = res.results[2 * b]["out"] + res.results[2 * b + 1]["out"]
    return out


# revision 12
# speedup vs baseline: 1.3723x; 1.0018x over previous
"""Causal self-attention Trainium2 Bass kernel, v4.

B=4, T=2048, C=2048, H=16, D=128, fp32 I/O. DP=4 x TP=2 (Megatron
head-group split); host sums TP pairs.

Per-core structure:
  [QK proj h0] [V proj] [fused: attn(h) + QK proj(h+1)] x8 [out proj]

- f16 activations/weights, fp32 PSUM accumulation everywhere.
- Fully SBUF-resident intermediates; q/k tiles rotate (lifetime ~2 head
  slots), wqk weight tiles stream with bufs=4, wp streams per 512-col
  block during the cb-outer output projection.
- Attention: score pairs packed into [128,1024] PSUM tiles -> one exp
  per pair; exp tiles merged into per-si accumulators S_a (DVE, even
  pairs) / S_b (gpsimd, odd pairs); per-si two ones-matmuls reduce them
  into sum[1,512]; reciprocal -> partition_broadcast -> normalize fused
  into the av PSUM->SBUF evacuation.
- QK projection of head h+1 interleaves between attention pairs of
  head h (4-matmul units, cc-contiguous into one PSUM bank) so the PE
  never waits on ACT exp.
- PSUM fused phase: sc pair 2 + av 3 + proj 2 + sum 1 = 8 banks; the
  output projection reuses the av/proj pools.
"""

import math
import os
import sys

import numpy as np

for _p in ("/opt/trn_rl_repo",):
    if _p not in sys.path:
        sys.path.insert(0, _p)

import ml_dtypes
import concourse.bass as bass
import concourse.mybir as mybir
from concourse import bacc
from concourse.tile import TileContext

B, T, C, H, D = 4, 2048, 2048, 16, 128
P = 128
NCORES = 8
HL = 8           # heads per core
FL = HL * D      # local feature dim = 1024
NCC = C // P     # 16 contraction chunks
NTC = T // P     # 16 t chunks
NSB = T // 512   # 4 t superblocks
NCB = C // 512   # 4 output col blocks
EXP_SCALE = 1.0 / math.sqrt(D)

f32 = mybir.dt.float32
f16 = mybir.dt.float16
np_f16 = np.float16


def _pair_layout(si, pr):
    """Packing of score pair pr (j-chunks 2pr, 2pr+1) of superblock si into a
    [128, 1024] PSUM tile. Returns [(jj, d_off, col_off, width), ...]."""
    js = (2 * pr, 2 * pr + 1)
    d0 = max(0, js[0] * P - si * 512)
    w0 = 512 - d0
    d1 = max(0, js[1] * P - si * 512)
    w1 = 512 - d1
    o1 = w0 if (w0 + w1) <= 512 else 512
    return [(js[0], d0, 0, w0), (js[1], d1, o1, w1)]


def build_nc():
    nc = bacc.Bacc()
    xt_d = nc.declare_dram_parameter("xt", [C, T], f16, isOutput=False)
    wqk_d = nc.declare_dram_parameter("wqk", [16, P, C], f16, isOutput=False)
    wv_d = nc.declare_dram_parameter("wv", [NCC, P, FL], f16, isOutput=False)
    wp_d = nc.declare_dram_parameter("wp", [HL, P, C], f16, isOutput=False)
    mask_d = nc.declare_dram_parameter("mask", [P, P], f16, isOutput=False)
    out_d = nc.declare_dram_parameter("out", [T, C], f32, isOutput=True)

    ACT = mybir.ActivationFunctionType

    with TileContext(nc) as tc:
        with tc.tile_pool(name="const", bufs=1) as cpool, \
             tc.tile_pool(name="avtp", bufs=1) as avt_pool:
            mask_sb = cpool.tile([P, P], f16)
            ones_sb = cpool.tile([P, 1], f16)
            nc.sync.dma_start(mask_sb[:], mask_d[:])
            # mask column 127 is all-ones; reuse it as the ones vector.
            nc.vector.tensor_copy(out=ones_sb[:], in_=mask_sb[:, P - 1:P])
            # pre-load the Exp activation table so the first attention exp
            # doesn't pay the ACT_TABLE_LOAD mid-pipeline.
            warm = cpool.tile([P, 1], f16)
            nc.scalar.activation(warm[:], ones_sb[:], ACT.Exp, scale=1.0)
            avts = [avt_pool.tile([P, T], f16, tag=f"avt{hh}",
                                  name=f"avt{hh}") for hh in range(HL)]

            state = {"cpi": 0}

            def evac(dst, src):
                if state["cpi"] % 2 == 0:
                    nc.vector.tensor_copy(out=dst, in_=src)
                else:
                    nc.scalar.copy(out=dst, in_=src)
                state["cpi"] += 1

            with tc.tile_pool(name="qkp", bufs=2) as qk_pool, \
                 tc.tile_pool(name="vp", bufs=1) as v_pool, \
                 tc.tile_pool(name="wqkp", bufs=4) as wqk_pool, \
                 tc.tile_pool(name="xtp", bufs=1) as xt_pool:
                vsb = [v_pool.tile([P, FL], f16, tag=f"v{tb}", name=f"v{tb}")
                       for tb in range(NTC)]
                xts = [xt_pool.tile([P, T], f16, tag=f"xt{cc}",
                                    name=f"xt{cc}") for cc in range(NCC)]

                qk_tiles = {}   # j -> rotating [P, T] tile
                wqk_tiles = {}  # j -> rotating [P, C] weight tile

                def fetch_wqk(j):
                    wt = wqk_pool.tile([P, C], f16, tag="wqk",
                                       name=f"wqk{j}")
                    nc.sync.dma_start(wt[:], wqk_d[j])
                    wqk_tiles[j] = wt

                def new_qk_tile(j):
                    t_ = qk_pool.tile([P, T], f16,
                                      tag="q" if j < HL else "k",
                                      name=f"qk{j}")
                    qk_tiles[j] = t_
                    return t_

                # DMA order: head-0 weights first (gates the first matmul),
                # then xt split across sync+scalar queues, wv halves behind
                # the xt halves, remaining wqk last on sync.
                fetch_wqk(0)
                fetch_wqk(8)
                for cc in range(8):
                    nc.sync.dma_start(xts[cc][:], xt_d[cc * P:(cc + 1) * P, :])
                for cc in range(8, NCC):
                    nc.scalar.dma_start(xts[cc][:],
                                        xt_d[cc * P:(cc + 1) * P, :])
                with tc.tile_pool(name="wvp", bufs=1) as wv_pool:
                    wvs = [wv_pool.tile([P, FL], f16, tag=f"wv{cc}",
                                        name=f"wv{cc}") for cc in range(NCC)]
                    for cc in range(8):
                        nc.sync.dma_start(wvs[cc][:], wv_d[cc])
                    for cc in range(8, NCC):
                        nc.scalar.dma_start(wvs[cc][:], wv_d[cc])
                    fetch_wqk(1)
                    fetch_wqk(9)

                    with tc.tile_pool(name="pp", bufs=8, space="PSUM") as pp:
                        # PE warm-up: dummy matmuls on the (tiny, first-to-
                        # arrive) mask tile keep the PE busy through the
                        # input-DMA window so HAM is at full clock when the
                        # real projection starts. Nothing reads the result.
                        jp = pp.tile([P, 512], f32, tag="pp", name="warm")
                        for _w in range(80):
                            nc.tensor.matmul(jp[:, :P], mask_sb[:],
                                             mask_sb[:], start=True,
                                             stop=True)

                        # ---------- QK projection, head 0 ----------
                        # cc order interleaves the two DMA queues' arrival
                        # order so the first groups march behind the input
                        # transfers instead of blocking on the last chunk.
                        cc_arr = [x for p_ in range(8) for x in (p_, 8 + p_)]
                        for j in (0, 8):
                            dst = new_qk_tile(j)
                            for sb in range(NSB):
                                ps = pp.tile([P, 512], f32, tag="pp",
                                             name=f"pj{j}_{sb}")
                                for ci, cc in enumerate(cc_arr):
                                    nc.tensor.matmul(
                                        ps[:],
                                        wqk_tiles[j][:, cc * P:(cc + 1) * P],
                                        xts[cc][:, sb * 512:(sb + 1) * 512],
                                        start=(ci == 0), stop=(ci == NCC - 1))
                                evac(dst[:, sb * 512:(sb + 1) * 512], ps[:])

                        # ---------- V projection ----------
                        for tb in range(NTC):
                            for vb in range(2):
                                ps = pp.tile([P, 512], f32, tag="pp",
                                             name=f"pv{tb}_{vb}")
                                for ci, cc in enumerate(cc_arr):
                                    nc.tensor.matmul(
                                        ps[:], xts[cc][:, tb * P:(tb + 1) * P],
                                        wvs[cc][:, vb * 512:(vb + 1) * 512],
                                        start=(ci == 0), stop=(ci == NCC - 1))
                                nc.scalar.copy(
                                    out=vsb[tb][:, vb * 512:(vb + 1) * 512],
                                    in_=ps[:])
                # wvs + pp freed

                # ---------- fused attention + next-head QK proj ----------
                with tc.tile_pool(name="scp", bufs=1, space="PSUM") as scp, \
                     tc.tile_pool(name="avp", bufs=2, space="PSUM") as avp, \
                     tc.tile_pool(name="pjp", bufs=2, space="PSUM") as pjp, \
                     tc.tile_pool(name="sup", bufs=2, space="PSUM") as sup, \
                     tc.tile_pool(name="etp", bufs=4) as etp, \
                     tc.tile_pool(name="Sp", bufs=2) as Sp, \
                     tc.tile_pool(name="stp", bufs=2) as stp, \
                     tc.tile_pool(name="wpp", bufs=2) as wp_pool, \
                     tc.tile_pool(name="obp", bufs=4) as obp:
                    # deferred si tails: each tail (sum-MMs, recip, broadcast,
                    # normalize) is emitted two pairs after its si completes
                    # so the in-order PE stream never blocks on the S chains.
                    gp = 0            # global pair counter
                    tails = []        # (ready_at_gp, emit_fn)
                    p3_done = set()   # (cb, tch) groups emitted early

                    def flush_tails(now):
                        while tails and tails[0][0] <= now:
                            tails.pop(0)[1]()

                    for h in range(HL):
                        qt, kt = qk_tiles[h], qk_tiles[HL + h]
                        # prefetch weights for head h+2's projection
                        if h + 2 < HL:
                            fetch_wqk(h + 2)
                            fetch_wqk(HL + h + 2)
                        # proj work units for head h+1
                        units = []
                        if h + 1 < HL:
                            for j in (h + 1, HL + h + 1):
                                new_qk_tile(j)
                                for sb in range(NSB):
                                    for cc0 in range(0, NCC, 4):
                                        units.append((j, sb, cc0))
                        if h == HL - 1:
                            # prefetch wp column-block 0 for the output
                            # projection (scalar queue is idle here)
                            wp_t = {}
                            for f in range(HL):
                                t_ = wp_pool.tile([P, 512], f16,
                                                  tag=f"wp{f}",
                                                  name=f"wp{f}_0")
                                nc.scalar.dma_start(t_[:], wp_d[f][:, 0:512])
                                wp_t[f] = t_
                        pjt = {}
                        ui = 0
                        pair_no = 0

                        evac_q = []

                        def drain_evacs():
                            for (j, sb, ps) in evac_q:
                                nc.vector.tensor_copy(
                                    out=qk_tiles[j][:, sb * 512:
                                                    (sb + 1) * 512],
                                    in_=ps[:])
                            del evac_q[:]

                        def emit_units(target):
                            # lazy evacs from the previous call: by now the
                            # group's matmuls have executed, so the DVE copy
                            # won't sit blocked at the head of the queue.
                            nonlocal ui
                            drain_evacs()
                            while ui < min(target, len(units)):
                                (j, sb, cc0) = units[ui]
                                if cc0 == 0:
                                    pjt[(j, sb)] = pjp.tile(
                                        [P, 512], f32, tag="pj",
                                        name=f"pj{j}_{sb}")
                                ps = pjt[(j, sb)]
                                for cc in range(cc0, cc0 + 4):
                                    nc.tensor.matmul(
                                        ps[:],
                                        wqk_tiles[j][:, cc * P:(cc + 1) * P],
                                        xts[cc][:, sb * 512:(sb + 1) * 512],
                                        start=(cc == 0), stop=(cc == NCC - 1))
                                if cc0 == NCC - 4:
                                    evac_q.append((j, sb, pjt.pop((j, sb))))
                                ui += 1

                        for si in range(NSB):
                            njc = 4 * si + 4
                            npr = njc // 2
                            av_ps = avp.tile([P, 512], f32, tag="av")
                            sum_ps = sup.tile([1, 512], f32, tag="sum")
                            pend = None
                            for pr in range(npr):
                                infos = _pair_layout(si, pr)
                                sc = scp.tile([P, 1024], f32, tag="sc")
                                for (jj, dd, oo, ww) in infos:
                                    nc.tensor.matmul(
                                        sc[:, oo:oo + ww],
                                        kt[:, jj * P:(jj + 1) * P],
                                        qt[:, si * 512 + dd:(si + 1) * 512],
                                        start=True, stop=True)
                                et = etp.tile([P, 1024], f16, tag="et")
                                end = infos[-1][2] + infos[-1][3]
                                nc.scalar.activation(
                                    et[:, :end], sc[:, :end], ACT.Exp,
                                    scale=EXP_SCALE)
                                for (jj, dd, oo, ww) in infos:
                                    if jj >= 4 * si:  # diagonal 128-block
                                        nc.vector.tensor_mul(
                                            out=et[:, oo:oo + P],
                                            in0=et[:, oo:oo + P],
                                            in1=mask_sb[:])
                                pair_no += 1
                                emit_units(3 * pair_no)
                                if pend is not None:
                                    pet, pinfos = pend
                                    for (jj, dd, oo, ww) in pinfos:
                                        nc.tensor.matmul(
                                            av_ps[:, dd:],
                                            vsb[jj][:, h * P:(h + 1) * P],
                                            pet[:, oo:oo + ww],
                                            start=(jj == 0), stop=False)
                                    for (jj, dd, oo, ww) in pinfos:
                                        nc.tensor.matmul(
                                            sum_ps[:, dd:], ones_sb[:],
                                            pet[:, oo:oo + ww],
                                            start=(jj == 0), stop=False)
                                pend = (et, infos)
                                gp += 1
                                flush_tails(gp)
                            pet, pinfos = pend
                            for (jj, dd, oo, ww) in pinfos:
                                nc.tensor.matmul(
                                    av_ps[:, dd:],
                                    vsb[jj][:, h * P:(h + 1) * P],
                                    pet[:, oo:oo + ww],
                                    start=(jj == 0), stop=(jj == njc - 1))
                            for (jj, dd, oo, ww) in pinfos:
                                nc.tensor.matmul(
                                    sum_ps[:, dd:], ones_sb[:],
                                    pet[:, oo:oo + ww],
                                    start=(jj == 0), stop=(jj == njc - 1))

                            # tail: recip -> broadcast -> normalize, each
                            # emitted with growing slack so no in-order queue
                            # ever blocks on a cross-engine dependency.
                            def make_t1(h=h, si=si, sum_ps=sum_ps, box=None):
                                def emit():
                                    rec = stp.tile([1, 512], f32, tag="rec",
                                                   name=f"rec{h}_{si}")
                                    nc.vector.reciprocal_approx_fast(
                                        out=rec[:], in_=sum_ps[:])
                                    box["rec"] = rec
                                return emit

                            def make_t2(h=h, si=si, box=None):
                                def emit():
                                    recb = stp.tile([P, 512], f32, tag="recb",
                                                    name=f"recb{h}_{si}")
                                    nc.gpsimd.partition_broadcast(
                                        recb[:], box["rec"][:])
                                    box["recb"] = recb
                                return emit

                            def make_t3(h=h, si=si, av_ps=av_ps, box=None):
                                def emit():
                                    nc.vector.tensor_mul(
                                        out=avts[h][:, si * 512:
                                                    (si + 1) * 512],
                                        in0=av_ps[:], in1=box["recb"][:])
                                return emit

                            box = {}
                            tails.append((gp + 2, make_t1(box=box)))
                            tails.append((gp + 3, make_t2(box=box)))
                            tails.append((gp + 4, make_t3(box=box)))
                            if h == HL - 1 and si < NSB - 1:
                                # head 7 has no next-head proj work; fill its
                                # slack with cb0 output-projection groups for
                                # the t-chunks this superblock just finished.
                                def make_p3(tch):
                                    def emit():
                                        ps = pjp.tile([P, 512], f32,
                                                      tag="pj",
                                                      name=f"po0_{tch}")
                                        for f in range(HL):
                                            nc.tensor.matmul(
                                                ps[:],
                                                avts[f][:, tch * P:
                                                        (tch + 1) * P],
                                                wp_t[f][:, :],
                                                start=(f == 0),
                                                stop=(f == HL - 1))
                                        ob = obp.tile([P, 512], f32,
                                                      tag="ob")
                                        nc.vector.tensor_copy(out=ob[:],
                                                              in_=ps[:])
                                        nc.sync.dma_start(
                                            out_d[tch * P:(tch + 1) * P,
                                                  0:512], ob[:])
                                    return emit
                                for k_, tch_ in enumerate(
                                        range(4 * si, 4 * si + 4)):
                                    tails.append((gp + 5 + k_,
                                                  make_p3(tch_)))
                                    p3_done.add((0, tch_))
                        emit_units(len(units))
                        drain_evacs()
                    flush_tails(10 ** 9)

                    # ---------- output projection (cb-outer, wp streamed) ---
                    gi = 0
                    for cb in range(NCB):
                        if cb + 1 < NCB:
                            wp_next = {}
                            for f in range(HL):
                                t_ = wp_pool.tile([P, 512], f16,
                                                  tag=f"wp{f}",
                                                  name=f"wp{f}_{cb + 1}")
                                nc.sync.dma_start(
                                    t_[:], wp_d[f][:, (cb + 1) * 512:
                                                   (cb + 2) * 512])
                                wp_next[f] = t_
                        for tch in range(NTC):
                            if (cb, tch) in p3_done:
                                continue
                            pool = avp if gi % 2 == 0 else pjp
                            tag = "av" if gi % 2 == 0 else "pj"
                            ps = pool.tile([P, 512], f32, tag=tag,
                                           name=f"po{cb}_{tch}")
                            gi += 1
                            for f in range(HL):
                                nc.tensor.matmul(
                                    ps[:], avts[f][:, tch * P:(tch + 1) * P],
                                    wp_t[f][:, :],
                                    start=(f == 0), stop=(f == HL - 1))
                            ob = obp.tile([P, 512], f32, tag="ob")
                            evac(ob[:], ps[:])
                            nc.sync.dma_start(
                                out_d[tch * P:(tch + 1) * P,
                                      cb * 512:(cb + 1) * 512], ob[:])
                        if cb + 1 < NCB:
                            wp_t = wp_next
    nc.compile()
    return nc


def _make_mask():
    pp_ = np.arange(P)[:, None]
    ff = np.arange(P)[None, :]
    return np.where(ff >= pp_, 1.0, 0.0).astype(np_f16)


def _prep_inputs(x, w_qkv, w_proj):
    mask = _make_mask()
    per_g = {}
    for g in range(2):
        q = w_qkv[:, g * FL:(g + 1) * FL]
        k = w_qkv[:, C + g * FL:C + (g + 1) * FL]
        v = w_qkv[:, 2 * C + g * FL:2 * C + (g + 1) * FL]
        wqk_cat = np.concatenate([q, k], axis=1)  # [C, 2048]
        wqk_p = np.ascontiguousarray(
            wqk_cat.reshape(NCC, P, 16, P).transpose(2, 1, 0, 3)
            .reshape(16, P, C)).astype(np_f16)
        wv_p = np.ascontiguousarray(v.reshape(NCC, P, FL)).astype(np_f16)
        wp_p = np.ascontiguousarray(
            w_proj[g * FL:(g + 1) * FL, :].reshape(HL, P, C)).astype(np_f16)
        per_g[g] = (wqk_p, wv_p, wp_p)
    in_maps = []
    for core in range(NCORES):
        b, g = core // 2, core % 2
        wqk_p, wv_p, wp_p = per_g[g]
        in_maps.append({
            "xt": np.ascontiguousarray(x[b].T).astype(np_f16),
            "wqk": wqk_p,
            "wv": wv_p,
            "wp": wp_p,
            "mask": mask,
        })
    return in_maps


_nc_cache = None
last_results = None  # BassKernelResults of the most recent run (for test.py)


def kernel(x, w_qkv, w_proj):
    global _nc_cache, last_results
    from concourse.bass_utils import run_bass_kernel_spmd

    x = np.asarray(x, dtype=np.float32)
    w_qkv = np.asarray(w_qkv, dtype=np.float32)
    w_proj = np.asarray(w_proj, dtype=np.float32)

    if _nc_cache is None:
        _nc_cache = build_nc()
    nc = _nc_cache

    in_maps = _prep_inputs(x, w_qkv, w_proj)
    trace = bool(int(os.environ.get("KERNEL_TRACE", "0")))
    res = run_bass_kernel_spmd(nc, in_maps, list(range(NCORES)), trace=trace)
    last_results = res

    out = np.empty((B, T, C), dtype=np.float32)
    for b in range(B):
        out[b] = res.results[2 * b]["out"] + res.results[2 * b + 1]["out"]
    return out


# revision 13
# speedup vs baseline: 1.4893x; 1.0853x over previous
"""Causal self-attention Trainium2 Bass kernel, v4.

B=4, T=2048, C=2048, H=16, D=128, fp32 I/O. DP=4 x TP=2 (Megatron
head-group split); host sums TP pairs.

Per-core structure:
  [QK proj h0] [V proj] [fused: attn(h) + QK proj(h+1)] x8 [out proj]

- f16 activations/weights, fp32 PSUM accumulation everywhere.
- Fully SBUF-resident intermediates; q/k tiles rotate (lifetime ~2 head
  slots), wqk weight tiles stream with bufs=4, wp streams per 512-col
  block during the cb-outer output projection.
- Attention: score pairs packed into [128,1024] PSUM tiles -> one exp
  per pair; exp tiles merged into per-si accumulators S_a (DVE, even
  pairs) / S_b (gpsimd, odd pairs); per-si two ones-matmuls reduce them
  into sum[1,512]; reciprocal -> partition_broadcast -> normalize fused
  into the av PSUM->SBUF evacuation.
- QK projection of head h+1 interleaves between attention pairs of
  head h (4-matmul units, cc-contiguous into one PSUM bank) so the PE
  never waits on ACT exp.
- PSUM fused phase: sc pair 2 + av 3 + proj 2 + sum 1 = 8 banks; the
  output projection reuses the av/proj pools.
"""

import math
import os
import sys

import numpy as np

for _p in ("/opt/trn_rl_repo",):
    if _p not in sys.path:
        sys.path.insert(0, _p)

import ml_dtypes
import concourse.bass as bass
import concourse.mybir as mybir
from concourse import bacc
from concourse.tile import TileContext

B, T, C, H, D = 4, 2048, 2048, 16, 128
P = 128
NCORES = 8
HL = 8           # heads per core
FL = HL * D      # local feature dim = 1024
NCC = C // P     # 16 contraction chunks
NTC = T // P     # 16 t chunks
NSB = T // 512   # 4 t superblocks
NCB = C // 512   # 4 output col blocks
EXP_SCALE = 1.0 / math.sqrt(D)

f32 = mybir.dt.float32
f16 = mybir.dt.float16
np_f16 = np.float16


def _pair_layout(si, pr):
    """Packing of score pair pr (j-chunks 2pr, 2pr+1) of superblock si into a
    [128, 1024] PSUM tile. Returns [(jj, d_off, col_off, width), ...]."""
    js = (2 * pr, 2 * pr + 1)
    d0 = max(0, js[0] * P - si * 512)
    w0 = 512 - d0
    d1 = max(0, js[1] * P - si * 512)
    w1 = 512 - d1
    o1 = w0 if (w0 + w1) <= 512 else 512
    return [(js[0], d0, 0, w0), (js[1], d1, o1, w1)]


def build_nc():
    nc = bacc.Bacc()
    xt_d = nc.declare_dram_parameter("xt", [C, T], f16, isOutput=False)
    wqk_d = nc.declare_dram_parameter("wqk", [16, P, C], f16, isOutput=False)
    wv_d = nc.declare_dram_parameter("wv", [NCC, P, FL], f16, isOutput=False)
    wp_d = nc.declare_dram_parameter("wp", [HL, P, C], f16, isOutput=False)
    mask_d = nc.declare_dram_parameter("mask", [P, P], f16, isOutput=False)
    out_d = nc.declare_dram_parameter("out", [T, C], f32, isOutput=True)

    ACT = mybir.ActivationFunctionType

    with TileContext(nc) as tc:
        with tc.tile_pool(name="const", bufs=1) as cpool, \
             tc.tile_pool(name="avtp", bufs=1) as avt_pool:
            mask_sb = cpool.tile([P, P], f16)
            ones_sb = cpool.tile([P, 1], f16)
            nc.sync.dma_start(mask_sb[:], mask_d[:])
            # mask column 127 is all-ones; reuse it as the ones vector.
            nc.vector.tensor_copy(out=ones_sb[:], in_=mask_sb[:, P - 1:P])
            # pre-load the Exp activation table so the first attention exp
            # doesn't pay the ACT_TABLE_LOAD mid-pipeline.
            warm = cpool.tile([P, 1], f16)
            nc.scalar.activation(warm[:], ones_sb[:], ACT.Exp, scale=1.0)
            avts = [avt_pool.tile([P, T], f16, tag=f"avt{hh}",
                                  name=f"avt{hh}") for hh in range(HL)]

            state = {"cpi": 0}

            def evac(dst, src):
                if state["cpi"] % 2 == 0:
                    nc.vector.tensor_copy(out=dst, in_=src)
                else:
                    nc.scalar.copy(out=dst, in_=src)
                state["cpi"] += 1

            with tc.tile_pool(name="qkp", bufs=2) as qk_pool, \
                 tc.tile_pool(name="vp", bufs=1) as v_pool, \
                 tc.tile_pool(name="wqkp", bufs=4) as wqk_pool, \
                 tc.tile_pool(name="xtp", bufs=1) as xt_pool:
                vsb = [v_pool.tile([P, FL], f16, tag=f"v{tb}", name=f"v{tb}")
                       for tb in range(NTC)]
                xts = [xt_pool.tile([P, T], f16, tag=f"xt{cc}",
                                    name=f"xt{cc}") for cc in range(NCC)]

                qk_tiles = {}   # j -> rotating [P, T] tile
                wqk_tiles = {}  # j -> rotating [P, C] weight tile

                def fetch_wqk(j):
                    wt = wqk_pool.tile([P, C], f16, tag="wqk",
                                       name=f"wqk{j}")
                    nc.sync.dma_start(wt[:], wqk_d[j])
                    wqk_tiles[j] = wt

                def new_qk_tile(j):
                    t_ = qk_pool.tile([P, T], f16,
                                      tag="q" if j < HL else "k",
                                      name=f"qk{j}")
                    qk_tiles[j] = t_
                    return t_

                # DMA order: head-0 weights first (gates the first matmul),
                # then xt split across sync+scalar queues, wv halves behind
                # the xt halves, remaining wqk last on sync.
                fetch_wqk(0)
                fetch_wqk(8)
                for cc in range(8):
                    nc.sync.dma_start(xts[cc][:], xt_d[cc * P:(cc + 1) * P, :])
                for cc in range(8, NCC):
                    nc.scalar.dma_start(xts[cc][:],
                                        xt_d[cc * P:(cc + 1) * P, :])
                with tc.tile_pool(name="wvp", bufs=1) as wv_pool:
                    wvs = [wv_pool.tile([P, FL], f16, tag=f"wv{cc}",
                                        name=f"wv{cc}") for cc in range(NCC)]
                    for cc in range(8):
                        nc.sync.dma_start(wvs[cc][:], wv_d[cc])
                    for cc in range(8, NCC):
                        nc.scalar.dma_start(wvs[cc][:], wv_d[cc])
                    fetch_wqk(1)
                    fetch_wqk(9)

                    with tc.tile_pool(name="pp", bufs=8, space="PSUM") as pp:
                        # PE warm-up: dummy matmuls on the (tiny, first-to-
                        # arrive) mask tile keep the PE busy through the
                        # input-DMA window so HAM is at full clock when the
                        # real projection starts. Nothing reads the result.
                        jp = pp.tile([P, 512], f32, tag="pp", name="warm")
                        for _w in range(80):
                            nc.tensor.matmul(jp[:, :P], mask_sb[:],
                                             mask_sb[:], start=True,
                                             stop=True)

                        # ---------- QK projection, head 0 ----------
                        # cc order interleaves the two DMA queues' arrival
                        # order so the first groups march behind the input
                        # transfers instead of blocking on the last chunk.
                        cc_arr = [x for p_ in range(8) for x in (p_, 8 + p_)]
                        for j in (0, 8):
                            dst = new_qk_tile(j)
                            for sb in range(NSB):
                                ps = pp.tile([P, 512], f32, tag="pp",
                                             name=f"pj{j}_{sb}")
                                for ci, cc in enumerate(cc_arr):
                                    nc.tensor.matmul(
                                        ps[:],
                                        wqk_tiles[j][:, cc * P:(cc + 1) * P],
                                        xts[cc][:, sb * 512:(sb + 1) * 512],
                                        start=(ci == 0), stop=(ci == NCC - 1))
                                evac(dst[:, sb * 512:(sb + 1) * 512], ps[:])

                        # ---------- V projection ----------
                        for tb in range(NTC):
                            for vb in range(2):
                                ps = pp.tile([P, 512], f32, tag="pp",
                                             name=f"pv{tb}_{vb}")
                                for ci, cc in enumerate(cc_arr):
                                    nc.tensor.matmul(
                                        ps[:], xts[cc][:, tb * P:(tb + 1) * P],
                                        wvs[cc][:, vb * 512:(vb + 1) * 512],
                                        start=(ci == 0), stop=(ci == NCC - 1))
                                nc.scalar.copy(
                                    out=vsb[tb][:, vb * 512:(vb + 1) * 512],
                                    in_=ps[:])
                # wvs + pp freed

                # ---------- fused attention + next-head QK proj ----------
                with tc.tile_pool(name="scp", bufs=1, space="PSUM") as scp, \
                     tc.tile_pool(name="avp", bufs=3, space="PSUM") as avp, \
                     tc.tile_pool(name="pjp", bufs=2, space="PSUM") as pjp, \
                     tc.tile_pool(name="sup", bufs=1, space="PSUM") as sup, \
                     tc.tile_pool(name="etp", bufs=5) as etp, \
                     tc.tile_pool(name="Sp", bufs=2) as Sp, \
                     tc.tile_pool(name="Sp", bufs=2) as Sp, \
                     tc.tile_pool(name="stp", bufs=2) as stp, \
                     tc.tile_pool(name="wpp", bufs=2) as wp_pool, \
                     tc.tile_pool(name="obp", bufs=4) as obp:
                    # deferred si tails: each tail (sum-MMs, recip, broadcast,
                    # normalize) is emitted two pairs after its si completes
                    # so the in-order PE stream never blocks on the S chains.
                    gp = 0            # global pair counter
                    tails = []        # (ready_at_gp, emit_fn)
                    p3_done = set()   # (cb, tch) groups emitted early

                    def flush_tails(now):
                        while tails and tails[0][0] <= now:
                            tails.pop(0)[1]()

                    for h in range(HL):
                        qt, kt = qk_tiles[h], qk_tiles[HL + h]
                        # prefetch weights for head h+2's projection
                        if h + 2 < HL:
                            fetch_wqk(h + 2)
                            fetch_wqk(HL + h + 2)
                        # proj work units for head h+1
                        units = []
                        if h + 1 < HL:
                            for j in (h + 1, HL + h + 1):
                                new_qk_tile(j)
                                for sb in range(NSB):
                                    for cc0 in range(0, NCC, 4):
                                        units.append((j, sb, cc0))
                        if h == HL - 1:
                            # prefetch wp column-block 0 for the output
                            # projection (scalar queue is idle here)
                            wp_t = {}
                            for f in range(HL):
                                t_ = wp_pool.tile([P, 512], f16,
                                                  tag=f"wp{f}",
                                                  name=f"wp{f}_0")
                                nc.scalar.dma_start(t_[:], wp_d[f][:, 0:512])
                                wp_t[f] = t_
                        pjt = {}
                        ui = 0
                        pair_no = 0

                        evac_q = []

                        def drain_evacs():
                            for (j, sb, ps) in evac_q:
                                nc.vector.tensor_copy(
                                    out=qk_tiles[j][:, sb * 512:
                                                    (sb + 1) * 512],
                                    in_=ps[:])
                            del evac_q[:]

                        def emit_units(target):
                            # lazy evacs from the previous call: by now the
                            # group's matmuls have executed, so the DVE copy
                            # won't sit blocked at the head of the queue.
                            nonlocal ui
                            drain_evacs()
                            while ui < min(target, len(units)):
                                (j, sb, cc0) = units[ui]
                                if cc0 == 0:
                                    pjt[(j, sb)] = pjp.tile(
                                        [P, 512], f32, tag="pj",
                                        name=f"pj{j}_{sb}")
                                ps = pjt[(j, sb)]
                                for cc in range(cc0, cc0 + 4):
                                    nc.tensor.matmul(
                                        ps[:],
                                        wqk_tiles[j][:, cc * P:(cc + 1) * P],
                                        xts[cc][:, sb * 512:(sb + 1) * 512],
                                        start=(cc == 0), stop=(cc == NCC - 1))
                                if cc0 == NCC - 4:
                                    evac_q.append((j, sb, pjt.pop((j, sb))))
                                ui += 1

                        for si in range(NSB):
                            njc = 4 * si + 4
                            npr = njc // 2
                            nearly = max(0, npr - 2)
                            av_ps = avp.tile([P, 512], f32, tag="av")
                            S = None
                            if nearly:
                                S = Sp.tile([P, 512], f16, tag="S")
                            tail_ets = []
                            pend = None
                            for pr in range(npr):
                                infos = _pair_layout(si, pr)
                                sc = scp.tile([P, 1024], f32, tag="sc")
                                for (jj, dd, oo, ww) in infos:
                                    nc.tensor.matmul(
                                        sc[:, oo:oo + ww],
                                        kt[:, jj * P:(jj + 1) * P],
                                        qt[:, si * 512 + dd:(si + 1) * 512],
                                        start=True, stop=True)
                                et = etp.tile([P, 1024], f16, tag="et")
                                end = infos[-1][2] + infos[-1][3]
                                nc.scalar.activation(
                                    et[:, :end], sc[:, :end], ACT.Exp,
                                    scale=EXP_SCALE)
                                for (jj, dd, oo, ww) in infos:
                                    if jj >= 4 * si:  # diagonal 128-block
                                        nc.vector.tensor_mul(
                                            out=et[:, oo:oo + P],
                                            in0=et[:, oo:oo + P],
                                            in1=mask_sb[:])
                                if pr < nearly:
                                    # early pairs are full-width, unmasked:
                                    # accumulate their row-sums on DVE.
                                    if pr == 0:
                                        nc.vector.tensor_copy(
                                            out=S[:], in_=et[:, 0:512])
                                    else:
                                        nc.vector.tensor_add(
                                            out=S[:], in0=S[:],
                                            in1=et[:, 0:512])
                                    nc.vector.tensor_add(
                                        out=S[:], in0=S[:],
                                        in1=et[:, 512:1024])
                                else:
                                    tail_ets.append((et, infos))
                                pair_no += 1
                                emit_units(3 * pair_no)
                                if pend is not None:
                                    pet, pinfos = pend
                                    for (jj, dd, oo, ww) in pinfos:
                                        nc.tensor.matmul(
                                            av_ps[:, dd:],
                                            vsb[jj][:, h * P:(h + 1) * P],
                                            pet[:, oo:oo + ww],
                                            start=(jj == 0), stop=False)
                                pend = (et, infos)
                                gp += 1
                                flush_tails(gp)
                            pet, pinfos = pend
                            for (jj, dd, oo, ww) in pinfos:
                                nc.tensor.matmul(
                                    av_ps[:, dd:],
                                    vsb[jj][:, h * P:(h + 1) * P],
                                    pet[:, oo:oo + ww],
                                    start=(jj == 0), stop=(jj == njc - 1))

                            # tail: recip -> broadcast -> normalize, each
                            # emitted with growing slack so no in-order queue
                            # ever blocks on a cross-engine dependency.
                            def make_t0(h=h, si=si, S=S, njc=njc,
                                        tail_ets=tail_ets, box=None):
                                def emit():
                                    sum_ps = sup.tile([1, 512], f32,
                                                      tag="sum",
                                                      name=f"sum{h}_{si}")
                                    first = True
                                    if S is not None:
                                        nc.tensor.matmul(
                                            sum_ps[:], ones_sb[:], S[:],
                                            start=True, stop=False)
                                        first = False
                                    for (et_, infos_) in tail_ets:
                                        for (jj, dd, oo, ww) in infos_:
                                            nc.tensor.matmul(
                                                sum_ps[:, dd:], ones_sb[:],
                                                et_[:, oo:oo + ww],
                                                start=first,
                                                stop=(jj == njc - 1))
                                            first = False
                                    box["sum"] = sum_ps
                                return emit

                            def make_t1(h=h, si=si, box=None):
                                def emit():
                                    rec = stp.tile([1, 512], f32, tag="rec",
                                                   name=f"rec{h}_{si}")
                                    nc.vector.reciprocal_approx_fast(
                                        out=rec[:], in_=box["sum"][:])
                                    box["rec"] = rec
                                return emit

                            def make_t2(h=h, si=si, box=None):
                                def emit():
                                    recb = stp.tile([P, 512], f32, tag="recb",
                                                    name=f"recb{h}_{si}")
                                    nc.gpsimd.partition_broadcast(
                                        recb[:], box["rec"][:])
                                    box["recb"] = recb
                                return emit

                            def make_t3(h=h, si=si, av_ps=av_ps, box=None):
                                def emit():
                                    nc.vector.tensor_mul(
                                        out=avts[h][:, si * 512:
                                                    (si + 1) * 512],
                                        in0=av_ps[:], in1=box["recb"][:])
                                return emit

                            box = {}
                            tails.append((gp + 2, make_t0(box=box)))
                            tails.append((gp + 3, make_t1(box=box)))
                            tails.append((gp + 4, make_t2(box=box)))
                            tails.append((gp + 5, make_t3(box=box)))
                            if h == HL - 1 and si < NSB - 1:
                                # head 7 has no next-head proj work; fill its
                                # slack with cb0 output-projection groups for
                                # the t-chunks this superblock just finished.
                                def make_p3(tch):
                                    def emit():
                                        ps = pjp.tile([P, 512], f32,
                                                      tag="pj",
                                                      name=f"po0_{tch}")
                                        for f in range(HL):
                                            nc.tensor.matmul(
                                                ps[:],
                                                avts[f][:, tch * P:
                                                        (tch + 1) * P],
                                                wp_t[f][:, :],
                                                start=(f == 0),
                                                stop=(f == HL - 1))
                                        ob = obp.tile([P, 512], f32,
                                                      tag="ob")
                                        nc.vector.tensor_copy(out=ob[:],
                                                              in_=ps[:])
                                        nc.sync.dma_start(
                                            out_d[tch * P:(tch + 1) * P,
                                                  0:512], ob[:])
                                    return emit
                                for k_, tch_ in enumerate(
                                        range(4 * si, 4 * si + 4)):
                                    tails.append((gp + 5 + k_,
                                                  make_p3(tch_)))
                                    p3_done.add((0, tch_))
                        emit_units(len(units))
                        drain_evacs()
                    flush_tails(10 ** 9)

                    # ---------- output projection (cb-outer, wp streamed) ---
                    gi = 0
                    for cb in range(NCB):
                        if cb + 1 < NCB:
                            wp_next = {}
                            for f in range(HL):
                                t_ = wp_pool.tile([P, 512], f16,
                                                  tag=f"wp{f}",
                                                  name=f"wp{f}_{cb + 1}")
                                nc.sync.dma_start(
                                    t_[:], wp_d[f][:, (cb + 1) * 512:
                                                   (cb + 2) * 512])
                                wp_next[f] = t_
                        for tch in range(NTC):
                            if (cb, tch) in p3_done:
                                continue
                            pool = avp if gi % 2 == 0 else pjp
                            tag = "av" if gi % 2 == 0 else "pj"
                            ps = pool.tile([P, 512], f32, tag=tag,
                                           name=f"po{cb}_{tch}")
                            gi += 1
                            for f in range(HL):
                                nc.tensor.matmul(
                                    ps[:], avts[f][:, tch * P:(tch + 1) * P],
                                    wp_t[f][:, :],
                                    start=(f == 0), stop=(f == HL - 1))
                            ob = obp.tile([P, 512], f32, tag="ob")
                            evac(ob[:], ps[:])
                            nc.sync.dma_start(
                                out_d[tch * P:(tch + 1) * P,
                                      cb * 512:(cb + 1) * 512], ob[:])
                        if cb + 1 < NCB:
                            wp_t = wp_next
    nc.compile()
    return nc


def _make_mask():
    pp_ = np.arange(P)[:, None]
    ff = np.arange(P)[None, :]
    return np.where(ff >= pp_, 1.0, 0.0).astype(np_f16)


def _prep_inputs(x, w_qkv, w_proj):
    mask = _make_mask()
    per_g = {}
    for g in range(2):
        q = w_qkv[:, g * FL:(g + 1) * FL]
        k = w_qkv[:, C + g * FL:C + (g + 1) * FL]
        v = w_qkv[:, 2 * C + g * FL:2 * C + (g + 1) * FL]
        wqk_cat = np.concatenate([q, k], axis=1)  # [C, 2048]
        wqk_p = np.ascontiguousarray(
            wqk_cat.reshape(NCC, P, 16, P).transpose(2, 1, 0, 3)
            .reshape(16, P, C)).astype(np_f16)
        wv_p = np.ascontiguousarray(v.reshape(NCC, P, FL)).astype(np_f16)
        wp_p = np.ascontiguousarray(
            w_proj[g * FL:(g + 1) * FL, :].reshape(HL, P, C)).astype(np_f16)
        per_g[g] = (wqk_p, wv_p, wp_p)
    in_maps = []
    for core in range(NCORES):
        b, g = core // 2, core % 2
        wqk_p, wv_p, wp_p = per_g[g]
        in_maps.append({
            "xt": np.ascontiguousarray(x[b].T).astype(np_f16),
            "wqk": wqk_p,
            "wv": wv_p,
            "wp": wp_p,
            "mask": mask,
        })
    return in_maps


_nc_cache = None
last_results = None  # BassKernelResults of the most recent run (for test.py)


def kernel(x, w_qkv, w_proj):
    global _nc_cache, last_results
    from concourse.bass_utils import run_bass_kernel_spmd

    x = np.asarray(x, dtype=np.float32)
    w_qkv = np.asarray(w_qkv, dtype=np.float32)
    w_proj = np.asarray(w_proj, dtype=np.float32)

    if _nc_cache is None:
        _nc_cache = build_nc()
    nc = _nc_cache

    in_maps = _prep_inputs(x, w_qkv, w_proj)
    trace = bool(int(os.environ.get("KERNEL_TRACE", "0")))
    res = run_bass_kernel_spmd(nc, in_maps, list(range(NCORES)), trace=trace)
    last_results = res

    out = np.empty((B, T, C), dtype=np.float32)
    for b in range(B):
        out[b] = res.results[2 * b]["out"] + res.results[2 * b + 1]["out"]
    return out


# revision 14
# speedup vs baseline: 1.5362x; 1.0315x over previous
"""Causal self-attention Trainium2 Bass kernel, v4.

B=4, T=2048, C=2048, H=16, D=128, fp32 I/O. DP=4 x TP=2 (Megatron
head-group split); host sums TP pairs.

Per-core structure:
  [QK proj h0] [V proj] [fused: attn(h) + QK proj(h+1)] x8 [out proj]

- f16 activations/weights, fp32 PSUM accumulation everywhere.
- Fully SBUF-resident intermediates; q/k tiles rotate (lifetime ~2 head
  slots), wqk weight tiles stream with bufs=4, wp streams per 512-col
  block during the cb-outer output projection.
- Attention: score pairs packed into [128,1024] PSUM tiles -> one exp
  per pair; exp tiles merged into per-si accumulators S_a (DVE, even
  pairs) / S_b (gpsimd, odd pairs); per-si two ones-matmuls reduce them
  into sum[1,512]; reciprocal -> partition_broadcast -> normalize fused
  into the av PSUM->SBUF evacuation.
- QK projection of head h+1 interleaves between attention pairs of
  head h (4-matmul units, cc-contiguous into one PSUM bank) so the PE
  never waits on ACT exp.
- PSUM fused phase: sc pair 2 + av 3 + proj 2 + sum 1 = 8 banks; the
  output projection reuses the av/proj pools.
"""

import math
import os
import sys

import numpy as np

for _p in ("/opt/trn_rl_repo",):
    if _p not in sys.path:
        sys.path.insert(0, _p)

import ml_dtypes
import concourse.bass as bass
import concourse.mybir as mybir
from concourse import bacc
from concourse.tile import TileContext

B, T, C, H, D = 4, 2048, 2048, 16, 128
P = 128
NCORES = 8
HL = 8           # heads per core
FL = HL * D      # local feature dim = 1024
NCC = C // P     # 16 contraction chunks
NTC = T // P     # 16 t chunks
NSB = T // 512   # 4 t superblocks
NCB = C // 512   # 4 output col blocks
EXP_SCALE = 1.0 / math.sqrt(D)

f32 = mybir.dt.float32
f16 = mybir.dt.float16
np_f16 = np.float16


def _pair_layout(si, pr):
    """Packing of score pair pr (j-chunks 2pr, 2pr+1) of superblock si into a
    [128, 1024] PSUM tile. Returns [(jj, d_off, col_off, width), ...]."""
    js = (2 * pr, 2 * pr + 1)
    d0 = max(0, js[0] * P - si * 512)
    w0 = 512 - d0
    d1 = max(0, js[1] * P - si * 512)
    w1 = 512 - d1
    o1 = w0 if (w0 + w1) <= 512 else 512
    return [(js[0], d0, 0, w0), (js[1], d1, o1, w1)]


def build_nc():
    nc = bacc.Bacc()
    xt_d = nc.declare_dram_parameter("xt", [C, T], f16, isOutput=False)
    wqk_d = nc.declare_dram_parameter("wqk", [16, P, C], f16, isOutput=False)
    wv_d = nc.declare_dram_parameter("wv", [NCC, P, FL], f16, isOutput=False)
    wp_d = nc.declare_dram_parameter("wp", [HL, P, C], f16, isOutput=False)
    mask_d = nc.declare_dram_parameter("mask", [P, P], f16, isOutput=False)
    out_d = nc.declare_dram_parameter("out", [T, C], f32, isOutput=True)

    ACT = mybir.ActivationFunctionType

    with TileContext(nc) as tc:
        with tc.tile_pool(name="const", bufs=1) as cpool, \
             tc.tile_pool(name="avtp", bufs=1) as avt_pool:
            mask_sb = cpool.tile([P, P], f16)
            ones_sb = cpool.tile([P, 1], f16)
            nc.sync.dma_start(mask_sb[:], mask_d[:])
            # mask column 127 is all-ones; reuse it as the ones vector.
            nc.vector.tensor_copy(out=ones_sb[:], in_=mask_sb[:, P - 1:P])
            # pre-load the Exp activation table so the first attention exp
            # doesn't pay the ACT_TABLE_LOAD mid-pipeline.
            warm = cpool.tile([P, 1], f16)
            nc.scalar.activation(warm[:], ones_sb[:], ACT.Exp, scale=1.0)
            avts = [avt_pool.tile([P, T], f16, tag=f"avt{hh}",
                                  name=f"avt{hh}") for hh in range(HL)]

            state = {"cpi": 0}

            def evac(dst, src):
                if state["cpi"] % 2 == 0:
                    nc.vector.tensor_copy(out=dst, in_=src)
                else:
                    nc.scalar.copy(out=dst, in_=src)
                state["cpi"] += 1

            with tc.tile_pool(name="qkp", bufs=2) as qk_pool, \
                 tc.tile_pool(name="vp", bufs=1) as v_pool, \
                 tc.tile_pool(name="wqkp", bufs=4) as wqk_pool, \
                 tc.tile_pool(name="xtp", bufs=1) as xt_pool:
                vsb = [v_pool.tile([P, FL], f16, tag=f"v{tb}", name=f"v{tb}")
                       for tb in range(NTC)]
                xts = [xt_pool.tile([P, T], f16, tag=f"xt{cc}",
                                    name=f"xt{cc}") for cc in range(NCC)]

                qk_tiles = {}   # j -> rotating [P, T] tile
                wqk_tiles = {}  # j -> rotating [P, C] weight tile

                def fetch_wqk(j):
                    wt = wqk_pool.tile([P, C], f16, tag="wqk",
                                       name=f"wqk{j}")
                    nc.sync.dma_start(wt[:], wqk_d[j])
                    wqk_tiles[j] = wt

                def new_qk_tile(j):
                    t_ = qk_pool.tile([P, T], f16,
                                      tag="q" if j < HL else "k",
                                      name=f"qk{j}")
                    qk_tiles[j] = t_
                    return t_

                # DMA order: head-0 weights first (gates the first matmul),
                # then xt split across sync+scalar queues, wv halves behind
                # the xt halves, remaining wqk last on sync.
                fetch_wqk(0)
                fetch_wqk(8)
                for cc in range(8):
                    nc.sync.dma_start(xts[cc][:], xt_d[cc * P:(cc + 1) * P, :])
                for cc in range(8, NCC):
                    nc.scalar.dma_start(xts[cc][:],
                                        xt_d[cc * P:(cc + 1) * P, :])
                with tc.tile_pool(name="wvp", bufs=1) as wv_pool:
                    wvs = [wv_pool.tile([P, FL], f16, tag=f"wv{cc}",
                                        name=f"wv{cc}") for cc in range(NCC)]
                    for cc in range(8):
                        nc.sync.dma_start(wvs[cc][:], wv_d[cc])
                    for cc in range(8, NCC):
                        nc.scalar.dma_start(wvs[cc][:], wv_d[cc])
                    fetch_wqk(1)
                    fetch_wqk(9)

                    with tc.tile_pool(name="pp", bufs=8, space="PSUM") as pp:
                        # PE warm-up: dummy matmuls on the (tiny, first-to-
                        # arrive) mask tile keep the PE busy through the
                        # input-DMA window so HAM is at full clock when the
                        # real projection starts. Nothing reads the result.
                        jp = pp.tile([P, 512], f32, tag="pp", name="warm")
                        for _w in range(80):
                            nc.tensor.matmul(jp[:, :P], mask_sb[:],
                                             mask_sb[:], start=True,
                                             stop=True)

                        # ---------- QK projection, head 0 ----------
                        # cc order interleaves the two DMA queues' arrival
                        # order so the first groups march behind the input
                        # transfers instead of blocking on the last chunk.
                        cc_arr = [x for p_ in range(8) for x in (p_, 8 + p_)]
                        for j in (0, 8):
                            dst = new_qk_tile(j)
                            for sb in range(NSB):
                                ps = pp.tile([P, 512], f32, tag="pp",
                                             name=f"pj{j}_{sb}")
                                for ci, cc in enumerate(cc_arr):
                                    nc.tensor.matmul(
                                        ps[:],
                                        wqk_tiles[j][:, cc * P:(cc + 1) * P],
                                        xts[cc][:, sb * 512:(sb + 1) * 512],
                                        start=(ci == 0), stop=(ci == NCC - 1))
                                evac(dst[:, sb * 512:(sb + 1) * 512], ps[:])

                        # ---------- V projection ----------
                        for tb in range(NTC):
                            for vb in range(2):
                                ps = pp.tile([P, 512], f32, tag="pp",
                                             name=f"pv{tb}_{vb}")
                                for ci, cc in enumerate(cc_arr):
                                    nc.tensor.matmul(
                                        ps[:], xts[cc][:, tb * P:(tb + 1) * P],
                                        wvs[cc][:, vb * 512:(vb + 1) * 512],
                                        start=(ci == 0), stop=(ci == NCC - 1))
                                nc.scalar.copy(
                                    out=vsb[tb][:, vb * 512:(vb + 1) * 512],
                                    in_=ps[:])
                # wvs + pp freed

                # ---------- fused attention + next-head QK proj ----------
                with tc.tile_pool(name="scp", bufs=1, space="PSUM") as scp, \
                     tc.tile_pool(name="avp", bufs=3, space="PSUM") as avp, \
                     tc.tile_pool(name="pjp", bufs=2, space="PSUM") as pjp, \
                     tc.tile_pool(name="sup", bufs=1, space="PSUM") as sup, \
                     tc.tile_pool(name="etp", bufs=5) as etp, \
                     tc.tile_pool(name="Sp", bufs=2) as Sp, \
                     tc.tile_pool(name="Sp", bufs=2) as Sp, \
                     tc.tile_pool(name="stp", bufs=2) as stp, \
                     tc.tile_pool(name="wpp", bufs=2) as wp_pool, \
                     tc.tile_pool(name="obp", bufs=4) as obp:
                    # deferred si tails: each tail (sum-MMs, recip, broadcast,
                    # normalize) is emitted two pairs after its si completes
                    # so the in-order PE stream never blocks on the S chains.
                    gp = 0            # global pair counter
                    tails = []        # (ready_at_gp, emit_fn)
                    p3_done = set()   # (cb, tch) groups emitted early

                    def flush_tails(now):
                        while tails and tails[0][0] <= now:
                            tails.pop(0)[1]()

                    for h in range(HL):
                        qt, kt = qk_tiles[h], qk_tiles[HL + h]
                        # prefetch weights for head h+2's projection
                        if h + 2 < HL:
                            fetch_wqk(h + 2)
                            fetch_wqk(HL + h + 2)
                        # proj work units for head h+1
                        units = []
                        if h + 1 < HL:
                            for j in (h + 1, HL + h + 1):
                                new_qk_tile(j)
                                for sb in range(NSB):
                                    for cc0 in range(0, NCC, 4):
                                        units.append((j, sb, cc0))
                        if h == HL - 1:
                            # prefetch wp column-block 0 for the output
                            # projection (scalar queue is idle here)
                            wp_t = {}
                            for f in range(HL):
                                t_ = wp_pool.tile([P, 512], f16,
                                                  tag=f"wp{f}",
                                                  name=f"wp{f}_0")
                                nc.scalar.dma_start(t_[:], wp_d[f][:, 0:512])
                                wp_t[f] = t_
                        pjt = {}
                        ui = 0
                        pair_no = 0

                        evac_q = []

                        def drain_evacs():
                            for (j, sb, ps) in evac_q:
                                nc.vector.tensor_copy(
                                    out=qk_tiles[j][:, sb * 512:
                                                    (sb + 1) * 512],
                                    in_=ps[:])
                            del evac_q[:]

                        def emit_units(target):
                            # lazy evacs from the previous call: by now the
                            # group's matmuls have executed, so the DVE copy
                            # won't sit blocked at the head of the queue.
                            nonlocal ui
                            drain_evacs()
                            while ui < min(target, len(units)):
                                (j, sb, cc0) = units[ui]
                                if cc0 == 0:
                                    pjt[(j, sb)] = pjp.tile(
                                        [P, 512], f32, tag="pj",
                                        name=f"pj{j}_{sb}")
                                ps = pjt[(j, sb)]
                                for cc in range(cc0, cc0 + 4):
                                    nc.tensor.matmul(
                                        ps[:],
                                        wqk_tiles[j][:, cc * P:(cc + 1) * P],
                                        xts[cc][:, sb * 512:(sb + 1) * 512],
                                        start=(cc == 0), stop=(cc == NCC - 1))
                                if cc0 == NCC - 4:
                                    evac_q.append((j, sb, pjt.pop((j, sb))))
                                ui += 1

                        for si in range(NSB):
                            njc = 4 * si + 4
                            npr = njc // 2
                            nearly = max(0, npr - 2)
                            av_ps = avp.tile([P, 512], f32, tag="av")
                            S = Sp.tile([P, 512], f16, tag="S")
                            first_S = True
                            pend = None
                            for pr in range(npr):
                                infos = _pair_layout(si, pr)
                                sc = scp.tile([P, 1024], f32, tag="sc")
                                for (jj, dd, oo, ww) in infos:
                                    nc.tensor.matmul(
                                        sc[:, oo:oo + ww],
                                        kt[:, jj * P:(jj + 1) * P],
                                        qt[:, si * 512 + dd:(si + 1) * 512],
                                        start=True, stop=True)
                                et = etp.tile([P, 1024], f16, tag="et")
                                end = infos[-1][2] + infos[-1][3]
                                nc.scalar.activation(
                                    et[:, :end], sc[:, :end], ACT.Exp,
                                    scale=EXP_SCALE)
                                for (jj, dd, oo, ww) in infos:
                                    if jj >= 4 * si:  # diagonal 128-block
                                        nc.vector.tensor_mul(
                                            out=et[:, oo:oo + P],
                                            in0=et[:, oo:oo + P],
                                            in1=mask_sb[:])
                                # accumulate row-sums on DVE; emitted after
                                # the masks on the same in-order queue, so
                                # diagonal pairs contribute masked values.
                                for (jj, dd, oo, ww) in infos:
                                    if first_S:
                                        nc.vector.tensor_copy(
                                            out=S[:, dd:],
                                            in_=et[:, oo:oo + ww])
                                        first_S = False
                                    else:
                                        nc.vector.tensor_add(
                                            out=S[:, dd:], in0=S[:, dd:],
                                            in1=et[:, oo:oo + ww])
                                pair_no += 1
                                emit_units(2 * pair_no)
                                if pend is not None:
                                    pet, pinfos = pend
                                    for (jj, dd, oo, ww) in pinfos:
                                        nc.tensor.matmul(
                                            av_ps[:, dd:],
                                            vsb[jj][:, h * P:(h + 1) * P],
                                            pet[:, oo:oo + ww],
                                            start=(jj == 0), stop=False)
                                pend = (et, infos)
                                gp += 1
                                flush_tails(gp)
                            pet, pinfos = pend
                            for (jj, dd, oo, ww) in pinfos:
                                nc.tensor.matmul(
                                    av_ps[:, dd:],
                                    vsb[jj][:, h * P:(h + 1) * P],
                                    pet[:, oo:oo + ww],
                                    start=(jj == 0), stop=(jj == njc - 1))

                            # tail: recip -> broadcast -> normalize, each
                            # emitted with growing slack so no in-order queue
                            # ever blocks on a cross-engine dependency.
                            def make_t0(h=h, si=si, S=S, box=None):
                                def emit():
                                    sum_ps = sup.tile([1, 512], f32,
                                                      tag="sum",
                                                      name=f"sum{h}_{si}")
                                    nc.tensor.matmul(
                                        sum_ps[:], ones_sb[:], S[:],
                                        start=True, stop=True)
                                    box["sum"] = sum_ps
                                return emit

                            def make_t1(h=h, si=si, box=None):
                                def emit():
                                    rec = stp.tile([1, 512], f32, tag="rec",
                                                   name=f"rec{h}_{si}")
                                    nc.vector.reciprocal_approx_fast(
                                        out=rec[:], in_=box["sum"][:])
                                    box["rec"] = rec
                                return emit

                            def make_t2(h=h, si=si, box=None):
                                def emit():
                                    recb = stp.tile([P, 512], f32, tag="recb",
                                                    name=f"recb{h}_{si}")
                                    nc.gpsimd.partition_broadcast(
                                        recb[:], box["rec"][:])
                                    box["recb"] = recb
                                return emit

                            def make_t3(h=h, si=si, av_ps=av_ps, box=None):
                                def emit():
                                    nc.vector.tensor_mul(
                                        out=avts[h][:, si * 512:
                                                    (si + 1) * 512],
                                        in0=av_ps[:], in1=box["recb"][:])
                                return emit

                            box = {}
                            tails.append((gp + 3, make_t0(box=box)))
                            tails.append((gp + 4, make_t1(box=box)))
                            tails.append((gp + 5, make_t2(box=box)))
                            tails.append((gp + 6, make_t3(box=box)))
                            if h == HL - 1 and si < NSB - 1:
                                # head 7 has no next-head proj work; fill its
                                # slack with cb0 output-projection groups for
                                # the t-chunks this superblock just finished.
                                def make_p3(tch):
                                    def emit():
                                        ps = pjp.tile([P, 512], f32,
                                                      tag="pj",
                                                      name=f"po0_{tch}")
                                        for f in range(HL):
                                            nc.tensor.matmul(
                                                ps[:],
                                                avts[f][:, tch * P:
                                                        (tch + 1) * P],
                                                wp_t[f][:, :],
                                                start=(f == 0),
                                                stop=(f == HL - 1))
                                        ob = obp.tile([P, 512], f32,
                                                      tag="ob")
                                        nc.vector.tensor_copy(out=ob[:],
                                                              in_=ps[:])
                                        nc.sync.dma_start(
                                            out_d[tch * P:(tch + 1) * P,
                                                  0:512], ob[:])
                                    return emit
                                for k_, tch_ in enumerate(
                                        range(4 * si, 4 * si + 4)):
                                    tails.append((gp + 7 + k_,
                                                  make_p3(tch_)))
                                    p3_done.add((0, tch_))
                        emit_units(len(units))
                        drain_evacs()
                    flush_tails(10 ** 9)

                    # ---------- output projection (cb-outer, wp streamed) ---
                    gi = 0
                    for cb in range(NCB):
                        if cb + 1 < NCB:
                            wp_next = {}
                            for f in range(HL):
                                t_ = wp_pool.tile([P, 512], f16,
                                                  tag=f"wp{f}",
                                                  name=f"wp{f}_{cb + 1}")
                                nc.sync.dma_start(
                                    t_[:], wp_d[f][:, (cb + 1) * 512:
                                                   (cb + 2) * 512])
                                wp_next[f] = t_
                        for tch in range(NTC):
                            if (cb, tch) in p3_done:
                                continue
                            pool = avp if gi % 2 == 0 else pjp
                            tag = "av" if gi % 2 == 0 else "pj"
                            ps = pool.tile([P, 512], f32, tag=tag,
                                           name=f"po{cb}_{tch}")
                            gi += 1
                            for f in range(HL):
                                nc.tensor.matmul(
                                    ps[:], avts[f][:, tch * P:(tch + 1) * P],
                                    wp_t[f][:, :],
                                    start=(f == 0), stop=(f == HL - 1))
                            ob = obp.tile([P, 512], f32, tag="ob")
                            evac(ob[:], ps[:])
                            nc.sync.dma_start(
                                out_d[tch * P:(tch + 1) * P,
                                      cb * 512:(cb + 1) * 512], ob[:])
                        if cb + 1 < NCB:
                            wp_t = wp_next
    nc.compile()
    return nc


def _make_mask():
    pp_ = np.arange(P)[:, None]
    ff = np.arange(P)[None, :]
    return np.where(ff >= pp_, 1.0, 0.0).astype(np_f16)


def _prep_inputs(x, w_qkv, w_proj):
    mask = _make_mask()
    per_g = {}
    for g in range(2):
        q = w_qkv[:, g * FL:(g + 1) * FL]
        k = w_qkv[:, C + g * FL:C + (g + 1) * FL]
        v = w_qkv[:, 2 * C + g * FL:2 * C + (g + 1) * FL]
        wqk_cat = np.concatenate([q, k], axis=1)  # [C, 2048]
        wqk_p = np.ascontiguousarray(
            wqk_cat.reshape(NCC, P, 16, P).transpose(2, 1, 0, 3)
            .reshape(16, P, C)).astype(np_f16)
        wv_p = np.ascontiguousarray(v.reshape(NCC, P, FL)).astype(np_f16)
        wp_p = np.ascontiguousarray(
            w_proj[g * FL:(g + 1) * FL, :].reshape(HL, P, C)).astype(np_f16)
        per_g[g] = (wqk_p, wv_p, wp_p)
    in_maps = []
    for core in range(NCORES):
        b, g = core // 2, core % 2
        wqk_p, wv_p, wp_p = per_g[g]
        in_maps.append({
            "xt": np.ascontiguousarray(x[b].T).astype(np_f16),
            "wqk": wqk_p,
            "wv": wv_p,
            "wp": wp_p,
            "mask": mask,
        })
    return in_maps


_nc_cache = None
last_results = None  # BassKernelResults of the most recent run (for test.py)


def kernel(x, w_qkv, w_proj):
    global _nc_cache, last_results
    from concourse.bass_utils import run_bass_kernel_spmd

    x = np.asarray(x, dtype=np.float32)
    w_qkv = np.asarray(w_qkv, dtype=np.float32)
    w_proj = np.asarray(w_proj, dtype=np.float32)

    if _nc_cache is None:
        _nc_cache = build_nc()
    nc = _nc_cache

    in_maps = _prep_inputs(x, w_qkv, w_proj)
    trace = bool(int(os.environ.get("KERNEL_TRACE", "0")))
    res = run_bass_kernel_spmd(nc, in_maps, list(range(NCORES)), trace=trace)
    last_results = res

    out = np.empty((B, T, C), dtype=np.float32)
    for b in range(B):
        out[b] = res.results[2 * b]["out"] + res.results[2 * b + 1]["out"]
    return out


# revision 15
# speedup vs baseline: 1.5470x; 1.0070x over previous
"""Causal self-attention Trainium2 Bass kernel, v4.

B=4, T=2048, C=2048, H=16, D=128, fp32 I/O. DP=4 x TP=2 (Megatron
head-group split); host sums TP pairs.

Per-core structure:
  [QK proj h0] [V proj] [fused: attn(h) + QK proj(h+1)] x8 [out proj]

- f16 activations/weights, fp32 PSUM accumulation everywhere.
- Fully SBUF-resident intermediates; q/k tiles rotate (lifetime ~2 head
  slots), wqk weight tiles stream with bufs=4, wp streams per 512-col
  block during the cb-outer output projection.
- Attention: score pairs packed into [128,1024] PSUM tiles -> one exp
  per pair; exp tiles merged into per-si accumulators S_a (DVE, even
  pairs) / S_b (gpsimd, odd pairs); per-si two ones-matmuls reduce them
  into sum[1,512]; reciprocal -> partition_broadcast -> normalize fused
  into the av PSUM->SBUF evacuation.
- QK projection of head h+1 interleaves between attention pairs of
  head h (4-matmul units, cc-contiguous into one PSUM bank) so the PE
  never waits on ACT exp.
- PSUM fused phase: sc pair 2 + av 3 + proj 2 + sum 1 = 8 banks; the
  output projection reuses the av/proj pools.
"""

import math
import os
import sys

import numpy as np

for _p in ("/opt/trn_rl_repo",):
    if _p not in sys.path:
        sys.path.insert(0, _p)

import ml_dtypes
import concourse.bass as bass
import concourse.mybir as mybir
from concourse import bacc
from concourse.tile import TileContext

B, T, C, H, D = 4, 2048, 2048, 16, 128
P = 128
NCORES = 8
HL = 8           # heads per core
FL = HL * D      # local feature dim = 1024
NCC = C // P     # 16 contraction chunks
NTC = T // P     # 16 t chunks
NSB = T // 512   # 4 t superblocks
NCB = C // 512   # 4 output col blocks
EXP_SCALE = 1.0 / math.sqrt(D)

f32 = mybir.dt.float32
f16 = mybir.dt.float16
np_f16 = np.float16


def _pair_layout(si, pr):
    """Packing of score pair pr (j-chunks 2pr, 2pr+1) of superblock si into a
    [128, 1024] PSUM tile. Returns [(jj, d_off, col_off, width), ...]."""
    js = (2 * pr, 2 * pr + 1)
    d0 = max(0, js[0] * P - si * 512)
    w0 = 512 - d0
    d1 = max(0, js[1] * P - si * 512)
    w1 = 512 - d1
    o1 = w0 if (w0 + w1) <= 512 else 512
    return [(js[0], d0, 0, w0), (js[1], d1, o1, w1)]


def build_nc():
    nc = bacc.Bacc()
    xt_d = nc.declare_dram_parameter("xt", [C, T], f16, isOutput=False)
    wqk_d = nc.declare_dram_parameter("wqk", [16, P, C], f16, isOutput=False)
    wv_d = nc.declare_dram_parameter("wv", [NCC, P, FL], f16, isOutput=False)
    wp_d = nc.declare_dram_parameter("wp", [HL, P, C], f16, isOutput=False)
    mask_d = nc.declare_dram_parameter("mask", [P, P], f16, isOutput=False)
    out_d = nc.declare_dram_parameter("out", [T, C], f32, isOutput=True)

    ACT = mybir.ActivationFunctionType

    with TileContext(nc) as tc:
        with tc.tile_pool(name="const", bufs=1) as cpool, \
             tc.tile_pool(name="avtp", bufs=1) as avt_pool:
            mask_sb = cpool.tile([P, P], f16)
            ones_sb = cpool.tile([P, 1], f16)
            nc.sync.dma_start(mask_sb[:], mask_d[:])
            # mask column 127 is all-ones; reuse it as the ones vector.
            nc.vector.tensor_copy(out=ones_sb[:], in_=mask_sb[:, P - 1:P])
            # pre-load the Exp activation table so the first attention exp
            # doesn't pay the ACT_TABLE_LOAD mid-pipeline.
            warm = cpool.tile([P, 1], f16)
            nc.scalar.activation(warm[:], ones_sb[:], ACT.Exp, scale=1.0)
            avts = [avt_pool.tile([P, T], f16, tag=f"avt{hh}",
                                  name=f"avt{hh}") for hh in range(HL)]

            state = {"cpi": 0}

            def evac(dst, src):
                if state["cpi"] % 2 == 0:
                    nc.vector.tensor_copy(out=dst, in_=src)
                else:
                    nc.scalar.copy(out=dst, in_=src)
                state["cpi"] += 1

            with tc.tile_pool(name="qkp", bufs=2) as qk_pool, \
                 tc.tile_pool(name="vp", bufs=1) as v_pool, \
                 tc.tile_pool(name="wqkp", bufs=4) as wqk_pool, \
                 tc.tile_pool(name="xtp", bufs=1) as xt_pool:
                vsb = [v_pool.tile([P, FL], f16, tag=f"v{tb}", name=f"v{tb}")
                       for tb in range(NTC)]
                xts = [xt_pool.tile([P, T], f16, tag=f"xt{cc}",
                                    name=f"xt{cc}") for cc in range(NCC)]

                qk_tiles = {}   # j -> rotating [P, T] tile
                wqk_tiles = {}  # j -> rotating [P, C] weight tile

                def fetch_wqk(j):
                    wt = wqk_pool.tile([P, C], f16, tag="wqk",
                                       name=f"wqk{j}")
                    nc.sync.dma_start(wt[:], wqk_d[j])
                    wqk_tiles[j] = wt

                def new_qk_tile(j):
                    t_ = qk_pool.tile([P, T], f16,
                                      tag="q" if j < HL else "k",
                                      name=f"qk{j}")
                    qk_tiles[j] = t_
                    return t_

                # DMA order: head-0 weights first (gates the first matmul),
                # then xt split across sync+scalar queues, wv halves behind
                # the xt halves, remaining wqk last on sync.
                fetch_wqk(0)
                fetch_wqk(8)
                for cc in range(8):
                    nc.sync.dma_start(xts[cc][:], xt_d[cc * P:(cc + 1) * P, :])
                for cc in range(8, NCC):
                    nc.scalar.dma_start(xts[cc][:],
                                        xt_d[cc * P:(cc + 1) * P, :])
                with tc.tile_pool(name="wvp", bufs=1) as wv_pool:
                    wvs = [wv_pool.tile([P, FL], f16, tag=f"wv{cc}",
                                        name=f"wv{cc}") for cc in range(NCC)]
                    for cc in range(8):
                        nc.sync.dma_start(wvs[cc][:], wv_d[cc])
                    for cc in range(8, NCC):
                        nc.scalar.dma_start(wvs[cc][:], wv_d[cc])
                    fetch_wqk(1)
                    fetch_wqk(9)

                    with tc.tile_pool(name="pp", bufs=6, space="PSUM") as pp:
                        # PE warm-up: dummy matmuls on the (tiny, first-to-
                        # arrive) mask tile keep the PE busy through the
                        # input-DMA window so HAM is at full clock when the
                        # real projection starts. Nothing reads the result.
                        jp = pp.tile([P, 512], f32, tag="pp", name="warm")
                        for _w in range(80):
                            nc.tensor.matmul(jp[:, :P], mask_sb[:],
                                             mask_sb[:], start=True,
                                             stop=True)

                        # ---------- QK projection, head 0 ----------
                        # cc order interleaves the two DMA queues' arrival
                        # order so the first groups march behind the input
                        # transfers instead of blocking on the last chunk.
                        cc_arr = [x for p_ in range(8) for x in (p_, 8 + p_)]
                        for j in (0, 8):
                            dst = new_qk_tile(j)
                            for sb in range(NSB):
                                ps = pp.tile([P, 512], f32, tag="pp",
                                             name=f"pj{j}_{sb}")
                                for ci, cc in enumerate(cc_arr):
                                    nc.tensor.matmul(
                                        ps[:],
                                        wqk_tiles[j][:, cc * P:(cc + 1) * P],
                                        xts[cc][:, sb * 512:(sb + 1) * 512],
                                        start=(ci == 0), stop=(ci == NCC - 1))
                                evac(dst[:, sb * 512:(sb + 1) * 512], ps[:])

                        # ---------- V projection ----------
                        for tb in range(NTC):
                            for vb in range(2):
                                ps = pp.tile([P, 512], f32, tag="pp",
                                             name=f"pv{tb}_{vb}")
                                for ci, cc in enumerate(cc_arr):
                                    nc.tensor.matmul(
                                        ps[:], xts[cc][:, tb * P:(tb + 1) * P],
                                        wvs[cc][:, vb * 512:(vb + 1) * 512],
                                        start=(ci == 0), stop=(ci == NCC - 1))
                                nc.scalar.copy(
                                    out=vsb[tb][:, vb * 512:(vb + 1) * 512],
                                    in_=ps[:])
                # wvs + pp freed

                # ---------- fused attention + next-head QK proj ----------
                with tc.tile_pool(name="scp", bufs=1, space="PSUM") as scp, \
                     tc.tile_pool(name="avp", bufs=3, space="PSUM") as avp, \
                     tc.tile_pool(name="pjp", bufs=2, space="PSUM") as pjp, \
                     tc.tile_pool(name="sup", bufs=1, space="PSUM") as sup, \
                     tc.tile_pool(name="etp", bufs=5) as etp, \
                     tc.tile_pool(name="Sp", bufs=2) as Sp, \
                     tc.tile_pool(name="Sp", bufs=2) as Sp, \
                     tc.tile_pool(name="stp", bufs=2) as stp, \
                     tc.tile_pool(name="wpp", bufs=2) as wp_pool, \
                     tc.tile_pool(name="obp", bufs=4) as obp:
                    # deferred si tails: each tail (sum-MMs, recip, broadcast,
                    # normalize) is emitted two pairs after its si completes
                    # so the in-order PE stream never blocks on the S chains.
                    gp = 0            # global pair counter
                    tails = []        # (ready_at_gp, emit_fn)
                    p3_done = set()   # (cb, tch) groups emitted early

                    def flush_tails(now):
                        while tails and tails[0][0] <= now:
                            tails.pop(0)[1]()

                    for h in range(HL):
                        qt, kt = qk_tiles[h], qk_tiles[HL + h]
                        # prefetch weights for head h+2's projection
                        if h + 2 < HL:
                            fetch_wqk(h + 2)
                            fetch_wqk(HL + h + 2)
                        # proj work units for head h+1
                        units = []
                        if h + 1 < HL:
                            for j in (h + 1, HL + h + 1):
                                new_qk_tile(j)
                                for sb in range(NSB):
                                    for cc0 in range(0, NCC, 4):
                                        units.append((j, sb, cc0))
                        if h == HL - 1:
                            # prefetch wp column-block 0 for the output
                            # projection (scalar queue is idle here)
                            wp_t = {}
                            for f in range(HL):
                                t_ = wp_pool.tile([P, 512], f16,
                                                  tag=f"wp{f}",
                                                  name=f"wp{f}_0")
                                nc.scalar.dma_start(t_[:], wp_d[f][:, 0:512])
                                wp_t[f] = t_
                        pjt = {}
                        ui = 0
                        pair_no = 0

                        evac_q = []

                        def drain_evacs():
                            for (j, sb, ps) in evac_q:
                                nc.vector.tensor_copy(
                                    out=qk_tiles[j][:, sb * 512:
                                                    (sb + 1) * 512],
                                    in_=ps[:])
                            del evac_q[:]

                        def emit_units(target):
                            # lazy evacs from the previous call: by now the
                            # group's matmuls have executed, so the DVE copy
                            # won't sit blocked at the head of the queue.
                            nonlocal ui
                            drain_evacs()
                            while ui < min(target, len(units)):
                                (j, sb, cc0) = units[ui]
                                if cc0 == 0:
                                    pjt[(j, sb)] = pjp.tile(
                                        [P, 512], f32, tag="pj",
                                        name=f"pj{j}_{sb}")
                                ps = pjt[(j, sb)]
                                for cc in range(cc0, cc0 + 4):
                                    nc.tensor.matmul(
                                        ps[:],
                                        wqk_tiles[j][:, cc * P:(cc + 1) * P],
                                        xts[cc][:, sb * 512:(sb + 1) * 512],
                                        start=(cc == 0), stop=(cc == NCC - 1))
                                if cc0 == NCC - 4:
                                    evac_q.append((j, sb, pjt.pop((j, sb))))
                                ui += 1

                        for si in range(NSB):
                            njc = 4 * si + 4
                            npr = njc // 2
                            nearly = max(0, npr - 2)
                            av_ps = avp.tile([P, 512], f32, tag="av")
                            S = Sp.tile([P, 512], f16, tag="S")
                            first_S = True
                            pend = None
                            for pr in range(npr):
                                infos = _pair_layout(si, pr)
                                sc = scp.tile([P, 1024], f32, tag="sc")
                                for (jj, dd, oo, ww) in infos:
                                    nc.tensor.matmul(
                                        sc[:, oo:oo + ww],
                                        kt[:, jj * P:(jj + 1) * P],
                                        qt[:, si * 512 + dd:(si + 1) * 512],
                                        start=True, stop=True)
                                et = etp.tile([P, 1024], f16, tag="et")
                                end = infos[-1][2] + infos[-1][3]
                                nc.scalar.activation(
                                    et[:, :end], sc[:, :end], ACT.Exp,
                                    scale=EXP_SCALE)
                                for (jj, dd, oo, ww) in infos:
                                    if jj >= 4 * si:  # diagonal 128-block
                                        nc.vector.tensor_mul(
                                            out=et[:, oo:oo + P],
                                            in0=et[:, oo:oo + P],
                                            in1=mask_sb[:])
                                # accumulate row-sums on DVE; emitted after
                                # the masks on the same in-order queue, so
                                # diagonal pairs contribute masked values.
                                for (jj, dd, oo, ww) in infos:
                                    if first_S:
                                        nc.vector.tensor_copy(
                                            out=S[:, dd:],
                                            in_=et[:, oo:oo + ww])
                                        first_S = False
                                    else:
                                        nc.vector.tensor_add(
                                            out=S[:, dd:], in0=S[:, dd:],
                                            in1=et[:, oo:oo + ww])
                                pair_no += 1
                                emit_units(2 * pair_no)
                                if pend is not None:
                                    pet, pinfos = pend
                                    for (jj, dd, oo, ww) in pinfos:
                                        nc.tensor.matmul(
                                            av_ps[:, dd:],
                                            vsb[jj][:, h * P:(h + 1) * P],
                                            pet[:, oo:oo + ww],
                                            start=(jj == 0), stop=False)
                                pend = (et, infos)
                                gp += 1
                                flush_tails(gp)
                            pet, pinfos = pend
                            for (jj, dd, oo, ww) in pinfos:
                                nc.tensor.matmul(
                                    av_ps[:, dd:],
                                    vsb[jj][:, h * P:(h + 1) * P],
                                    pet[:, oo:oo + ww],
                                    start=(jj == 0), stop=(jj == njc - 1))

                            # tail: recip -> broadcast -> normalize, each
                            # emitted with growing slack so no in-order queue
                            # ever blocks on a cross-engine dependency.
                            def make_t0(h=h, si=si, S=S, box=None):
                                def emit():
                                    sum_ps = sup.tile([1, 512], f32,
                                                      tag="sum",
                                                      name=f"sum{h}_{si}")
                                    nc.tensor.matmul(
                                        sum_ps[:], ones_sb[:], S[:],
                                        start=True, stop=True)
                                    box["sum"] = sum_ps
                                return emit

                            def make_t1(h=h, si=si, box=None):
                                def emit():
                                    rec = stp.tile([1, 512], f32, tag="rec",
                                                   name=f"rec{h}_{si}")
                                    nc.vector.reciprocal_approx_fast(
                                        out=rec[:], in_=box["sum"][:])
                                    box["rec"] = rec
                                return emit

                            def make_t2(h=h, si=si, box=None):
                                def emit():
                                    recb = stp.tile([P, 512], f32, tag="recb",
                                                    name=f"recb{h}_{si}")
                                    nc.gpsimd.partition_broadcast(
                                        recb[:], box["rec"][:])
                                    box["recb"] = recb
                                return emit

                            def make_t3(h=h, si=si, av_ps=av_ps, box=None):
                                def emit():
                                    nc.vector.tensor_mul(
                                        out=avts[h][:, si * 512:
                                                    (si + 1) * 512],
                                        in0=av_ps[:], in1=box["recb"][:])
                                return emit

                            box = {}
                            tails.append((gp + 3, make_t0(box=box)))
                            tails.append((gp + 4, make_t1(box=box)))
                            tails.append((gp + 5, make_t2(box=box)))
                            tails.append((gp + 6, make_t3(box=box)))
                            if h == HL - 1 and si < NSB - 1:
                                # head 7 has no next-head proj work; fill its
                                # slack with cb0 output-projection groups for
                                # the t-chunks this superblock just finished.
                                def make_p3(tch):
                                    def emit():
                                        ps = pjp.tile([P, 512], f32,
                                                      tag="pj",
                                                      name=f"po0_{tch}")
                                        for f in range(HL):
                                            nc.tensor.matmul(
                                                ps[:],
                                                avts[f][:, tch * P:
                                                        (tch + 1) * P],
                                                wp_t[f][:, :],
                                                start=(f == 0),
                                                stop=(f == HL - 1))
                                        ob = obp.tile([P, 512], f32,
                                                      tag="ob")
                                        nc.vector.tensor_copy(out=ob[:],
                                                              in_=ps[:])
                                        nc.sync.dma_start(
                                            out_d[tch * P:(tch + 1) * P,
                                                  0:512], ob[:])
                                    return emit
                                for k_, tch_ in enumerate(
                                        range(4 * si, 4 * si + 4)):
                                    tails.append((gp + 7 + k_,
                                                  make_p3(tch_)))
                                    p3_done.add((0, tch_))
                        emit_units(len(units))
                        drain_evacs()
                    flush_tails(10 ** 9)

                    # ---------- output projection (cb-outer, wp streamed) ---
                    gi = 0
                    for cb in range(NCB):
                        if cb + 1 < NCB:
                            wp_next = {}
                            for f in range(HL):
                                t_ = wp_pool.tile([P, 512], f16,
                                                  tag=f"wp{f}",
                                                  name=f"wp{f}_{cb + 1}")
                                nc.sync.dma_start(
                                    t_[:], wp_d[f][:, (cb + 1) * 512:
                                                   (cb + 2) * 512])
                                wp_next[f] = t_
                        for tch in range(NTC):
                            if (cb, tch) in p3_done:
                                continue
                            pool = avp if gi % 2 == 0 else pjp
                            tag = "av" if gi % 2 == 0 else "pj"
                            ps = pool.tile([P, 512], f32, tag=tag,
                                           name=f"po{cb}_{tch}")
                            gi += 1
                            for f in range(HL):
                                nc.tensor.matmul(
                                    ps[:], avts[f][:, tch * P:(tch + 1) * P],
                                    wp_t[f][:, :],
                                    start=(f == 0), stop=(f == HL - 1))
                            ob = obp.tile([P, 512], f32, tag="ob")
                            evac(ob[:], ps[:])
                            nc.sync.dma_start(
                                out_d[tch * P:(tch + 1) * P,
                                      cb * 512:(cb + 1) * 512], ob[:])
                        if cb + 1 < NCB:
                            wp_t = wp_next
    nc.compile()
    return nc


def _make_mask():
    pp_ = np.arange(P)[:, None]
    ff = np.arange(P)[None, :]
    return np.where(ff >= pp_, 1.0, 0.0).astype(np_f16)


def _prep_inputs(x, w_qkv, w_proj):
    mask = _make_mask()
    per_g = {}
    for g in range(2):
        q = w_qkv[:, g * FL:(g + 1) * FL]
        k = w_qkv[:, C + g * FL:C + (g + 1) * FL]
        v = w_qkv[:, 2 * C + g * FL:2 * C + (g + 1) * FL]
        wqk_cat = np.concatenate([q, k], axis=1)  # [C, 2048]
        wqk_p = np.ascontiguousarray(
            wqk_cat.reshape(NCC, P, 16, P).transpose(2, 1, 0, 3)
            .reshape(16, P, C)).astype(np_f16)
        wv_p = np.ascontiguousarray(v.reshape(NCC, P, FL)).astype(np_f16)
        wp_p = np.ascontiguousarray(
            w_proj[g * FL:(g + 1) * FL, :].reshape(HL, P, C)).astype(np_f16)
        per_g[g] = (wqk_p, wv_p, wp_p)
    in_maps = []
    for core in range(NCORES):
        b, g = core // 2, core % 2
        wqk_p, wv_p, wp_p = per_g[g]
        in_maps.append({
            "xt": np.ascontiguousarray(x[b].T).astype(np_f16),
            "wqk": wqk_p,
            "wv": wv_p,
            "wp": wp_p,
            "mask": mask,
        })
    return in_maps


_nc_cache = None
last_results = None  # BassKernelResults of the most recent run (for test.py)


def kernel(x, w_qkv, w_proj):
    global _nc_cache, last_results
    from concourse.bass_utils import run_bass_kernel_spmd

    x = np.asarray(x, dtype=np.float32)
    w_qkv = np.asarray(w_qkv, dtype=np.float32)
    w_proj = np.asarray(w_proj, dtype=np.float32)

    if _nc_cache is None:
        _nc_cache = build_nc()
    nc = _nc_cache

    in_maps = _prep_inputs(x, w_qkv, w_proj)
    trace = bool(int(os.environ.get("KERNEL_TRACE", "0")))
    res = run_bass_kernel_spmd(nc, in_maps, list(range(NCORES)), trace=trace)
    last_results = res

    out = np.empty((B, T, C), dtype=np.float32)
    for b in range(B):
        out[b] = res.results[2 * b]["out"] + res.results[2 * b + 1]["out"]
    return out
